# revision 2
# baseline (speedup 1.0000x reference)
"""JT-MPN GNN kernel for 8 trn2 NeuronCores (self-contained).

Two-hop dma_gather message passing: hop-1 packs needed message rows from
int16-addressable 32K-row windows of the AllGathered message table into
SBUF chunks; hop-2 re-gathers them SBUF->SBUF in consumer (bin, round,
slot) order directly in transposed (TT) layout. DVE sums rounds, PE runs
the W_h matmuls (bf16, f32 PSUM), ACT applies relu. bf16 AllGather
between the three BP iterations; graph mean-pool via a selection matmul.
"""

import numpy as np

N_NODES = 150000
N_EDGES = 300000
H = 256
DEPTH = 4
AF = 35
BF = 5
NG = 2048
CORES = 8

EPC = 37504               # edges per core (293 bins * 128)
NBINS_E = EPC // 128      # 293
GPC = NG // CORES         # 256
SUP = 2
ZR = 37500                # global msg row guaranteed zero
E_ALL = CORES * EPC       # 300032
WIN = 32768

TREE_PAD = 60416
ZT = 60000                # zero row in padded tree table

CHUNK_ROWS = 12288        # hop-1 chunk tile rows (B=96 blocks)
HOP2_MAX = 768            # transpose-mode ucode ring cap (1024 crashes)
H1_MAX = 1024
SEG_R = HOP2_MAX // 128   # max rounds per consumer segment


def _group_by(dst, n_groups):
    order = np.argsort(dst, kind="stable")
    counts = np.bincount(dst, minlength=n_groups)
    starts = np.zeros(n_groups + 1, dtype=np.int64)
    np.cumsum(counts, out=starts[1:])
    return order, starts


def wrap_idx(vals, cols):
    """[n] ints -> wrapped [128, cols] int16 (8x replicated); pad -1.
    Index i lives at [i%16, i//16]."""
    flat = np.full(cols * 16, -1, np.int16)
    flat[:len(vals)] = vals.astype(np.int16)
    w = np.ascontiguousarray(flat.reshape(cols, 16).T)
    return np.tile(w, (8, 1))


def build_2hop(src_rows, table_rows, block_sizes,
               chunk_rows=CHUNK_ROWS, hop2_max=HOP2_MAX):
    """Unified 2-hop tables for all cores.

    src_rows: [CORES, n_cons] global source row per consumer column.
    block_sizes: per atomic consumer block (bin), each multiple of 128.

    Returns dict:
      hop1_calls: list of (chunk, win_base, n_pad, col_off) ; n_valid is per
        core and encoded by -1 padding in idx (num_idxs_reg: use n_pad minus
        trailing -1 count? -> device passes per-core reg via ... ) NOTE:
        num_idxs_reg must be a compile-time constant in the unified program,
        so we pass n_pad and set padded idx entries to ZR-in-window when the
        window contains a guaranteed-zero row, else repeat the last valid
        index (harmless extra gather).
      hop1_idx: [CORES][128, C1] int16
      hop1_blocks: per chunk block count (unified)
      hop2_calls: list of (chunk, n, col_off, out_off)
      hop2_idx: [CORES][128, C2] int16
      n_chunks
    """
    n_wins = (table_rows + WIN - 1) // WIN
    nb = len(block_sizes)
    block_start = np.zeros(nb + 1, np.int64)
    np.cumsum(block_sizes, out=block_start[1:])
    n_cons = int(block_start[-1])
    assert src_rows.shape == (CORES, n_cons)

    # --- chunk assignment (unified): estimate per-core unique counts ---
    chunks = []
    cur_first = 0
    cur_rows = [set() for _ in range(CORES)]
    for b in range(nb):
        sl = slice(block_start[b], block_start[b + 1])
        newmax = 0
        for c in range(CORES):
            s = set(src_rows[c, sl].tolist())
            newmax = max(newmax, len(cur_rows[c] | s))
        if newmax > chunk_rows - 128 * n_wins and b > cur_first:
            chunks.append((cur_first, b))
            cur_first = b
            cur_rows = [set(src_rows[c, sl].tolist()) for c in range(CORES)]
        else:
            for c in range(CORES):
                cur_rows[c] |= set(src_rows[c, sl].tolist())
    chunks.append((cur_first, nb))

    hop1_calls = []
    hop1_vals = [[] for _ in range(CORES)]   # list of (colpos, array)
    hop1_blocks = []
    hop2_calls = []
    hop2_vals = [[] for _ in range(CORES)]
    c1_off = 0
    c2_off = 0
    for t, (b0, b1) in enumerate(chunks):
        sl = slice(block_start[b0], block_start[b1])
        uniqs = [np.unique(src_rows[c, sl]) for c in range(CORES)]
        poss = [np.full(len(u), -1, np.int64) for u in uniqs]
        p = 0
        for w in range(n_wins):
            wlo, whi = w * WIN, min((w + 1) * WIN, table_rows)
            sels = [(u >= wlo) & (u < whi) for u in uniqs]
            n_valid = [int(s.sum()) for s in sels]
            n_max = max(n_valid)
            if n_max == 0:
                continue
            n_pad = (n_max + 127) // 128 * 128
            for c in range(CORES):
                poss[c][sels[c]] = p + np.arange(n_valid[c])
                v = uniqs[c][sels[c]] - wlo
                if len(v) == 0:
                    v = np.array([0], np.int64)  # dummy row in window
                pad = np.full(n_pad - len(v), v[-1], np.int64)
                hop1_vals[c].append(np.concatenate([v, pad]))
            # split into sub-calls of <= H1_MAX indices (ucode ring cap)
            done = 0
            while done < n_pad:
                sub = min(H1_MAX, n_pad - done)
                hop1_calls.append((t, wlo, sub, c1_off))
                c1_off += sub // 16
                done += sub
            p += n_pad
        assert p <= chunk_rows, (p, chunk_rows)
        hop1_blocks.append(p // 128)

        h2 = []
        for c in range(CORES):
            j = np.searchsorted(uniqs[c], src_rows[c, sl])
            assert (uniqs[c][j] == src_rows[c, sl]).all()
            h2.append(poss[c][j])
            assert (poss[c][j] >= 0).all()
        # split into calls at block boundaries
        local_bs = block_start[b0:b1 + 1] - block_start[b0]
        bi = 0
        cstart = 0
        total = int(local_bs[-1])
        while cstart < total:
            cend = cstart
            while bi < b1 - b0 and local_bs[bi + 1] - cstart <= hop2_max:
                bi += 1
                cend = int(local_bs[bi])
            assert cend > cstart
            n = cend - cstart
            for c in range(CORES):
                hop2_vals[c].append(h2[c][cstart:cend])
            hop2_calls.append((t, n, c2_off, int(block_start[b0] + cstart)))
            c2_off += (n + 15) // 16
            cstart = cend

    hop1_idx = [wrap_idx(np.concatenate(hop1_vals[c]), max(c1_off, 1))
                for c in range(CORES)]
    hop2_idx = [wrap_idx(np.concatenate(hop2_vals[c]), max(c2_off, 1))
                for c in range(CORES)]
    return dict(hop1_calls=hop1_calls, hop1_idx=hop1_idx,
                hop1_blocks=hop1_blocks, hop2_calls=hop2_calls,
                hop2_idx=hop2_idx, n_chunks=len(chunks),
                c1_cols=max(c1_off, 1), c2_cols=max(c2_off, 1))


def preprocess(inputs):
    edge_src = np.asarray(inputs["edge_src"], dtype=np.int64)
    edge_dst = np.asarray(inputs["edge_dst"], dtype=np.int64)
    lg_src = np.asarray(inputs["lg_src"], dtype=np.int64)
    lg_dst = np.asarray(inputs["lg_dst"], dtype=np.int64)
    tgt_nodes = np.asarray(inputs["tgt_nodes"], dtype=np.int64)
    graph_ids = np.asarray(inputs["graph_ids"], dtype=np.int64)
    node_x = np.asarray(inputs["node_x"], dtype=np.float32)
    bond_x = np.asarray(inputs["bond_x"], dtype=np.float32)

    meta = {}

    # ---- edge -> core (snake deal by lg in-degree desc) ----
    deg = np.bincount(lg_dst, minlength=N_EDGES)
    order = np.argsort(-deg, kind="stable")
    cyc = np.arange(N_EDGES) % (2 * CORES)
    core_of_rank = np.where(cyc < CORES, cyc, 2 * CORES - 1 - cyc)
    slots = np.full((CORES, EPC), -1, dtype=np.int64)
    for c in range(CORES):
        mine = order[core_of_rank == c]
        slots[c, :len(mine)] = mine
    new_id = np.full(N_EDGES, -1, dtype=np.int64)
    for c in range(CORES):
        valid = slots[c] >= 0
        new_id[slots[c][valid]] = c * EPC + np.nonzero(valid)[0]
    assert (new_id >= 0).all()
    meta["slots"] = slots
    meta["new_id"] = new_id

    lg_order, lg_starts = _group_by(lg_dst, N_EDGES)
    slot_deg = np.where(slots >= 0, deg[np.clip(slots, 0, None)], 0)
    R_lg = slot_deg.reshape(CORES, NBINS_E, 128).max(axis=2).max(axis=0)
    meta["R_lg"] = R_lg

    # ---- nodes ----
    g_starts = np.zeros(NG + 1, dtype=np.int64)
    np.cumsum(np.bincount(graph_ids, minlength=NG), out=g_starts[1:])
    n_deg = np.bincount(edge_dst, minlength=N_NODES)
    t_cnt = np.bincount(tgt_nodes, minlength=N_NODES)
    counts_g = np.bincount(graph_ids, minlength=NG).astype(np.float64)

    sup_nodes = []
    for c in range(CORES):
        for u in range(SUP):
            g0 = c * GPC + u * 128
            nodes = np.arange(g_starts[g0], g_starts[g0 + 128])
            nodes = nodes[np.argsort(-n_deg[nodes], kind="stable")]
            sup_nodes.append(nodes)
    NBINS_N = int(max((len(x) + 127) // 128 for x in sup_nodes))
    NPS = NBINS_N * 128
    meta["NBINS_N"] = NBINS_N
    meta["NPS"] = NPS
    nslot = np.full((CORES, SUP, NPS), -1, dtype=np.int64)
    for c in range(CORES):
        for u in range(SUP):
            nodes = sup_nodes[c * SUP + u]
            nslot[c, u, :len(nodes)] = nodes
    meta["nslot"] = nslot

    BPC = SUP * NPS
    B_ALL = CORES * BPC
    beta_row_of_node = np.full(N_NODES, -1, np.int64)
    for c in range(CORES):
        sl = nslot[c].reshape(-1)
        v = sl >= 0
        beta_row_of_node[sl[v]] = c * BPC + np.nonzero(v)[0]
    assert (beta_row_of_node >= 0).all()
    meta["BPC"] = BPC
    meta["B_ALL"] = B_ALL
    pad_pos = np.nonzero(nslot[0].reshape(-1) < 0)[0]
    ZB = int(pad_pos[0]) if len(pad_pos) else 0
    meta["ZB"] = ZB

    slot_nd = np.where(nslot >= 0, n_deg[np.clip(nslot, 0, None)], 0)
    slot_nt = np.where(nslot >= 0, t_cnt[np.clip(nslot, 0, None)], 0)
    R_m = slot_nd.reshape(CORES, SUP * NBINS_N, 128).max(axis=2).max(axis=0)
    R_tn = slot_nt.reshape(CORES, SUP * NBINS_N, 128).max(axis=2).max(axis=0)
    meta["R_m"] = R_m
    meta["R_tn"] = R_tn

    e_order, e_starts = _group_by(edge_dst, N_NODES)
    t_order, t_starts = _group_by(tgt_nodes, N_NODES)

    def consumer_rows_edges(Rs, order_, starts_, src_map, zero_row, degs):
        """Build [CORES, n_cons] consumer source rows for edge bins.
        Segments of <= SEG_R rounds per bin (ucode call cap).
        bins_list entries: (bin, coloff, Rseg, first, last)."""
        bins_list = []
        blocks = []
        coff = 0
        for b in range(NBINS_E):
            R = int(Rs[b])
            if R == 0:
                continue
            r0 = 0
            while r0 < R:
                rs = min(SEG_R, R - r0)
                bins_list.append((b, coff, rs, r0 == 0, r0 + rs == R))
                blocks.append(rs * 128)
                coff += rs * 128
                r0 += rs
        n_cons = coff
        rows = np.full((CORES, n_cons), zero_row, np.int64)
        seg_round0 = {}
        r_run = {}
        for (b, co, rs, first, last) in bins_list:
            if first:
                r_run[b] = 0
            seg_round0[(b, co)] = r_run[b]
            r_run[b] += rs
        for c in range(CORES):
            for (b, co, rs, first, last) in bins_list:
                r0 = seg_round0[(b, co)]
                sl = slots[c, b*128:(b+1)*128]
                blk = np.full((rs, 128), zero_row, np.int64)
                for s in range(128):
                    e = sl[s]
                    if e < 0:
                        continue
                    d = int(degs[e])
                    lo, hi = min(r0, d), min(r0 + rs, d)
                    if hi <= lo:
                        continue
                    js = order_[starts_[e] + lo:starts_[e] + hi]
                    blk[:hi - lo, s] = src_map(js)
                rows[c, co:co + rs * 128] = blk.reshape(-1)
        return rows, blocks, bins_list

    # ---- lg ----
    rows_lg, blocks_lg, bins_lg = consumer_rows_edges(
        R_lg, lg_order, lg_starts, lambda js: new_id[lg_src[js]], ZR, deg)
    meta["lg"] = build_2hop(rows_lg, E_ALL, blocks_lg)
    meta["lg_bins"] = bins_lg
    meta["lg_zero_bins"] = [b for b in range(NBINS_E) if R_lg[b] == 0]

    # ---- beta-gather: 1 round per edge bin ----
    rows_bg = np.full((CORES, EPC), ZB, np.int64)
    for c in range(CORES):
        v = slots[c] >= 0
        rows_bg[c, v] = beta_row_of_node[edge_src[slots[c][v]]]
    meta["bg"] = build_2hop(rows_bg, B_ALL, [128] * NBINS_E)

    def consumer_rows_nodes(Rs, order_, starts_, src_map, zero_row, degs):
        """bins_list entries: (u, b, coloff, Rseg, first, last)."""
        bins_list = []
        blocks = []
        coff = 0
        for ub in range(SUP * NBINS_N):
            R = int(Rs[ub])
            if R == 0:
                continue
            r0 = 0
            while r0 < R:
                rs = min(SEG_R, R - r0)
                bins_list.append((ub // NBINS_N, ub % NBINS_N, coff, rs,
                                  r0 == 0, r0 + rs == R))
                blocks.append(rs * 128)
                coff += rs * 128
                r0 += rs
        n_cons = coff
        rows = np.full((CORES, n_cons), zero_row, np.int64)
        seg_round0 = {}
        r_run = {}
        for (u, b, co, rs, first, last) in bins_list:
            if first:
                r_run[(u, b)] = 0
            seg_round0[co] = r_run[(u, b)]
            r_run[(u, b)] += rs
        for c in range(CORES):
            for (u, b, co, rs, first, last) in bins_list:
                r0 = seg_round0[co]
                sl = nslot[c, u, b*128:(b+1)*128]
                blk = np.full((rs, 128), zero_row, np.int64)
                for s in range(128):
                    vtx = sl[s]
                    if vtx < 0:
                        continue
                    d = int(degs[vtx])
                    lo, hi = min(r0, d), min(r0 + rs, d)
                    if hi <= lo:
                        continue
                    js = order_[starts_[vtx] + lo:starts_[vtx] + hi]
                    blk[:hi - lo, s] = src_map(js)
                rows[c, co:co + rs * 128] = blk.reshape(-1)
        return rows, blocks, bins_list

    # ---- tree ----
    rows_tr, blocks_tr, bins_tr = consumer_rows_nodes(
        R_tn, t_order, t_starts, lambda js: js, ZT, t_cnt)
    meta["tr"] = build_2hop(rows_tr, TREE_PAD, blocks_tr)
    meta["tr_bins"] = bins_tr
    meta["tr_zero_bins"] = [(ub // NBINS_N, ub % NBINS_N)
                            for ub in range(SUP * NBINS_N) if R_tn[ub] == 0]

    # ---- m ----
    rows_m, blocks_m, bins_m = consumer_rows_nodes(
        R_m, e_order, e_starts, lambda js: new_id[js], ZR, n_deg)
    meta["m"] = build_2hop(rows_m, E_ALL, blocks_m)
    meta["m_bins"] = bins_m
    meta["m_zero_bins"] = [(ub // NBINS_N, ub % NBINS_N)
                           for ub in range(SUP * NBINS_N) if R_m[ub] == 0]

    # ---- per-core float layouts (pure permutations of inputs) ----
    per_core = []
    inv_cnt = (1.0 / np.maximum(counts_g, 1.0)).astype(np.float32)
    for c in range(CORES):
        pc = {}
        featT = np.zeros((AF + BF, EPC), np.float32)
        v = slots[c] >= 0
        featT[:AF, v] = node_x[edge_src[slots[c][v]]].T
        featT[AF:, v] = bond_x[slots[c][v]].T
        pc["featT"] = featT

        featTn = np.zeros((AF + 1, SUP * NPS), np.float32)
        spool = np.zeros((SUP * NPS, 128), np.float32)
        for u in range(SUP):
            sl = nslot[c, u]
            vv = sl >= 0
            base = u * NPS
            featTn[:AF, base:base + NPS][:, vv] = node_x[sl[vv]].T
            featTn[AF, base:base + NPS][vv] = 1.0
            gl = graph_ids[np.clip(sl, 0, None)] - (c * GPC + u * 128)
            idxs = np.nonzero(vv)[0]
            spool[base + idxs, gl[idxs]] = inv_cnt[graph_ids[sl[idxs]]]
        pc["featTn"] = featTn
        pc["spool"] = spool
        pc["lg_h1"] = meta["lg"]["hop1_idx"][c]
        pc["lg_h2"] = meta["lg"]["hop2_idx"][c]
        pc["bg_h1"] = meta["bg"]["hop1_idx"][c]
        pc["bg_h2"] = meta["bg"]["hop2_idx"][c]
        pc["tr_h1"] = meta["tr"]["hop1_idx"][c]
        pc["tr_h2"] = meta["tr"]["hop2_idx"][c]
        pc["m_h1"] = meta["m"]["hop1_idx"][c]
        pc["m_h2"] = meta["m"]["hop2_idx"][c]
        per_core.append(pc)

    return per_core, meta


"""JT-MPN GNN kernel v2: 2-hop dma_gather message passing on 8 trn2 cores.

Per iteration: hop-1 window dma_gathers pack needed msg rows into SBUF
chunks (int16 indices), hop-2 SBUF-source transpose dma_gather re-reads
them in consumer (bin, round, slot) order directly in TT layout; DVE sums
rounds; PE does the W_h matmuls; DVE adds input2; ACT applies relu.
AllGather (bf16) between iterations. All float math on device.
"""
import concourse.bacc as bacc
import concourse.bass as bass
import concourse.mybir as mybir
import concourse.tile as tile

F32 = mybir.dt.float32
BF16 = mybir.dt.bfloat16
I16 = mybir.dt.int16
AluOp = mybir.AluOpType
Act = mybir.ActivationFunctionType


def build(meta, stub_collectives=False, n_iters=DEPTH - 1):
    NBINS_N = meta["NBINS_N"]
    NPS = meta["NPS"]
    BPC = meta["BPC"]
    B_ALL = meta["B_ALL"]
    lg, bg, tr, m = meta["lg"], meta["bg"], meta["tr"], meta["m"]
    SW1 = max(bg["c1_cols"], tr["c1_cols"], m["c1_cols"])
    SW2 = max(bg["c2_cols"], tr["c2_cols"], m["c2_cols"])

    nc = bacc.Bacc("TRN2", target_bir_lowering=False, debug=False)

    # ---- external IO ----
    featT_d = nc.dram_tensor("featT", [AF + BF, EPC], F32, kind="ExternalInput")
    featTn_d = nc.dram_tensor("featTn", [AF + 1, SUP * NPS], F32, kind="ExternalInput")
    spool_d = nc.dram_tensor("spool", [SUP * NPS, 128], F32, kind="ExternalInput")
    tree_d = nc.dram_tensor("tree_bf", [TREE_PAD, H], BF16, kind="ExternalInput")
    Wi_d = nc.dram_tensor("Wi", [AF + BF, H], F32, kind="ExternalInput")
    Wh_d = nc.dram_tensor("Wh_bf", [H, H], BF16, kind="ExternalInput")
    WoT_d = nc.dram_tensor("WoTop", [AF + 1, H], F32, kind="ExternalInput")
    Wob_d = nc.dram_tensor("Wob_bf", [H, H], BF16, kind="ExternalInput")
    identb_d = nc.dram_tensor("ident_bf", [128, 128], BF16, kind="ExternalInput")
    idx_d = {}
    for nm, tab in (("lg", lg), ("bg", bg), ("tr", tr), ("m", m)):
        idx_d[nm + "_h1"] = nc.dram_tensor(nm + "_h1", [128, tab["c1_cols"]], I16,
                                           kind="ExternalInput")
        idx_d[nm + "_h2"] = nc.dram_tensor(nm + "_h2", [128, tab["c2_cols"]], I16,
                                           kind="ExternalInput")
    gout = nc.dram_tensor("gout", [GPC, H], F32, kind="ExternalOutput")

    with tile.TileContext(nc) as tc:
        with tc.tile_pool(name="dram", bufs=1, space="DRAM") as dram, \
             tc.tile_pool(name="const", bufs=1) as cpool, \
             tc.tile_pool(name="idxp", bufs=1) as idxp, \
             tc.tile_pool(name="chunk", bufs=2) as chp, \
             tc.tile_pool(name="tt", bufs=2) as ttp, \
             tc.tile_pool(name="stream", bufs=2) as stp, \
             tc.tile_pool(name="scratch", bufs=2) as scp, \
             tc.tile_pool(name="psum_b", bufs=3, space="PSUM") as ppb, \
             tc.tile_pool(name="psum_g", bufs=1, space="PSUM") as ppg:

            beta_shard = dram.tile([BPC, H], BF16)
            beta_full = dram.tile([B_ALL, H], BF16,
                                  addr_space=("Local" if stub_collectives else "Shared"))
            in2_d = dram.tile([EPC, H], BF16)
            gamma_d = dram.tile([SUP * NPS, H], BF16)
            msg_shard = dram.tile([EPC, H], BF16)
            n_ags = 1 + n_iters
            msg_fulls = [dram.tile([E_ALL, H], BF16,
                                   addr_space=("Local" if stub_collectives else "Shared"),
                                   name=f"msg_full_{k}") for k in range(n_ags)]

            # ---- constants ----
            wi_sb = cpool.tile([AF + BF, H], F32)
            nc.sync.dma_start(wi_sb[:], Wi_d[:])
            whA = cpool.tile([128, H], BF16)
            whB = cpool.tile([128, H], BF16)
            nc.sync.dma_start(whA[:], Wh_d[0:128, :])
            nc.sync.dma_start(whB[:], Wh_d[128:256, :])
            wot = cpool.tile([AF + 1, H], F32)
            nc.sync.dma_start(wot[:], WoT_d[:])
            wobA = cpool.tile([128, H], BF16)
            wobB = cpool.tile([128, H], BF16)
            nc.sync.dma_start(wobA[:], Wob_d[0:128, :])
            nc.sync.dma_start(wobB[:], Wob_d[128:256, :])
            idb = cpool.tile([128, 128], BF16)
            nc.sync.dma_start(idb[:], identb_d[:])

            lg_h1 = idxp.tile([128, lg["c1_cols"]], I16)
            nc.sync.dma_start(lg_h1[:], idx_d["lg_h1"][:])
            lg_h2 = idxp.tile([128, lg["c2_cols"]], I16)
            nc.sync.dma_start(lg_h2[:], idx_d["lg_h2"][:])

            def load_idx(nm, tab):
                h1 = idxp.tile([128, SW1], I16, tag="sw1")
                nc.sync.dma_start(h1[:, :tab["c1_cols"]], idx_d[nm + "_h1"][:])
                h2 = idxp.tile([128, SW2], I16, tag="sw2")
                nc.sync.dma_start(h2[:, :tab["c2_cols"]], idx_d[nm + "_h2"][:])
                return h1, h2

            def hop1_chunks(tab, h1, table_dram, table_rows):
                calls_by_chunk = {}
                for (t, wlo, n_pad, c1off) in tab["hop1_calls"]:
                    calls_by_chunk.setdefault(t, []).append((wlo, n_pad, c1off))
                for t in range(tab["n_chunks"]):
                    B = tab["hop1_blocks"][t]
                    ct = chp.tile([128, CHUNK_ROWS // 128, H], BF16, tag="ct")
                    o = 0
                    for (wlo, n_pad, c1off) in calls_by_chunk[t]:
                        wlen = min(WIN, table_rows - wlo)
                        nb = n_pad // 128
                        nc.gpsimd.dma_gather(
                            out_ap=ct[:, o:o + nb, :],
                            in_ap=table_dram[wlo:wlo + wlen, :],
                            idxs_ap=h1[:, c1off:c1off + n_pad // 16],
                            num_idxs=n_pad, num_idxs_reg=n_pad, elem_size=H)
                        o += nb
                    assert o == B
                    yield t, ct, B

            def hop2(h2, ct, call):
                (t, n, c2off, outoff) = call
                flat = ttp.tile([128, 2 * HOP2_MAX], BF16, tag="tt")
                ttt = flat[:, 0:2 * n].rearrange("p (k n) -> p k n", k=2)
                nc.gpsimd.dma_gather(
                    out_ap=ttt,
                    in_ap=ct[:],
                    idxs_ap=h2[:, c2off:c2off + n // 16],
                    num_idxs=n, num_idxs_reg=n, elem_size=H,
                    transpose=True,
                    sbuf_tokens_per_rank=128,
                    sbuf_free_dim_per_rank=H * 2)
                return ttt

            def reduce_rounds(ttt, c0, R):
                acc = ttt[:, :, c0:c0 + 128]
                for r in range(1, R):
                    nc.vector.tensor_tensor(
                        out=acc, in0=acc,
                        in1=ttt[:, :, c0 + r * 128:c0 + (r + 1) * 128],
                        op=AluOp.add)
                return acc

            def allgather(src, dst, shard_rows):
                if stub_collectives:
                    for rep in range(2):
                        lo = (rep * shard_rows) % max(dst.shape[0] - shard_rows, 1) \
                            if dst.shape[0] > shard_rows else 0
                        nc.sync.dma_start(dst[lo:lo + shard_rows, :], src[:])
                    return
                nc.gpsimd.collective_compute(
                    "AllGather", AluOp.bypass,
                    replica_groups=[list(range(CORES))],
                    ins=[src[:].opt()], outs=[dst[:].opt()])

            # ================= phase 0a: msg0 + AG0 =================
            for g0 in range(0, NBINS_E, 8):
                gsz = min(8, NBINS_E - g0)
                ft = stp.tile([AF + BF, 8 * 128], F32, tag="ft")
                nc.sync.dma_start(ft[:, :gsz * 128],
                                  featT_d[:, g0 * 128:(g0 + gsz) * 128])
                msgb = stp.tile([128, 8, H], BF16, tag="msgb")
                for bi in range(gsz):
                    pa = ppb.tile([128, H], F32, tag="ps")
                    nc.tensor.matmul(pa[:], lhsT=ft[:, bi * 128:(bi + 1) * 128],
                                     rhs=wi_sb[:], start=True, stop=True)
                    nc.scalar.activation(msgb[:, bi, :], pa[:], Act.Relu)
                nc.sync.dma_start(
                    msg_shard[g0 * 128:(g0 + gsz) * 128, :]
                    .rearrange("(m p) d -> p m d", p=128), msgb[:, :gsz, :])
            allgather(msg_shard, msg_fulls[0], EPC)

            # ================= phase 0b: tree -> beta/gamma =================
            tr_h1, tr_h2 = load_idx("tr", tr)
            tr_calls = tr["hop2_calls"]
            zgb = scp.tile([128, H], BF16, tag="zgb")
            nc.vector.memset(zgb[:], 0.0)
            for (u, b) in meta["tr_zero_bins"]:
                base = u * NPS + b * 128
                nc.sync.dma_start(beta_shard[base:base + 128, :], zgb[:])
                nc.sync.dma_start(gamma_d[base:base + 128, :], zgb[:])
            tr_chunks = hop1_chunks(tr, tr_h1, tree_d, TREE_PAD)
            cur = {"t": -1, "ct": None, "tt": None, "rng": (0, 0), "ci": 0}

            def advance_to(tab, calls, h2, chunks_iter, coff):
                """Ensure the hop-2 call containing coff is current."""
                while not (cur["rng"][0] <= coff < cur["rng"][1]):
                    call = calls[cur["ci"]]
                    while cur["t"] < call[0]:
                        t_, ct_, B_ = next(chunks_iter)
                        cur["t"] = t_
                        cur["ct"] = ct_
                    cur["tt"] = hop2(h2, cur["ct"], call)
                    cur["rng"] = (call[3], call[3] + call[1])
                    cur["ci"] += 1
                return cur["tt"], cur["rng"][0]

            acc_hold = {}
            for (u, b, coff, R, first, last) in meta["tr_bins"]:
                ttt, o0 = advance_to(tr, tr_calls, tr_h2, tr_chunks, coff)
                acc = reduce_rounds(ttt, coff - o0, R)
                if not first:
                    nc.vector.tensor_tensor(out=acc_hold[(u, b)],
                                            in0=acc_hold[(u, b)], in1=acc,
                                            op=AluOp.add)
                else:
                    acc_hold[(u, b)] = acc
                if not last:
                    continue
                acc = acc_hold.pop((u, b))
                base = u * NPS + b * 128
                pb_ = ppb.tile([128, H], F32, tag="ps")
                nc.tensor.matmul(pb_[:], lhsT=acc[:, 0, :], rhs=whA[:],
                                 start=True, stop=False)
                nc.tensor.matmul(pb_[:], lhsT=acc[:, 1, :], rhs=whB[:],
                                 start=False, stop=True)
                bout = scp.tile([128, H], BF16, tag="bout")
                nc.scalar.activation(bout[:], pb_[:], Act.Copy)
                nc.sync.dma_start(beta_shard[base:base + 128, :], bout[:])
                pg_ = ppb.tile([128, H], F32, tag="ps")
                nc.tensor.matmul(pg_[:], lhsT=acc[:, 0, :], rhs=wobA[:],
                                 start=True, stop=False)
                nc.tensor.matmul(pg_[:], lhsT=acc[:, 1, :], rhs=wobB[:],
                                 start=False, stop=True)
                gt = scp.tile([128, H], BF16, tag="gt")
                nc.scalar.activation(gt[:], pg_[:], Act.Copy)
                nc.sync.dma_start(gamma_d[base:base + 128, :], gt[:])
            for _ in tr_chunks:
                pass
            allgather(beta_shard, beta_full, BPC)

            # ================= phase 0c: input2 =================
            bg_h1, bg_h2 = load_idx("bg", bg)
            bg_calls = bg["hop2_calls"]
            ci = 0
            for t, ct, B in hop1_chunks(bg, bg_h1, beta_full, B_ALL):
                while ci < len(bg_calls) and bg_calls[ci][0] == t:
                    call = bg_calls[ci]
                    (tt_, n, c2off, outoff) = call
                    ttt = hop2(bg_h2, ct, call)
                    nb = n // 128
                    for j0 in range(0, nb, 8):
                        jn = min(8, nb - j0)
                        lo = outoff + j0 * 128
                        ft = stp.tile([AF + BF, 8 * 128], F32, tag="ft")
                        nc.sync.dma_start(ft[:, :jn * 128],
                                          featT_d[:, lo:lo + jn * 128])
                        i2b = stp.tile([128, 8, H], BF16, tag="msgb")
                        for j in range(jn):
                            jj = j0 + j
                            pt = ppb.tile([128, H], BF16, tag="pt")
                            nc.tensor.transpose(
                                pt[:, 0:128], ttt[:, 0, jj * 128:(jj + 1) * 128], idb[:])
                            nc.tensor.transpose(
                                pt[:, 128:256], ttt[:, 1, jj * 128:(jj + 1) * 128], idb[:])
                            brow = scp.tile([128, H], BF16, tag="brow")
                            nc.scalar.activation(brow[:], pt[:], Act.Copy)
                            pa = ppb.tile([128, H], F32, tag="ps")
                            nc.tensor.matmul(pa[:], lhsT=ft[:, j * 128:(j + 1) * 128],
                                             rhs=wi_sb[:], start=True, stop=True)
                            nc.vector.tensor_tensor(out=i2b[:, j, :], in0=pa[:],
                                                    in1=brow[:], op=AluOp.add)
                        nc.sync.dma_start(
                            in2_d[lo:lo + jn * 128, :]
                            .rearrange("(m p) d -> p m d", p=128), i2b[:, :jn, :])
                    ci += 1

            # ================= BP iterations =================
            lg_calls = lg["hop2_calls"]
            lg_bin_list = meta["lg_bins"]   # (b, coff, R)
            nz_bins = len(lg_bin_list)
            for it in range(n_iters):
                src_full = msg_fulls[it]
                ci = 0
                for t, ct, B in hop1_chunks(lg, lg_h1, src_full, E_ALL):
                    while ci < len(lg_calls) and lg_calls[ci][0] == t:
                        call = lg_calls[ci]
                        (tt_, n, c2off, outoff) = call
                        ttt = hop2(lg_h2, ct, call)
                        bins_in = [x for x in lg_bin_list
                                   if outoff <= x[1] < outoff + n]
                        for j0 in range(0, len(bins_in), 8):
                            sub = bins_in[j0:j0 + 8]
                            b_first = sub[0][0]
                            jn = len(sub)
                            i2l = stp.tile([128, 8, H], BF16, tag="i2l")
                            nc.sync.dma_start(
                                i2l[:, :jn, :],
                                in2_d[b_first * 128:(b_first + jn) * 128, :]
                                .rearrange("(m p) d -> p m d", p=128))
                            msgb = stp.tile([128, 8, H], BF16, tag="msgb")
                            for j, (b, coff, R) in enumerate(sub):
                                assert b == b_first + j
                                acc = reduce_rounds(ttt, coff - outoff, R)
                                pb_ = ppb.tile([128, H], F32, tag="ps")
                                nc.tensor.matmul(pb_[:], lhsT=acc[:, 0, :],
                                                 rhs=whA[:], start=True, stop=False)
                                nc.tensor.matmul(pb_[:], lhsT=acc[:, 1, :],
                                                 rhs=whB[:], start=False, stop=True)
                                tmp = scp.tile([128, H], BF16, tag="tmp")
                                nc.vector.tensor_tensor(out=tmp[:], in0=pb_[:],
                                                        in1=i2l[:, j, :], op=AluOp.add)
                                nc.scalar.activation(msgb[:, j, :], tmp[:], Act.Relu)
                            nc.sync.dma_start(
                                msg_shard[b_first * 128:(b_first + jn) * 128, :]
                                .rearrange("(m p) d -> p m d", p=128), msgb[:, :jn, :])
                        ci += 1
                # zero-R tail bins: msg = relu(in2)
                for g0 in range(nz_bins, NBINS_E, 8):
                    gsz = min(8, NBINS_E - g0)
                    i2l = stp.tile([128, 8, H], BF16, tag="i2l")
                    nc.sync.dma_start(
                        i2l[:, :gsz, :],
                        in2_d[g0 * 128:(g0 + gsz) * 128, :]
                        .rearrange("(m p) d -> p m d", p=128))
                    msgb = stp.tile([128, 8, H], BF16, tag="msgb")
                    for j in range(gsz):
                        nc.scalar.activation(msgb[:, j, :], i2l[:, j, :], Act.Relu)
                    nc.sync.dma_start(
                        msg_shard[g0 * 128:(g0 + gsz) * 128, :]
                        .rearrange("(m p) d -> p m d", p=128), msgb[:, :gsz, :])
                allgather(msg_shard, msg_fulls[it + 1], EPC)

            # ================= final =================
            m_h1, m_h2 = load_idx("m", m)
            m_calls = m["hop2_calls"]
            m_bin_map = {(u, b): (coff, R) for (u, b, coff, R) in meta["m_bins"]}
            m_tiles = {}
            for t, ct, B in hop1_chunks(m, m_h1, msg_fulls[n_iters], E_ALL):
                m_tiles[t] = ct
            ci = 0
            cur_tt = None
            cur_range = (0, 0)
            for u in range(SUP):
                pg = ppg.tile([128, H], F32, tag="pg")
                for b in range(NBINS_N):
                    base = u * NPS + b * 128
                    ftn = scp.tile([AF + 1, 128], F32, tag="ftn")
                    nc.sync.dma_start(ftn[:], featTn_d[:, base:base + 128])
                    pc_ = ppb.tile([128, H], F32, tag="ps")
                    if (u, b) in m_bin_map:
                        coff, R = m_bin_map[(u, b)]
                        if not (cur_range[0] <= coff < cur_range[1]):
                            call = m_calls[ci]
                            assert call[3] == coff, (call, coff)
                            cur_tt = hop2(m_h2, m_tiles[call[0]], call)
                            cur_range = (call[3], call[3] + call[1])
                            ci += 1
                        acc = reduce_rounds(cur_tt, coff - cur_range[0], R)
                        nc.tensor.matmul(pc_[:], lhsT=ftn[:], rhs=wot[:],
                                         start=True, stop=False)
                        nc.tensor.matmul(pc_[:], lhsT=acc[:, 0, :], rhs=wobA[:],
                                         start=False, stop=False)
                        nc.tensor.matmul(pc_[:], lhsT=acc[:, 1, :], rhs=wobB[:],
                                         start=False, stop=True)
                    else:
                        nc.tensor.matmul(pc_[:], lhsT=ftn[:], rhs=wot[:],
                                         start=True, stop=True)
                    gml = scp.tile([128, H], BF16, tag="gml")
                    nc.sync.dma_start(gml[:], gamma_d[base:base + 128, :])
                    hsum = scp.tile([128, H], F32, tag="hsum")
                    nc.vector.tensor_tensor(out=hsum[:], in0=pc_[:], in1=gml[:],
                                            op=AluOp.add)
                    h = scp.tile([128, H], F32, tag="h")
                    nc.scalar.activation(h[:], hsum[:], Act.Relu)
                    sp = scp.tile([128, 128], F32, tag="sp")
                    nc.sync.dma_start(sp[:], spool_d[base:base + 128, :])
                    nc.tensor.matmul(pg[:], lhsT=sp[:], rhs=h[:],
                                     start=(b == 0), stop=(b == NBINS_N - 1))
                go = scp.tile([128, H], F32, tag="go")
                nc.scalar.activation(go[:], pg[:], Act.Copy)
                nc.sync.dma_start(gout[u * 128:(u + 1) * 128, :], go[:])

    nc.finalize()
    return nc


def make_in_maps(inputs, per_core, meta):
    import ml_dtypes
    W_i = np.asarray(inputs["W_i"], np.float32)
    W_h = np.asarray(inputs["W_h"], np.float32)
    W_o = np.asarray(inputs["W_o"], np.float32)
    b_o = np.asarray(inputs["b_o"], np.float32)
    tree_pad = np.zeros((TREE_PAD, H), ml_dtypes.bfloat16)
    tree_pad[:60000] = np.asarray(inputs["tree_mess"], np.float32
                                  ).astype(ml_dtypes.bfloat16)
    shared = {
        "tree_bf": tree_pad,
        "Wi": W_i,
        "Wh_bf": W_h.astype(ml_dtypes.bfloat16),
        "WoTop": np.concatenate([W_o[:AF], b_o[None, :]], 0),
        "Wob_bf": W_o[AF:].astype(ml_dtypes.bfloat16),
        "ident_bf": np.eye(128).astype(ml_dtypes.bfloat16),
    }
    maps = []
    for c in range(CORES):
        pc = per_core[c]
        mp = dict(shared)
        mp["featT"] = pc["featT"]
        mp["featTn"] = pc["featTn"]
        mp["spool"] = pc["spool"]
        for nm in ("lg", "bg", "tr", "m"):
            mp[nm + "_h1"] = pc[nm + "_h1"]
            mp[nm + "_h2"] = pc[nm + "_h2"]
        maps.append({k: np.ascontiguousarray(v) for k, v in mp.items()})
    return maps


_BUILD_CACHE = {}


def kernel(**inputs):
    from concourse import bass_utils
    per_core, meta = preprocess(inputs)
    key = (meta["lg"]["c1_cols"], meta["lg"]["c2_cols"], meta["m"]["c1_cols"],
           meta["bg"]["c1_cols"], meta["tr"]["c1_cols"], meta["NBINS_N"])
    nc = _BUILD_CACHE.get(key)
    if nc is None:
        nc = build(meta)
        _BUILD_CACHE[key] = nc
    in_maps = make_in_maps(inputs, per_core, meta)
    res = bass_utils.run_bass_kernel_spmd(nc, in_maps, core_ids=list(range(CORES)))
    out = np.concatenate([res.results[c]["gout"] for c in range(CORES)], axis=0)
    return out.astype(np.float32)


# revision 5
# speedup vs baseline: 1.0357x; 1.0357x over previous
"""JT-MPN GNN kernel for 8 trn2 NeuronCores (self-contained).

Two-hop dma_gather message passing: hop-1 packs needed message rows from
int16-addressable 32K-row windows of the AllGathered message table into
SBUF chunks; hop-2 re-gathers them SBUF->SBUF in consumer (bin, round,
slot) order directly in transposed (TT) layout. DVE sums rounds, PE runs
the W_h matmuls (bf16, f32 PSUM), ACT applies relu. bf16 AllGather
between the three BP iterations; graph mean-pool via a selection matmul.
"""

import numpy as np

N_NODES = 150000
N_EDGES = 300000
H = 256
DEPTH = 4
AF = 35
BF = 5
NG = 2048
CORES = 8

EPC = 37504               # edges per core (293 bins * 128)
NBINS_E = EPC // 128      # 293
GPC = NG // CORES         # 256
SUP = 2
ZR = 37500                # global msg row guaranteed zero
E_ALL = CORES * EPC       # 300032
WIN = 32768

TREE_PAD = 60416
ZT = 60000                # zero row in padded tree table

CHUNK_ROWS = 12288        # hop-1 chunk tile rows (B=96 blocks)
HOP2_MAX = 768            # transpose-mode ucode ring cap (1024 crashes)
H1_MAX = 1024
SEG_R = HOP2_MAX // 128   # max rounds per consumer segment


def _group_by(dst, n_groups):
    order = np.argsort(dst, kind="stable")
    counts = np.bincount(dst, minlength=n_groups)
    starts = np.zeros(n_groups + 1, dtype=np.int64)
    np.cumsum(counts, out=starts[1:])
    return order, starts


def wrap_idx(vals, cols):
    """[n] ints -> wrapped [128, cols] int16 (8x replicated); pad -1.
    Index i lives at [i%16, i//16]."""
    flat = np.full(cols * 16, -1, np.int16)
    flat[:len(vals)] = vals.astype(np.int16)
    w = np.ascontiguousarray(flat.reshape(cols, 16).T)
    return np.tile(w, (8, 1))


def build_2hop(src_rows, table_rows, block_sizes,
               chunk_rows=CHUNK_ROWS, hop2_max=HOP2_MAX):
    """Unified 2-hop tables for all cores.

    src_rows: [CORES, n_cons] global source row per consumer column.
    block_sizes: per atomic consumer block (bin), each multiple of 128.

    Returns dict:
      hop1_calls: list of (chunk, win_base, n_pad, col_off) ; n_valid is per
        core and encoded by -1 padding in idx (num_idxs_reg: use n_pad minus
        trailing -1 count? -> device passes per-core reg via ... ) NOTE:
        num_idxs_reg must be a compile-time constant in the unified program,
        so we pass n_pad and set padded idx entries to ZR-in-window when the
        window contains a guaranteed-zero row, else repeat the last valid
        index (harmless extra gather).
      hop1_idx: [CORES][128, C1] int16
      hop1_blocks: per chunk block count (unified)
      hop2_calls: list of (chunk, n, col_off, out_off)
      hop2_idx: [CORES][128, C2] int16
      n_chunks
    """
    n_wins = (table_rows + WIN - 1) // WIN
    nb = len(block_sizes)
    block_start = np.zeros(nb + 1, np.int64)
    np.cumsum(block_sizes, out=block_start[1:])
    n_cons = int(block_start[-1])
    assert src_rows.shape == (CORES, n_cons)

    # --- chunk assignment (unified): estimate per-core unique counts ---
    chunks = []
    cur_first = 0
    cur_rows = [set() for _ in range(CORES)]
    for b in range(nb):
        sl = slice(block_start[b], block_start[b + 1])
        newmax = 0
        for c in range(CORES):
            s = set(src_rows[c, sl].tolist())
            newmax = max(newmax, len(cur_rows[c] | s))
        if newmax > chunk_rows - 128 * n_wins and b > cur_first:
            chunks.append((cur_first, b))
            cur_first = b
            cur_rows = [set(src_rows[c, sl].tolist()) for c in range(CORES)]
        else:
            for c in range(CORES):
                cur_rows[c] |= set(src_rows[c, sl].tolist())
    chunks.append((cur_first, nb))

    hop1_calls = []
    hop1_vals = [[] for _ in range(CORES)]   # list of (colpos, array)
    hop1_blocks = []
    hop2_calls = []
    hop2_vals = [[] for _ in range(CORES)]
    c1_off = 0
    c2_off = 0
    for t, (b0, b1) in enumerate(chunks):
        sl = slice(block_start[b0], block_start[b1])
        uniqs = [np.unique(src_rows[c, sl]) for c in range(CORES)]
        poss = [np.full(len(u), -1, np.int64) for u in uniqs]
        p = 0
        for w in range(n_wins):
            wlo, whi = w * WIN, min((w + 1) * WIN, table_rows)
            sels = [(u >= wlo) & (u < whi) for u in uniqs]
            n_valid = [int(s.sum()) for s in sels]
            n_max = max(n_valid)
            if n_max == 0:
                continue
            n_pad = (n_max + 127) // 128 * 128
            for c in range(CORES):
                poss[c][sels[c]] = p + np.arange(n_valid[c])
                v = uniqs[c][sels[c]] - wlo
                if len(v) == 0:
                    v = np.array([0], np.int64)  # dummy row in window
                pad = np.full(n_pad - len(v), v[-1], np.int64)
                hop1_vals[c].append(np.concatenate([v, pad]))
            # split into sub-calls of <= H1_MAX indices (ucode ring cap)
            done = 0
            while done < n_pad:
                sub = min(H1_MAX, n_pad - done)
                hop1_calls.append((t, wlo, sub, c1_off))
                c1_off += sub // 16
                done += sub
            p += n_pad
        assert p <= chunk_rows, (p, chunk_rows)
        hop1_blocks.append(p // 128)

        h2 = []
        for c in range(CORES):
            j = np.searchsorted(uniqs[c], src_rows[c, sl])
            assert (uniqs[c][j] == src_rows[c, sl]).all()
            h2.append(poss[c][j])
            assert (poss[c][j] >= 0).all()
        # split into calls at block boundaries
        local_bs = block_start[b0:b1 + 1] - block_start[b0]
        bi = 0
        cstart = 0
        total = int(local_bs[-1])
        while cstart < total:
            cend = cstart
            while bi < b1 - b0 and local_bs[bi + 1] - cstart <= hop2_max:
                bi += 1
                cend = int(local_bs[bi])
            assert cend > cstart
            n = cend - cstart
            for c in range(CORES):
                hop2_vals[c].append(h2[c][cstart:cend])
            hop2_calls.append((t, n, c2_off, int(block_start[b0] + cstart)))
            c2_off += (n + 15) // 16
            cstart = cend

    hop1_idx = [wrap_idx(np.concatenate(hop1_vals[c]), max(c1_off, 1))
                for c in range(CORES)]
    hop2_idx = [wrap_idx(np.concatenate(hop2_vals[c]), max(c2_off, 1))
                for c in range(CORES)]
    return dict(hop1_calls=hop1_calls, hop1_idx=hop1_idx,
                hop1_blocks=hop1_blocks, hop2_calls=hop2_calls,
                hop2_idx=hop2_idx, n_chunks=len(chunks),
                c1_cols=max(c1_off, 1), c2_cols=max(c2_off, 1))


def preprocess(inputs):
    edge_src = np.asarray(inputs["edge_src"], dtype=np.int64)
    edge_dst = np.asarray(inputs["edge_dst"], dtype=np.int64)
    lg_src = np.asarray(inputs["lg_src"], dtype=np.int64)
    lg_dst = np.asarray(inputs["lg_dst"], dtype=np.int64)
    tgt_nodes = np.asarray(inputs["tgt_nodes"], dtype=np.int64)
    graph_ids = np.asarray(inputs["graph_ids"], dtype=np.int64)
    node_x = np.asarray(inputs["node_x"], dtype=np.float32)
    bond_x = np.asarray(inputs["bond_x"], dtype=np.float32)

    meta = {}

    # ---- edge -> core (snake deal by lg in-degree desc) ----
    deg = np.bincount(lg_dst, minlength=N_EDGES)
    order = np.argsort(-deg, kind="stable")
    cyc = np.arange(N_EDGES) % (2 * CORES)
    core_of_rank = np.where(cyc < CORES, cyc, 2 * CORES - 1 - cyc)
    slots = np.full((CORES, EPC), -1, dtype=np.int64)
    for c in range(CORES):
        mine = order[core_of_rank == c]
        slots[c, :len(mine)] = mine
    new_id = np.full(N_EDGES, -1, dtype=np.int64)
    for c in range(CORES):
        valid = slots[c] >= 0
        new_id[slots[c][valid]] = c * EPC + np.nonzero(valid)[0]
    assert (new_id >= 0).all()
    meta["slots"] = slots
    meta["new_id"] = new_id

    lg_order, lg_starts = _group_by(lg_dst, N_EDGES)
    slot_deg = np.where(slots >= 0, deg[np.clip(slots, 0, None)], 0)
    R_lg = slot_deg.reshape(CORES, NBINS_E, 128).max(axis=2).max(axis=0)
    meta["R_lg"] = R_lg

    # ---- nodes ----
    g_starts = np.zeros(NG + 1, dtype=np.int64)
    np.cumsum(np.bincount(graph_ids, minlength=NG), out=g_starts[1:])
    n_deg = np.bincount(edge_dst, minlength=N_NODES)
    t_cnt = np.bincount(tgt_nodes, minlength=N_NODES)
    counts_g = np.bincount(graph_ids, minlength=NG).astype(np.float64)

    sup_nodes = []
    for c in range(CORES):
        for u in range(SUP):
            g0 = c * GPC + u * 128
            nodes = np.arange(g_starts[g0], g_starts[g0 + 128])
            nodes = nodes[np.argsort(-n_deg[nodes], kind="stable")]
            sup_nodes.append(nodes)
    NBINS_N = int(max((len(x) + 127) // 128 for x in sup_nodes))
    NPS = NBINS_N * 128
    meta["NBINS_N"] = NBINS_N
    meta["NPS"] = NPS
    nslot = np.full((CORES, SUP, NPS), -1, dtype=np.int64)
    for c in range(CORES):
        for u in range(SUP):
            nodes = sup_nodes[c * SUP + u]
            nslot[c, u, :len(nodes)] = nodes
    meta["nslot"] = nslot

    BPC = SUP * NPS
    B_ALL = CORES * BPC
    beta_row_of_node = np.full(N_NODES, -1, np.int64)
    for c in range(CORES):
        sl = nslot[c].reshape(-1)
        v = sl >= 0
        beta_row_of_node[sl[v]] = c * BPC + np.nonzero(v)[0]
    assert (beta_row_of_node >= 0).all()
    meta["BPC"] = BPC
    meta["B_ALL"] = B_ALL
    pad_pos = np.nonzero(nslot[0].reshape(-1) < 0)[0]
    ZB = int(pad_pos[0]) if len(pad_pos) else 0
    meta["ZB"] = ZB

    slot_nd = np.where(nslot >= 0, n_deg[np.clip(nslot, 0, None)], 0)
    slot_nt = np.where(nslot >= 0, t_cnt[np.clip(nslot, 0, None)], 0)
    R_m = slot_nd.reshape(CORES, SUP * NBINS_N, 128).max(axis=2).max(axis=0)
    R_tn = slot_nt.reshape(CORES, SUP * NBINS_N, 128).max(axis=2).max(axis=0)
    meta["R_m"] = R_m
    meta["R_tn"] = R_tn

    e_order, e_starts = _group_by(edge_dst, N_NODES)
    t_order, t_starts = _group_by(tgt_nodes, N_NODES)

    def consumer_rows_edges(Rs, order_, starts_, src_map, zero_row, degs):
        """Build [CORES, n_cons] consumer source rows for edge bins.
        Segments of <= SEG_R rounds per bin (ucode call cap).
        bins_list entries: (bin, coloff, Rseg, first, last)."""
        bins_list = []
        blocks = []
        coff = 0
        for b in range(NBINS_E):
            R = int(Rs[b])
            if R == 0:
                continue
            r0 = 0
            while r0 < R:
                rs = min(SEG_R, R - r0)
                bins_list.append((b, coff, rs, r0 == 0, r0 + rs == R))
                blocks.append(rs * 128)
                coff += rs * 128
                r0 += rs
        n_cons = coff
        rows = np.full((CORES, n_cons), zero_row, np.int64)
        seg_round0 = {}
        r_run = {}
        for (b, co, rs, first, last) in bins_list:
            if first:
                r_run[b] = 0
            seg_round0[(b, co)] = r_run[b]
            r_run[b] += rs
        for c in range(CORES):
            for (b, co, rs, first, last) in bins_list:
                r0 = seg_round0[(b, co)]
                sl = slots[c, b*128:(b+1)*128]
                blk = np.full((rs, 128), zero_row, np.int64)
                for s in range(128):
                    e = sl[s]
                    if e < 0:
                        continue
                    d = int(degs[e])
                    lo, hi = min(r0, d), min(r0 + rs, d)
                    if hi <= lo:
                        continue
                    js = order_[starts_[e] + lo:starts_[e] + hi]
                    blk[:hi - lo, s] = src_map(js)
                rows[c, co:co + rs * 128] = blk.reshape(-1)
        return rows, blocks, bins_list

    # ---- lg ----
    rows_lg, blocks_lg, bins_lg = consumer_rows_edges(
        R_lg, lg_order, lg_starts, lambda js: new_id[lg_src[js]], ZR, deg)
    meta["lg"] = build_2hop(rows_lg, E_ALL, blocks_lg)
    meta["lg_bins"] = bins_lg
    meta["lg_zero_bins"] = [b for b in range(NBINS_E) if R_lg[b] == 0]

    # ---- beta-gather: 1 round per edge bin ----
    rows_bg = np.full((CORES, EPC), ZB, np.int64)
    for c in range(CORES):
        v = slots[c] >= 0
        rows_bg[c, v] = beta_row_of_node[edge_src[slots[c][v]]]
    meta["bg"] = build_2hop(rows_bg, B_ALL, [128] * NBINS_E)

    def consumer_rows_nodes(Rs, order_, starts_, src_map, zero_row, degs):
        """bins_list entries: (u, b, coloff, Rseg, first, last)."""
        bins_list = []
        blocks = []
        coff = 0
        for ub in range(SUP * NBINS_N):
            R = int(Rs[ub])
            if R == 0:
                continue
            r0 = 0
            while r0 < R:
                rs = min(SEG_R, R - r0)
                bins_list.append((ub // NBINS_N, ub % NBINS_N, coff, rs,
                                  r0 == 0, r0 + rs == R))
                blocks.append(rs * 128)
                coff += rs * 128
                r0 += rs
        n_cons = coff
        rows = np.full((CORES, n_cons), zero_row, np.int64)
        seg_round0 = {}
        r_run = {}
        for (u, b, co, rs, first, last) in bins_list:
            if first:
                r_run[(u, b)] = 0
            seg_round0[co] = r_run[(u, b)]
            r_run[(u, b)] += rs
        for c in range(CORES):
            for (u, b, co, rs, first, last) in bins_list:
                r0 = seg_round0[co]
                sl = nslot[c, u, b*128:(b+1)*128]
                blk = np.full((rs, 128), zero_row, np.int64)
                for s in range(128):
                    vtx = sl[s]
                    if vtx < 0:
                        continue
                    d = int(degs[vtx])
                    lo, hi = min(r0, d), min(r0 + rs, d)
                    if hi <= lo:
                        continue
                    js = order_[starts_[vtx] + lo:starts_[vtx] + hi]
                    blk[:hi - lo, s] = src_map(js)
                rows[c, co:co + rs * 128] = blk.reshape(-1)
        return rows, blocks, bins_list

    # ---- tree ----
    rows_tr, blocks_tr, bins_tr = consumer_rows_nodes(
        R_tn, t_order, t_starts, lambda js: js, ZT, t_cnt)
    meta["tr"] = build_2hop(rows_tr, TREE_PAD, blocks_tr)
    meta["tr_bins"] = bins_tr
    meta["tr_zero_bins"] = [(ub // NBINS_N, ub % NBINS_N)
                            for ub in range(SUP * NBINS_N) if R_tn[ub] == 0]

    # ---- m ----
    rows_m, blocks_m, bins_m = consumer_rows_nodes(
        R_m, e_order, e_starts, lambda js: new_id[js], ZR, n_deg)
    meta["m"] = build_2hop(rows_m, E_ALL, blocks_m)
    meta["m_bins"] = bins_m
    meta["m_zero_bins"] = [(ub // NBINS_N, ub % NBINS_N)
                           for ub in range(SUP * NBINS_N) if R_m[ub] == 0]

    # ---- per-core float layouts (pure permutations of inputs) ----
    per_core = []
    inv_cnt = (1.0 / np.maximum(counts_g, 1.0)).astype(np.float32)
    for c in range(CORES):
        pc = {}
        featT = np.zeros((AF + BF, EPC), np.float32)
        v = slots[c] >= 0
        featT[:AF, v] = node_x[edge_src[slots[c][v]]].T
        featT[AF:, v] = bond_x[slots[c][v]].T
        pc["featT"] = featT

        featTn = np.zeros((AF + 1, SUP * NPS), np.float32)
        spool = np.zeros((SUP * NPS, 128), np.float32)
        for u in range(SUP):
            sl = nslot[c, u]
            vv = sl >= 0
            base = u * NPS
            featTn[:AF, base:base + NPS][:, vv] = node_x[sl[vv]].T
            featTn[AF, base:base + NPS][vv] = 1.0
            gl = graph_ids[np.clip(sl, 0, None)] - (c * GPC + u * 128)
            idxs = np.nonzero(vv)[0]
            spool[base + idxs, gl[idxs]] = inv_cnt[graph_ids[sl[idxs]]]
        pc["featTn"] = featTn
        pc["spool"] = spool
        pc["lg_h1"] = meta["lg"]["hop1_idx"][c]
        pc["lg_h2"] = meta["lg"]["hop2_idx"][c]
        pc["bg_h1"] = meta["bg"]["hop1_idx"][c]
        pc["bg_h2"] = meta["bg"]["hop2_idx"][c]
        pc["tr_h1"] = meta["tr"]["hop1_idx"][c]
        pc["tr_h2"] = meta["tr"]["hop2_idx"][c]
        pc["m_h1"] = meta["m"]["hop1_idx"][c]
        pc["m_h2"] = meta["m"]["hop2_idx"][c]
        per_core.append(pc)

    return per_core, meta


"""JT-MPN GNN kernel v2: 2-hop dma_gather message passing on 8 trn2 cores.

Per iteration: hop-1 window dma_gathers pack needed msg rows into SBUF
chunks (int16 indices), hop-2 SBUF-source transpose dma_gather re-reads
them in consumer (bin, round, slot) order directly in TT layout; DVE sums
rounds; PE does the W_h matmuls; DVE adds input2; ACT applies relu.
AllGather (bf16) between iterations. All float math on device.
"""
import concourse.bacc as bacc
import concourse.bass as bass
import concourse.mybir as mybir
import concourse.tile as tile

F32 = mybir.dt.float32
BF16 = mybir.dt.bfloat16
I16 = mybir.dt.int16
AluOp = mybir.AluOpType
Act = mybir.ActivationFunctionType


def build(meta, stub_collectives=False, n_iters=DEPTH - 1):
    NBINS_N = meta["NBINS_N"]
    NPS = meta["NPS"]
    BPC = meta["BPC"]
    B_ALL = meta["B_ALL"]
    lg, bg, tr, m = meta["lg"], meta["bg"], meta["tr"], meta["m"]
    SW1 = max(bg["c1_cols"], tr["c1_cols"], m["c1_cols"])
    SW2 = max(bg["c2_cols"], tr["c2_cols"], m["c2_cols"])

    nc = bacc.Bacc("TRN2", target_bir_lowering=False, debug=False)

    # ---- external IO ----
    featT_d = nc.dram_tensor("featT", [AF + BF, EPC], F32, kind="ExternalInput")
    featTn_d = nc.dram_tensor("featTn", [AF + 1, SUP * NPS], F32, kind="ExternalInput")
    spool_d = nc.dram_tensor("spool", [SUP * NPS, 128], F32, kind="ExternalInput")
    tree_d = nc.dram_tensor("tree_bf", [TREE_PAD, H], BF16, kind="ExternalInput")
    Wi_d = nc.dram_tensor("Wi", [AF + BF, H], F32, kind="ExternalInput")
    Wh_d = nc.dram_tensor("Wh_bf", [H, H], BF16, kind="ExternalInput")
    WoT_d = nc.dram_tensor("WoTop", [AF + 1, H], F32, kind="ExternalInput")
    Wob_d = nc.dram_tensor("Wob_bf", [H, H], BF16, kind="ExternalInput")
    identb_d = nc.dram_tensor("ident_bf", [128, 128], BF16, kind="ExternalInput")
    idx_d = {}
    for nm, tab in (("lg", lg), ("bg", bg), ("tr", tr), ("m", m)):
        idx_d[nm + "_h1"] = nc.dram_tensor(nm + "_h1", [128, tab["c1_cols"]], I16,
                                           kind="ExternalInput")
        idx_d[nm + "_h2"] = nc.dram_tensor(nm + "_h2", [128, tab["c2_cols"]], I16,
                                           kind="ExternalInput")
    gout = nc.dram_tensor("gout", [GPC, H], F32, kind="ExternalOutput")

    with tile.TileContext(nc) as tc:
        with tc.tile_pool(name="dram", bufs=1, space="DRAM") as dram, \
             tc.tile_pool(name="const", bufs=1) as cpool, \
             tc.tile_pool(name="idxp", bufs=1) as idxp, \
             tc.tile_pool(name="chunk", bufs=2) as chp, \
             tc.tile_pool(name="tt", bufs=2) as ttp, \
             tc.tile_pool(name="stream", bufs=3) as stp, \
             tc.tile_pool(name="scratch", bufs=3) as scp, \
             tc.tile_pool(name="psum_b", bufs=3, space="PSUM") as ppb, \
             tc.tile_pool(name="psum_g", bufs=1, space="PSUM") as ppg:

            beta_shard = dram.tile([BPC, H], BF16)
            beta_full = dram.tile([B_ALL, H], BF16,
                                  addr_space=("Local" if stub_collectives else "Shared"))
            in2_d = dram.tile([EPC, H], BF16)
            gamma_d = dram.tile([SUP * NPS, H], BF16)
            msg_shard = dram.tile([EPC, H], BF16)
            n_ags = 1 + n_iters
            msg_fulls = [dram.tile([E_ALL, H], BF16,
                                   addr_space=("Local" if stub_collectives else "Shared"),
                                   name=f"msg_full_{k}") for k in range(n_ags)]

            # ---- constants ----
            wi_sb = cpool.tile([AF + BF, H], F32)
            nc.sync.dma_start(wi_sb[:], Wi_d[:])
            whA = cpool.tile([128, H], BF16)
            whB = cpool.tile([128, H], BF16)
            nc.sync.dma_start(whA[:], Wh_d[0:128, :])
            nc.sync.dma_start(whB[:], Wh_d[128:256, :])
            wot = cpool.tile([AF + 1, H], F32)
            nc.sync.dma_start(wot[:], WoT_d[:])
            wobA = cpool.tile([128, H], BF16)
            wobB = cpool.tile([128, H], BF16)
            nc.sync.dma_start(wobA[:], Wob_d[0:128, :])
            nc.sync.dma_start(wobB[:], Wob_d[128:256, :])
            idb = cpool.tile([128, 128], BF16)
            nc.sync.dma_start(idb[:], identb_d[:])

            lg_h1 = idxp.tile([128, lg["c1_cols"]], I16)
            nc.sync.dma_start(lg_h1[:], idx_d["lg_h1"][:])
            lg_h2 = idxp.tile([128, lg["c2_cols"]], I16)
            nc.sync.dma_start(lg_h2[:], idx_d["lg_h2"][:])

            def load_idx(nm, tab):
                h1 = idxp.tile([128, SW1], I16, tag="sw1")
                nc.sync.dma_start(h1[:, :tab["c1_cols"]], idx_d[nm + "_h1"][:])
                h2 = idxp.tile([128, SW2], I16, tag="sw2")
                nc.sync.dma_start(h2[:, :tab["c2_cols"]], idx_d[nm + "_h2"][:])
                return h1, h2

            def hop1_chunks(tab, h1, table_dram, table_rows):
                calls_by_chunk = {}
                for (t, wlo, n_pad, c1off) in tab["hop1_calls"]:
                    calls_by_chunk.setdefault(t, []).append((wlo, n_pad, c1off))
                for t in range(tab["n_chunks"]):
                    B = tab["hop1_blocks"][t]
                    ct = chp.tile([128, CHUNK_ROWS // 128, H], BF16, tag="ct")
                    o = 0
                    for (wlo, n_pad, c1off) in calls_by_chunk[t]:
                        wlen = min(WIN, table_rows - wlo)
                        nb = n_pad // 128
                        nc.gpsimd.dma_gather(
                            out_ap=ct[:, o:o + nb, :],
                            in_ap=table_dram[wlo:wlo + wlen, :],
                            idxs_ap=h1[:, c1off:c1off + n_pad // 16],
                            num_idxs=n_pad, num_idxs_reg=n_pad, elem_size=H)
                        o += nb
                    assert o == B
                    yield t, ct, B

            def hop2(h2, ct, call):
                (t, n, c2off, outoff) = call
                flat = ttp.tile([128, 2 * HOP2_MAX], BF16, tag="tt")
                ttt = flat[:, 0:2 * n].rearrange("p (k n) -> p k n", k=2)
                nc.gpsimd.dma_gather(
                    out_ap=ttt,
                    in_ap=ct[:],
                    idxs_ap=h2[:, c2off:c2off + n // 16],
                    num_idxs=n, num_idxs_reg=n, elem_size=H,
                    transpose=True,
                    sbuf_tokens_per_rank=128,
                    sbuf_free_dim_per_rank=H * 2)
                return ttt

            def reduce_rounds(ttt, c0, R):
                acc = ttt[:, :, c0:c0 + 128]
                for r in range(1, R):
                    nc.vector.tensor_tensor(
                        out=acc, in0=acc,
                        in1=ttt[:, :, c0 + r * 128:c0 + (r + 1) * 128],
                        op=AluOp.add)
                return acc

            def allgather(src, dst, shard_rows):
                if stub_collectives:
                    for rep in range(2):
                        lo = (rep * shard_rows) % max(dst.shape[0] - shard_rows, 1) \
                            if dst.shape[0] > shard_rows else 0
                        nc.sync.dma_start(dst[lo:lo + shard_rows, :], src[:])
                    return
                nc.gpsimd.collective_compute(
                    "AllGather", AluOp.bypass,
                    replica_groups=[list(range(CORES))],
                    ins=[src[:].opt()], outs=[dst[:].opt()])

            # ================= phase 0a: msg0 + AG0 =================
            for g0 in range(0, NBINS_E, 8):
                gsz = min(8, NBINS_E - g0)
                ft = stp.tile([AF + BF, 8 * 128], F32, tag="ft")
                nc.sync.dma_start(ft[:, :gsz * 128],
                                  featT_d[:, g0 * 128:(g0 + gsz) * 128])
                msgb = stp.tile([128, 8, H], BF16, tag="msgb")
                for bi in range(gsz):
                    pa = ppb.tile([128, H], F32, tag="ps")
                    nc.tensor.matmul(pa[:], lhsT=ft[:, bi * 128:(bi + 1) * 128],
                                     rhs=wi_sb[:], start=True, stop=True)
                    nc.scalar.activation(msgb[:, bi, :], pa[:], Act.Relu)
                nc.sync.dma_start(
                    msg_shard[g0 * 128:(g0 + gsz) * 128, :]
                    .rearrange("(m p) d -> p m d", p=128), msgb[:, :gsz, :])
            allgather(msg_shard, msg_fulls[0], EPC)

            # ================= phase 0b: tree -> beta/gamma =================
            tr_h1, tr_h2 = load_idx("tr", tr)
            tr_calls = tr["hop2_calls"]
            zgb = scp.tile([128, H], BF16, tag="zgb")
            nc.vector.memset(zgb[:], 0.0)
            for (u, b) in meta["tr_zero_bins"]:
                base = u * NPS + b * 128
                nc.sync.dma_start(beta_shard[base:base + 128, :], zgb[:])
                nc.sync.dma_start(gamma_d[base:base + 128, :], zgb[:])
            tr_chunks = hop1_chunks(tr, tr_h1, tree_d, TREE_PAD)
            cur = {"t": -1, "ct": None, "tt": None, "rng": (0, 0), "ci": 0}

            def advance_to(tab, calls, h2, chunks_iter, coff):
                """Ensure the hop-2 call containing coff is current."""
                while not (cur["rng"][0] <= coff < cur["rng"][1]):
                    call = calls[cur["ci"]]
                    while cur["t"] < call[0]:
                        t_, ct_, B_ = next(chunks_iter)
                        cur["t"] = t_
                        cur["ct"] = ct_
                    cur["tt"] = hop2(h2, cur["ct"], call)
                    cur["rng"] = (call[3], call[3] + call[1])
                    cur["ci"] += 1
                return cur["tt"], cur["rng"][0]

            acc_hold = {}
            for (u, b, coff, R, first, last) in meta["tr_bins"]:
                ttt, o0 = advance_to(tr, tr_calls, tr_h2, tr_chunks, coff)
                acc = reduce_rounds(ttt, coff - o0, R)
                if not first:
                    nc.vector.tensor_tensor(out=acc_hold[(u, b)],
                                            in0=acc_hold[(u, b)], in1=acc,
                                            op=AluOp.add)
                else:
                    acc_hold[(u, b)] = acc
                if not last:
                    continue
                acc = acc_hold.pop((u, b))
                base = u * NPS + b * 128
                pb_ = ppb.tile([128, H], F32, tag="ps")
                nc.tensor.matmul(pb_[:], lhsT=acc[:, 0, :], rhs=whA[:],
                                 start=True, stop=False)
                nc.tensor.matmul(pb_[:], lhsT=acc[:, 1, :], rhs=whB[:],
                                 start=False, stop=True)
                bout = scp.tile([128, H], BF16, tag="bout")
                nc.scalar.activation(bout[:], pb_[:], Act.Copy)
                nc.sync.dma_start(beta_shard[base:base + 128, :], bout[:])
                pg_ = ppb.tile([128, H], F32, tag="ps")
                nc.tensor.matmul(pg_[:], lhsT=acc[:, 0, :], rhs=wobA[:],
                                 start=True, stop=False)
                nc.tensor.matmul(pg_[:], lhsT=acc[:, 1, :], rhs=wobB[:],
                                 start=False, stop=True)
                gt = scp.tile([128, H], BF16, tag="gt")
                nc.scalar.activation(gt[:], pg_[:], Act.Copy)
                nc.sync.dma_start(gamma_d[base:base + 128, :], gt[:])
            for _ in tr_chunks:
                pass
            allgather(beta_shard, beta_full, BPC)

            # ================= phase 0c: input2 =================
            bg_h1, bg_h2 = load_idx("bg", bg)
            bg_calls = bg["hop2_calls"]
            ci = 0
            for t, ct, B in hop1_chunks(bg, bg_h1, beta_full, B_ALL):
                while ci < len(bg_calls) and bg_calls[ci][0] == t:
                    call = bg_calls[ci]
                    (tt_, n, c2off, outoff) = call
                    ttt = hop2(bg_h2, ct, call)
                    nb = n // 128
                    for j0 in range(0, nb, 8):
                        jn = min(8, nb - j0)
                        lo = outoff + j0 * 128
                        ft = stp.tile([AF + BF, 8 * 128], F32, tag="ft")
                        nc.sync.dma_start(ft[:, :jn * 128],
                                          featT_d[:, lo:lo + jn * 128])
                        i2b = stp.tile([128, 8, H], BF16, tag="msgb")
                        for j in range(jn):
                            jj = j0 + j
                            pt = ppb.tile([128, H], BF16, tag="pt")
                            nc.tensor.transpose(
                                pt[:, 0:128], ttt[:, 0, jj * 128:(jj + 1) * 128], idb[:])
                            nc.tensor.transpose(
                                pt[:, 128:256], ttt[:, 1, jj * 128:(jj + 1) * 128], idb[:])
                            brow = scp.tile([128, H], BF16, tag="brow")
                            nc.scalar.activation(brow[:], pt[:], Act.Copy)
                            pa = ppb.tile([128, H], F32, tag="ps")
                            nc.tensor.matmul(pa[:], lhsT=ft[:, j * 128:(j + 1) * 128],
                                             rhs=wi_sb[:], start=True, stop=True)
                            nc.vector.tensor_tensor(out=i2b[:, j, :], in0=pa[:],
                                                    in1=brow[:], op=AluOp.add)
                        nc.sync.dma_start(
                            in2_d[lo:lo + jn * 128, :]
                            .rearrange("(m p) d -> p m d", p=128), i2b[:, :jn, :])
                    ci += 1

            # ================= BP iterations =================
            lg_calls = lg["hop2_calls"]
            lg_bin_list = meta["lg_bins"]   # (b, coff, R)
            nz_bins = len(lg_bin_list)
            for it in range(n_iters):
                src_full = msg_fulls[it]
                ci = 0
                for t, ct, B in hop1_chunks(lg, lg_h1, src_full, E_ALL):
                    while ci < len(lg_calls) and lg_calls[ci][0] == t:
                        call = lg_calls[ci]
                        (tt_, n, c2off, outoff) = call
                        ttt = hop2(lg_h2, ct, call)
                        bins_in = [x for x in lg_bin_list
                                   if outoff <= x[1] < outoff + n]
                        for j0 in range(0, len(bins_in), 8):
                            sub = bins_in[j0:j0 + 8]
                            b_first = sub[0][0]
                            jn = len(sub)
                            i2l = stp.tile([128, 8, H], BF16, tag="i2l")
                            nc.sync.dma_start(
                                i2l[:, :jn, :],
                                in2_d[b_first * 128:(b_first + jn) * 128, :]
                                .rearrange("(m p) d -> p m d", p=128))
                            msgb = stp.tile([128, 8, H], BF16, tag="msgb")
                            for j, (b, coff, R) in enumerate(sub):
                                assert b == b_first + j
                                acc = reduce_rounds(ttt, coff - outoff, R)
                                pb_ = ppb.tile([128, H], F32, tag="ps")
                                nc.tensor.matmul(pb_[:], lhsT=acc[:, 0, :],
                                                 rhs=whA[:], start=True, stop=False)
                                nc.tensor.matmul(pb_[:], lhsT=acc[:, 1, :],
                                                 rhs=whB[:], start=False, stop=True)
                                tmp = scp.tile([128, H], BF16, tag="tmp")
                                nc.vector.tensor_tensor(out=tmp[:], in0=pb_[:],
                                                        in1=i2l[:, j, :], op=AluOp.add)
                                nc.scalar.activation(msgb[:, j, :], tmp[:], Act.Relu)
                            nc.sync.dma_start(
                                msg_shard[b_first * 128:(b_first + jn) * 128, :]
                                .rearrange("(m p) d -> p m d", p=128), msgb[:, :jn, :])
                        ci += 1
                # zero-R tail bins: msg = relu(in2)
                for g0 in range(nz_bins, NBINS_E, 8):
                    gsz = min(8, NBINS_E - g0)
                    i2l = stp.tile([128, 8, H], BF16, tag="i2l")
                    nc.sync.dma_start(
                        i2l[:, :gsz, :],
                        in2_d[g0 * 128:(g0 + gsz) * 128, :]
                        .rearrange("(m p) d -> p m d", p=128))
                    msgb = stp.tile([128, 8, H], BF16, tag="msgb")
                    for j in range(gsz):
                        nc.scalar.activation(msgb[:, j, :], i2l[:, j, :], Act.Relu)
                    nc.sync.dma_start(
                        msg_shard[g0 * 128:(g0 + gsz) * 128, :]
                        .rearrange("(m p) d -> p m d", p=128), msgb[:, :gsz, :])
                allgather(msg_shard, msg_fulls[it + 1], EPC)

            # ================= final =================
            m_h1, m_h2 = load_idx("m", m)
            m_calls = m["hop2_calls"]
            m_bin_map = {(u, b): (coff, R) for (u, b, coff, R) in meta["m_bins"]}
            m_tiles = {}
            for t, ct, B in hop1_chunks(m, m_h1, msg_fulls[n_iters], E_ALL):
                m_tiles[t] = ct
            ci = 0
            cur_tt = None
            cur_range = (0, 0)
            for u in range(SUP):
                pg = ppg.tile([128, H], F32, tag="pg")
                for b in range(NBINS_N):
                    base = u * NPS + b * 128
                    ftn = scp.tile([AF + 1, 128], F32, tag="ftn")
                    nc.sync.dma_start(ftn[:], featTn_d[:, base:base + 128])
                    pc_ = ppb.tile([128, H], F32, tag="ps")
                    if (u, b) in m_bin_map:
                        coff, R = m_bin_map[(u, b)]
                        if not (cur_range[0] <= coff < cur_range[1]):
                            call = m_calls[ci]
                            assert call[3] == coff, (call, coff)
                            cur_tt = hop2(m_h2, m_tiles[call[0]], call)
                            cur_range = (call[3], call[3] + call[1])
                            ci += 1
                        acc = reduce_rounds(cur_tt, coff - cur_range[0], R)
                        nc.tensor.matmul(pc_[:], lhsT=ftn[:], rhs=wot[:],
                                         start=True, stop=False)
                        nc.tensor.matmul(pc_[:], lhsT=acc[:, 0, :], rhs=wobA[:],
                                         start=False, stop=False)
                        nc.tensor.matmul(pc_[:], lhsT=acc[:, 1, :], rhs=wobB[:],
                                         start=False, stop=True)
                    else:
                        nc.tensor.matmul(pc_[:], lhsT=ftn[:], rhs=wot[:],
                                         start=True, stop=True)
                    gml = scp.tile([128, H], BF16, tag="gml")
                    nc.sync.dma_start(gml[:], gamma_d[base:base + 128, :])
                    hsum = scp.tile([128, H], F32, tag="hsum")
                    nc.vector.tensor_tensor(out=hsum[:], in0=pc_[:], in1=gml[:],
                                            op=AluOp.add)
                    h = scp.tile([128, H], F32, tag="h")
                    nc.scalar.activation(h[:], hsum[:], Act.Relu)
                    sp = scp.tile([128, 128], F32, tag="sp")
                    nc.sync.dma_start(sp[:], spool_d[base:base + 128, :])
                    nc.tensor.matmul(pg[:], lhsT=sp[:], rhs=h[:],
                                     start=(b == 0), stop=(b == NBINS_N - 1))
                go = scp.tile([128, H], F32, tag="go")
                nc.scalar.activation(go[:], pg[:], Act.Copy)
                nc.sync.dma_start(gout[u * 128:(u + 1) * 128, :], go[:])

    nc.finalize()
    return nc


def make_in_maps(inputs, per_core, meta):
    import ml_dtypes
    W_i = np.asarray(inputs["W_i"], np.float32)
    W_h = np.asarray(inputs["W_h"], np.float32)
    W_o = np.asarray(inputs["W_o"], np.float32)
    b_o = np.asarray(inputs["b_o"], np.float32)
    tree_pad = np.zeros((TREE_PAD, H), ml_dtypes.bfloat16)
    tree_pad[:60000] = np.asarray(inputs["tree_mess"], np.float32
                                  ).astype(ml_dtypes.bfloat16)
    shared = {
        "tree_bf": tree_pad,
        "Wi": W_i,
        "Wh_bf": W_h.astype(ml_dtypes.bfloat16),
        "WoTop": np.concatenate([W_o[:AF], b_o[None, :]], 0),
        "Wob_bf": W_o[AF:].astype(ml_dtypes.bfloat16),
        "ident_bf": np.eye(128).astype(ml_dtypes.bfloat16),
    }
    maps = []
    for c in range(CORES):
        pc = per_core[c]
        mp = dict(shared)
        mp["featT"] = pc["featT"]
        mp["featTn"] = pc["featTn"]
        mp["spool"] = pc["spool"]
        for nm in ("lg", "bg", "tr", "m"):
            mp[nm + "_h1"] = pc[nm + "_h1"]
            mp[nm + "_h2"] = pc[nm + "_h2"]
        maps.append({k: np.ascontiguousarray(v) for k, v in mp.items()})
    return maps


_BUILD_CACHE = {}


def kernel(**inputs):
    from concourse import bass_utils
    per_core, meta = preprocess(inputs)
    key = (meta["lg"]["c1_cols"], meta["lg"]["c2_cols"], meta["m"]["c1_cols"],
           meta["bg"]["c1_cols"], meta["tr"]["c1_cols"], meta["NBINS_N"])
    nc = _BUILD_CACHE.get(key)
    if nc is None:
        nc = build(meta)
        _BUILD_CACHE[key] = nc
    in_maps = make_in_maps(inputs, per_core, meta)
    res = bass_utils.run_bass_kernel_spmd(nc, in_maps, core_ids=list(range(CORES)))
    out = np.concatenate([res.results[c]["gout"] for c in range(CORES)], axis=0)
    return out.astype(np.float32)


# revision 7
# speedup vs baseline: 1.0783x; 1.0411x over previous
"""JT-MPN GNN kernel for 8 trn2 NeuronCores (self-contained).

Two-hop dma_gather message passing: hop-1 packs needed message rows from
int16-addressable 32K-row windows of the AllGathered message table into
SBUF chunks; hop-2 re-gathers them SBUF->SBUF in consumer (bin, round,
slot) order directly in transposed (TT) layout. DVE sums rounds, PE runs
the W_h matmuls (bf16, f32 PSUM), ACT applies relu. bf16 AllGather
between the three BP iterations; graph mean-pool via a selection matmul.
"""

import numpy as np

N_NODES = 150000
N_EDGES = 300000
H = 256
DEPTH = 4
AF = 35
BF = 5
NG = 2048
CORES = 8

EPC = 37504               # edges per core (293 bins * 128)
NBINS_E = EPC // 128      # 293
GPC = NG // CORES         # 256
SUP = 2
ZR = 37500                # global msg row guaranteed zero
E_ALL = CORES * EPC       # 300032
WIN = 32768

TREE_PAD = 60416
ZT = 60000                # zero row in padded tree table

CHUNK_ROWS = 12288        # hop-1 chunk tile rows (B=96 blocks)
HOP2_MAX = 768            # transpose-mode ucode ring cap (1024 crashes)
H1_MAX = 1024
SEG_R = HOP2_MAX // 128   # max rounds per consumer segment


def _group_by(dst, n_groups):
    order = np.argsort(dst, kind="stable")
    counts = np.bincount(dst, minlength=n_groups)
    starts = np.zeros(n_groups + 1, dtype=np.int64)
    np.cumsum(counts, out=starts[1:])
    return order, starts


def wrap_idx(vals, cols):
    """[n] ints -> wrapped [128, cols] int16 (8x replicated); pad -1.
    Index i lives at [i%16, i//16]."""
    flat = np.full(cols * 16, -1, np.int16)
    flat[:len(vals)] = vals.astype(np.int16)
    w = np.ascontiguousarray(flat.reshape(cols, 16).T)
    return np.tile(w, (8, 1))


def build_2hop(src_rows, table_rows, block_sizes,
               chunk_rows=CHUNK_ROWS, hop2_max=HOP2_MAX):
    """Unified 2-hop tables for all cores.

    src_rows: [CORES, n_cons] global source row per consumer column.
    block_sizes: per atomic consumer block (bin), each multiple of 128.

    Returns dict:
      hop1_calls: list of (chunk, win_base, n_pad, col_off) ; n_valid is per
        core and encoded by -1 padding in idx (num_idxs_reg: use n_pad minus
        trailing -1 count? -> device passes per-core reg via ... ) NOTE:
        num_idxs_reg must be a compile-time constant in the unified program,
        so we pass n_pad and set padded idx entries to ZR-in-window when the
        window contains a guaranteed-zero row, else repeat the last valid
        index (harmless extra gather).
      hop1_idx: [CORES][128, C1] int16
      hop1_blocks: per chunk block count (unified)
      hop2_calls: list of (chunk, n, col_off, out_off)
      hop2_idx: [CORES][128, C2] int16
      n_chunks
    """
    n_wins = (table_rows + WIN - 1) // WIN
    nb = len(block_sizes)
    block_start = np.zeros(nb + 1, np.int64)
    np.cumsum(block_sizes, out=block_start[1:])
    n_cons = int(block_start[-1])
    assert src_rows.shape == (CORES, n_cons)

    # --- chunk assignment (unified): estimate per-core unique counts ---
    chunks = []
    cur_first = 0
    cur_rows = [set() for _ in range(CORES)]
    for b in range(nb):
        sl = slice(block_start[b], block_start[b + 1])
        newmax = 0
        for c in range(CORES):
            s = set(src_rows[c, sl].tolist())
            newmax = max(newmax, len(cur_rows[c] | s))
        if newmax > chunk_rows - 128 * n_wins and b > cur_first:
            chunks.append((cur_first, b))
            cur_first = b
            cur_rows = [set(src_rows[c, sl].tolist()) for c in range(CORES)]
        else:
            for c in range(CORES):
                cur_rows[c] |= set(src_rows[c, sl].tolist())
    chunks.append((cur_first, nb))

    hop1_calls = []
    hop1_vals = [[] for _ in range(CORES)]   # list of (colpos, array)
    hop1_blocks = []
    hop2_calls = []
    hop2_vals = [[] for _ in range(CORES)]
    c1_off = 0
    c2_off = 0
    for t, (b0, b1) in enumerate(chunks):
        sl = slice(block_start[b0], block_start[b1])
        uniqs = [np.unique(src_rows[c, sl]) for c in range(CORES)]
        poss = [np.full(len(u), -1, np.int64) for u in uniqs]
        p = 0
        for w in range(n_wins):
            wlo, whi = w * WIN, min((w + 1) * WIN, table_rows)
            sels = [(u >= wlo) & (u < whi) for u in uniqs]
            n_valid = [int(s.sum()) for s in sels]
            n_max = max(n_valid)
            if n_max == 0:
                continue
            n_pad = (n_max + 127) // 128 * 128
            for c in range(CORES):
                poss[c][sels[c]] = p + np.arange(n_valid[c])
                v = uniqs[c][sels[c]] - wlo
                if len(v) == 0:
                    v = np.array([0], np.int64)  # dummy row in window
                pad = np.full(n_pad - len(v), v[-1], np.int64)
                hop1_vals[c].append(np.concatenate([v, pad]))
            # split into sub-calls of <= H1_MAX indices (ucode ring cap)
            done = 0
            while done < n_pad:
                sub = min(H1_MAX, n_pad - done)
                hop1_calls.append((t, wlo, sub, c1_off))
                c1_off += sub // 16
                done += sub
            p += n_pad
        assert p <= chunk_rows, (p, chunk_rows)
        hop1_blocks.append(p // 128)

        h2 = []
        for c in range(CORES):
            j = np.searchsorted(uniqs[c], src_rows[c, sl])
            assert (uniqs[c][j] == src_rows[c, sl]).all()
            h2.append(poss[c][j])
            assert (poss[c][j] >= 0).all()
        # split into calls at block boundaries
        local_bs = block_start[b0:b1 + 1] - block_start[b0]
        bi = 0
        cstart = 0
        total = int(local_bs[-1])
        while cstart < total:
            cend = cstart
            while bi < b1 - b0 and local_bs[bi + 1] - cstart <= hop2_max:
                bi += 1
                cend = int(local_bs[bi])
            assert cend > cstart
            n = cend - cstart
            for c in range(CORES):
                hop2_vals[c].append(h2[c][cstart:cend])
            hop2_calls.append((t, n, c2_off, int(block_start[b0] + cstart)))
            c2_off += (n + 15) // 16
            cstart = cend

    hop1_idx = [wrap_idx(np.concatenate(hop1_vals[c]), max(c1_off, 1))
                for c in range(CORES)]
    hop2_idx = [wrap_idx(np.concatenate(hop2_vals[c]), max(c2_off, 1))
                for c in range(CORES)]
    return dict(hop1_calls=hop1_calls, hop1_idx=hop1_idx,
                hop1_blocks=hop1_blocks, hop2_calls=hop2_calls,
                hop2_idx=hop2_idx, n_chunks=len(chunks),
                c1_cols=max(c1_off, 1), c2_cols=max(c2_off, 1))


def preprocess(inputs):
    edge_src = np.asarray(inputs["edge_src"], dtype=np.int64)
    edge_dst = np.asarray(inputs["edge_dst"], dtype=np.int64)
    lg_src = np.asarray(inputs["lg_src"], dtype=np.int64)
    lg_dst = np.asarray(inputs["lg_dst"], dtype=np.int64)
    tgt_nodes = np.asarray(inputs["tgt_nodes"], dtype=np.int64)
    graph_ids = np.asarray(inputs["graph_ids"], dtype=np.int64)
    node_x = np.asarray(inputs["node_x"], dtype=np.float32)
    bond_x = np.asarray(inputs["bond_x"], dtype=np.float32)

    meta = {}

    # ---- edge -> core (snake deal by lg in-degree desc) ----
    deg = np.bincount(lg_dst, minlength=N_EDGES)
    order = np.argsort(-deg, kind="stable")
    cyc = np.arange(N_EDGES) % (2 * CORES)
    core_of_rank = np.where(cyc < CORES, cyc, 2 * CORES - 1 - cyc)
    slots = np.full((CORES, EPC), -1, dtype=np.int64)
    for c in range(CORES):
        mine = order[core_of_rank == c]
        slots[c, :len(mine)] = mine
    new_id = np.full(N_EDGES, -1, dtype=np.int64)
    for c in range(CORES):
        valid = slots[c] >= 0
        new_id[slots[c][valid]] = c * EPC + np.nonzero(valid)[0]
    assert (new_id >= 0).all()
    meta["slots"] = slots
    meta["new_id"] = new_id

    lg_order, lg_starts = _group_by(lg_dst, N_EDGES)
    slot_deg = np.where(slots >= 0, deg[np.clip(slots, 0, None)], 0)
    R_lg = slot_deg.reshape(CORES, NBINS_E, 128).max(axis=2).max(axis=0)
    meta["R_lg"] = R_lg

    # ---- nodes ----
    g_starts = np.zeros(NG + 1, dtype=np.int64)
    np.cumsum(np.bincount(graph_ids, minlength=NG), out=g_starts[1:])
    n_deg = np.bincount(edge_dst, minlength=N_NODES)
    t_cnt = np.bincount(tgt_nodes, minlength=N_NODES)
    counts_g = np.bincount(graph_ids, minlength=NG).astype(np.float64)

    sup_nodes = []
    for c in range(CORES):
        for u in range(SUP):
            g0 = c * GPC + u * 128
            nodes = np.arange(g_starts[g0], g_starts[g0 + 128])
            nodes = nodes[np.argsort(-n_deg[nodes], kind="stable")]
            sup_nodes.append(nodes)
    NBINS_N = int(max((len(x) + 127) // 128 for x in sup_nodes))
    NPS = NBINS_N * 128
    meta["NBINS_N"] = NBINS_N
    meta["NPS"] = NPS
    nslot = np.full((CORES, SUP, NPS), -1, dtype=np.int64)
    for c in range(CORES):
        for u in range(SUP):
            nodes = sup_nodes[c * SUP + u]
            nslot[c, u, :len(nodes)] = nodes
    meta["nslot"] = nslot

    BPC = SUP * NPS
    B_ALL = CORES * BPC
    beta_row_of_node = np.full(N_NODES, -1, np.int64)
    for c in range(CORES):
        sl = nslot[c].reshape(-1)
        v = sl >= 0
        beta_row_of_node[sl[v]] = c * BPC + np.nonzero(v)[0]
    assert (beta_row_of_node >= 0).all()
    meta["BPC"] = BPC
    meta["B_ALL"] = B_ALL
    pad_pos = np.nonzero(nslot[0].reshape(-1) < 0)[0]
    ZB = int(pad_pos[0]) if len(pad_pos) else 0
    meta["ZB"] = ZB

    slot_nd = np.where(nslot >= 0, n_deg[np.clip(nslot, 0, None)], 0)
    slot_nt = np.where(nslot >= 0, t_cnt[np.clip(nslot, 0, None)], 0)
    R_m = slot_nd.reshape(CORES, SUP * NBINS_N, 128).max(axis=2).max(axis=0)
    R_tn = slot_nt.reshape(CORES, SUP * NBINS_N, 128).max(axis=2).max(axis=0)
    meta["R_m"] = R_m
    meta["R_tn"] = R_tn

    e_order, e_starts = _group_by(edge_dst, N_NODES)
    t_order, t_starts = _group_by(tgt_nodes, N_NODES)

    def consumer_rows_edges(Rs, order_, starts_, src_map, zero_row, degs):
        """Build [CORES, n_cons] consumer source rows for edge bins.
        Segments of <= SEG_R rounds per bin (ucode call cap).
        bins_list entries: (bin, coloff, Rseg, first, last)."""
        bins_list = []
        blocks = []
        coff = 0
        for b in range(NBINS_E):
            R = int(Rs[b])
            if R == 0:
                continue
            r0 = 0
            while r0 < R:
                rs = min(SEG_R, R - r0)
                bins_list.append((b, coff, rs, r0 == 0, r0 + rs == R))
                blocks.append(rs * 128)
                coff += rs * 128
                r0 += rs
        n_cons = coff
        rows = np.full((CORES, n_cons), zero_row, np.int64)
        seg_round0 = {}
        r_run = {}
        for (b, co, rs, first, last) in bins_list:
            if first:
                r_run[b] = 0
            seg_round0[(b, co)] = r_run[b]
            r_run[b] += rs
        for c in range(CORES):
            for (b, co, rs, first, last) in bins_list:
                r0 = seg_round0[(b, co)]
                sl = slots[c, b*128:(b+1)*128]
                blk = np.full((rs, 128), zero_row, np.int64)
                for s in range(128):
                    e = sl[s]
                    if e < 0:
                        continue
                    d = int(degs[e])
                    lo, hi = min(r0, d), min(r0 + rs, d)
                    if hi <= lo:
                        continue
                    js = order_[starts_[e] + lo:starts_[e] + hi]
                    blk[:hi - lo, s] = src_map(js)
                rows[c, co:co + rs * 128] = blk.reshape(-1)
        return rows, blocks, bins_list

    # ---- lg ----
    rows_lg, blocks_lg, bins_lg = consumer_rows_edges(
        R_lg, lg_order, lg_starts, lambda js: new_id[lg_src[js]], ZR, deg)
    meta["lg"] = build_2hop(rows_lg, E_ALL, blocks_lg)
    meta["lg_bins"] = bins_lg
    meta["lg_zero_bins"] = [b for b in range(NBINS_E) if R_lg[b] == 0]

    # ---- beta-gather: 1 round per edge bin ----
    rows_bg = np.full((CORES, EPC), ZB, np.int64)
    for c in range(CORES):
        v = slots[c] >= 0
        rows_bg[c, v] = beta_row_of_node[edge_src[slots[c][v]]]
    meta["bg"] = build_2hop(rows_bg, B_ALL, [128] * NBINS_E)

    def consumer_rows_nodes(Rs, order_, starts_, src_map, zero_row, degs):
        """bins_list entries: (u, b, coloff, Rseg, first, last)."""
        bins_list = []
        blocks = []
        coff = 0
        for ub in range(SUP * NBINS_N):
            R = int(Rs[ub])
            if R == 0:
                continue
            r0 = 0
            while r0 < R:
                rs = min(SEG_R, R - r0)
                bins_list.append((ub // NBINS_N, ub % NBINS_N, coff, rs,
                                  r0 == 0, r0 + rs == R))
                blocks.append(rs * 128)
                coff += rs * 128
                r0 += rs
        n_cons = coff
        rows = np.full((CORES, n_cons), zero_row, np.int64)
        seg_round0 = {}
        r_run = {}
        for (u, b, co, rs, first, last) in bins_list:
            if first:
                r_run[(u, b)] = 0
            seg_round0[co] = r_run[(u, b)]
            r_run[(u, b)] += rs
        for c in range(CORES):
            for (u, b, co, rs, first, last) in bins_list:
                r0 = seg_round0[co]
                sl = nslot[c, u, b*128:(b+1)*128]
                blk = np.full((rs, 128), zero_row, np.int64)
                for s in range(128):
                    vtx = sl[s]
                    if vtx < 0:
                        continue
                    d = int(degs[vtx])
                    lo, hi = min(r0, d), min(r0 + rs, d)
                    if hi <= lo:
                        continue
                    js = order_[starts_[vtx] + lo:starts_[vtx] + hi]
                    blk[:hi - lo, s] = src_map(js)
                rows[c, co:co + rs * 128] = blk.reshape(-1)
        return rows, blocks, bins_list

    # ---- tree ----
    rows_tr, blocks_tr, bins_tr = consumer_rows_nodes(
        R_tn, t_order, t_starts, lambda js: js, ZT, t_cnt)
    meta["tr"] = build_2hop(rows_tr, TREE_PAD, blocks_tr)
    meta["tr_bins"] = bins_tr
    meta["tr_zero_bins"] = [(ub // NBINS_N, ub % NBINS_N)
                            for ub in range(SUP * NBINS_N) if R_tn[ub] == 0]

    # ---- m ----
    rows_m, blocks_m, bins_m = consumer_rows_nodes(
        R_m, e_order, e_starts, lambda js: new_id[js], ZR, n_deg)
    meta["m"] = build_2hop(rows_m, E_ALL, blocks_m)
    meta["m_bins"] = bins_m
    meta["m_zero_bins"] = [(ub // NBINS_N, ub % NBINS_N)
                           for ub in range(SUP * NBINS_N) if R_m[ub] == 0]

    # ---- per-core float layouts (pure permutations of inputs) ----
    per_core = []
    inv_cnt = (1.0 / np.maximum(counts_g, 1.0)).astype(np.float32)
    for c in range(CORES):
        pc = {}
        featT = np.zeros((AF + BF, EPC), np.float32)
        v = slots[c] >= 0
        featT[:AF, v] = node_x[edge_src[slots[c][v]]].T
        featT[AF:, v] = bond_x[slots[c][v]].T
        pc["featT"] = featT

        featTn = np.zeros((AF + 1, SUP * NPS), np.float32)
        spool = np.zeros((SUP * NPS, 128), np.float32)
        for u in range(SUP):
            sl = nslot[c, u]
            vv = sl >= 0
            base = u * NPS
            featTn[:AF, base:base + NPS][:, vv] = node_x[sl[vv]].T
            featTn[AF, base:base + NPS][vv] = 1.0
            gl = graph_ids[np.clip(sl, 0, None)] - (c * GPC + u * 128)
            idxs = np.nonzero(vv)[0]
            spool[base + idxs, gl[idxs]] = inv_cnt[graph_ids[sl[idxs]]]
        pc["featTn"] = featTn
        pc["spool"] = spool
        # features of the lg hop-1 rows (chunk-position order) for direct
        # on-device computation of the iter-1 chunk contents (skips AG0)
        i1 = meta["lg"]["hop1_idx"][c][:16].T.reshape(-1).astype(np.int64)
        rows_glob = np.zeros(len(i1), np.int64)
        pos = 0
        for (t, wlo, n_pad, c1off) in meta["lg"]["hop1_calls"]:
            iv = i1[c1off * 16: c1off * 16 + n_pad]
            rows_glob[c1off * 16: c1off * 16 + n_pad] = wlo + iv
            pos += n_pad
        # map global msg row -> original edge id (pads -> -1)
        row2edge = np.full(E_ALL, -1, np.int64)
        for cc in range(CORES):
            vv = slots[cc] >= 0
            row2edge[cc * EPC + np.nonzero(vv)[0]] = slots[cc][vv]
        eid = row2edge[np.clip(rows_glob, 0, E_ALL - 1)]
        fh1 = np.zeros((AF + BF, len(i1)), np.float32)
        ve = eid >= 0
        fh1[:AF, ve] = node_x[edge_src[eid[ve]]].T
        fh1[AF:, ve] = bond_x[eid[ve]].T
        pc["featH1"] = fh1
        pc["lg_h1"] = meta["lg"]["hop1_idx"][c]
        pc["lg_h2"] = meta["lg"]["hop2_idx"][c]
        pc["bg_h1"] = meta["bg"]["hop1_idx"][c]
        pc["bg_h2"] = meta["bg"]["hop2_idx"][c]
        pc["tr_h1"] = meta["tr"]["hop1_idx"][c]
        pc["tr_h2"] = meta["tr"]["hop2_idx"][c]
        pc["m_h1"] = meta["m"]["hop1_idx"][c]
        pc["m_h2"] = meta["m"]["hop2_idx"][c]
        per_core.append(pc)

    return per_core, meta


"""JT-MPN GNN kernel v2: 2-hop dma_gather message passing on 8 trn2 cores.

Per iteration: hop-1 window dma_gathers pack needed msg rows into SBUF
chunks (int16 indices), hop-2 SBUF-source transpose dma_gather re-reads
them in consumer (bin, round, slot) order directly in TT layout; DVE sums
rounds; PE does the W_h matmuls; DVE adds input2; ACT applies relu.
AllGather (bf16) between iterations. All float math on device.
"""
import concourse.bacc as bacc
import concourse.bass as bass
import concourse.mybir as mybir
import concourse.tile as tile

F32 = mybir.dt.float32
BF16 = mybir.dt.bfloat16
I16 = mybir.dt.int16
AluOp = mybir.AluOpType
Act = mybir.ActivationFunctionType


def build(meta, stub_collectives=False, n_iters=DEPTH - 1):
    NBINS_N = meta["NBINS_N"]
    NPS = meta["NPS"]
    BPC = meta["BPC"]
    B_ALL = meta["B_ALL"]
    lg, bg, tr, m = meta["lg"], meta["bg"], meta["tr"], meta["m"]
    SW1 = max(bg["c1_cols"], tr["c1_cols"], m["c1_cols"])
    SW2 = max(bg["c2_cols"], tr["c2_cols"], m["c2_cols"])

    nc = bacc.Bacc("TRN2", target_bir_lowering=False, debug=False)

    # ---- external IO ----
    featT_d = nc.dram_tensor("featT", [AF + BF, EPC], F32, kind="ExternalInput")
    featTn_d = nc.dram_tensor("featTn", [AF + 1, SUP * NPS], F32, kind="ExternalInput")
    spool_d = nc.dram_tensor("spool", [SUP * NPS, 128], F32, kind="ExternalInput")
    tree_d = nc.dram_tensor("tree_bf", [TREE_PAD, H], BF16, kind="ExternalInput")
    Wi_d = nc.dram_tensor("Wi", [AF + BF, H], F32, kind="ExternalInput")
    Wh_d = nc.dram_tensor("Wh_bf", [H, H], BF16, kind="ExternalInput")
    WoT_d = nc.dram_tensor("WoTop", [AF + 1, H], F32, kind="ExternalInput")
    Wob_d = nc.dram_tensor("Wob_bf", [H, H], BF16, kind="ExternalInput")
    identb_d = nc.dram_tensor("ident_bf", [128, 128], BF16, kind="ExternalInput")
    FH1 = 16 * lg["c1_cols"]
    featH1_d = nc.dram_tensor("featH1", [AF + BF, FH1], F32, kind="ExternalInput")
    idx_d = {}
    for nm, tab in (("lg", lg), ("bg", bg), ("tr", tr), ("m", m)):
        idx_d[nm + "_h1"] = nc.dram_tensor(nm + "_h1", [128, tab["c1_cols"]], I16,
                                           kind="ExternalInput")
        idx_d[nm + "_h2"] = nc.dram_tensor(nm + "_h2", [128, tab["c2_cols"]], I16,
                                           kind="ExternalInput")
    gout = nc.dram_tensor("gout", [GPC, H], F32, kind="ExternalOutput")

    with tile.TileContext(nc) as tc:
        with tc.tile_pool(name="dram", bufs=1, space="DRAM") as dram, \
             tc.tile_pool(name="const", bufs=1) as cpool, \
             tc.tile_pool(name="idxp", bufs=1) as idxp, \
             tc.tile_pool(name="chunk", bufs=2) as chp, \
             tc.tile_pool(name="tt", bufs=2) as ttp, \
             tc.tile_pool(name="stream", bufs=3) as stp, \
             tc.tile_pool(name="scratch", bufs=3) as scp, \
             tc.tile_pool(name="psum_b", bufs=3, space="PSUM") as ppb, \
             tc.tile_pool(name="psum_g", bufs=1, space="PSUM") as ppg:

            beta_shard = dram.tile([BPC, H], BF16)
            beta_full = dram.tile([B_ALL, H], BF16,
                                  addr_space=("Local" if stub_collectives else "Shared"))
            in2_d = dram.tile([EPC, H], BF16)
            gamma_d = dram.tile([SUP * NPS, H], BF16)
            msg_shard = dram.tile([EPC, H], BF16)
            msg_fulls = [dram.tile([E_ALL, H], BF16,
                                   addr_space=("Local" if stub_collectives else "Shared"),
                                   name=f"msg_full_{k}") for k in range(n_iters)]

            # ---- constants ----
            wi_sb = cpool.tile([AF + BF, H], F32)
            nc.sync.dma_start(wi_sb[:], Wi_d[:])
            whA = cpool.tile([128, H], BF16)
            whB = cpool.tile([128, H], BF16)
            nc.sync.dma_start(whA[:], Wh_d[0:128, :])
            nc.sync.dma_start(whB[:], Wh_d[128:256, :])
            wot = cpool.tile([AF + 1, H], F32)
            nc.sync.dma_start(wot[:], WoT_d[:])
            wobA = cpool.tile([128, H], BF16)
            wobB = cpool.tile([128, H], BF16)
            nc.sync.dma_start(wobA[:], Wob_d[0:128, :])
            nc.sync.dma_start(wobB[:], Wob_d[128:256, :])
            idb = cpool.tile([128, 128], BF16)
            nc.sync.dma_start(idb[:], identb_d[:])

            lg_h1 = idxp.tile([128, lg["c1_cols"]], I16)
            nc.sync.dma_start(lg_h1[:], idx_d["lg_h1"][:])
            lg_h2 = idxp.tile([128, lg["c2_cols"]], I16)
            nc.sync.dma_start(lg_h2[:], idx_d["lg_h2"][:])

            def load_idx(nm, tab):
                h1 = idxp.tile([128, SW1], I16, tag="sw1")
                nc.sync.dma_start(h1[:, :tab["c1_cols"]], idx_d[nm + "_h1"][:])
                h2 = idxp.tile([128, SW2], I16, tag="sw2")
                nc.sync.dma_start(h2[:, :tab["c2_cols"]], idx_d[nm + "_h2"][:])
                return h1, h2

            def hop1_chunks(tab, h1, table_dram, table_rows):
                calls_by_chunk = {}
                for (t, wlo, n_pad, c1off) in tab["hop1_calls"]:
                    calls_by_chunk.setdefault(t, []).append((wlo, n_pad, c1off))
                for t in range(tab["n_chunks"]):
                    B = tab["hop1_blocks"][t]
                    ct = chp.tile([128, CHUNK_ROWS // 128, H], BF16, tag="ct")
                    o = 0
                    for (wlo, n_pad, c1off) in calls_by_chunk[t]:
                        wlen = min(WIN, table_rows - wlo)
                        nb = n_pad // 128
                        nc.gpsimd.dma_gather(
                            out_ap=ct[:, o:o + nb, :],
                            in_ap=table_dram[wlo:wlo + wlen, :],
                            idxs_ap=h1[:, c1off:c1off + n_pad // 16],
                            num_idxs=n_pad, num_idxs_reg=n_pad, elem_size=H)
                        o += nb
                    assert o == B
                    yield t, ct, B

            def hop2(h2, ct, call):
                (t, n, c2off, outoff) = call
                flat = ttp.tile([128, 2 * HOP2_MAX], BF16, tag="tt")
                ttt = flat[:, 0:2 * n].rearrange("p (k n) -> p k n", k=2)
                nc.gpsimd.dma_gather(
                    out_ap=ttt,
                    in_ap=ct[:],
                    idxs_ap=h2[:, c2off:c2off + n // 16],
                    num_idxs=n, num_idxs_reg=n, elem_size=H,
                    transpose=True,
                    sbuf_tokens_per_rank=128,
                    sbuf_free_dim_per_rank=H * 2)
                return ttt

            def reduce_rounds(ttt, c0, R):
                acc = ttt[:, :, c0:c0 + 128]
                for r in range(1, R):
                    nc.vector.tensor_tensor(
                        out=acc, in0=acc,
                        in1=ttt[:, :, c0 + r * 128:c0 + (r + 1) * 128],
                        op=AluOp.add)
                return acc

            def allgather(src, dst, shard_rows):
                if stub_collectives:
                    for rep in range(2):
                        lo = (rep * shard_rows) % max(dst.shape[0] - shard_rows, 1) \
                            if dst.shape[0] > shard_rows else 0
                        nc.sync.dma_start(dst[lo:lo + shard_rows, :], src[:])
                    return
                nc.gpsimd.collective_compute(
                    "AllGather", AluOp.bypass,
                    replica_groups=[list(range(CORES))],
                    ins=[src[:].opt()], outs=[dst[:].opt()])

            # ================= phase 0b: tree -> beta/gamma =================
            tr_h1, tr_h2 = load_idx("tr", tr)
            tr_calls = tr["hop2_calls"]
            zgb = scp.tile([128, H], BF16, tag="zgb")
            nc.vector.memset(zgb[:], 0.0)
            for (u, b) in meta["tr_zero_bins"]:
                base = u * NPS + b * 128
                nc.sync.dma_start(beta_shard[base:base + 128, :], zgb[:])
                nc.sync.dma_start(gamma_d[base:base + 128, :], zgb[:])
            tr_chunks = hop1_chunks(tr, tr_h1, tree_d, TREE_PAD)
            cur = {"t": -1, "ct": None, "tt": None, "rng": (0, 0), "ci": 0}

            def advance_to(tab, calls, h2, chunks_iter, coff):
                """Ensure the hop-2 call containing coff is current."""
                while not (cur["rng"][0] <= coff < cur["rng"][1]):
                    call = calls[cur["ci"]]
                    while cur["t"] < call[0]:
                        t_, ct_, B_ = next(chunks_iter)
                        cur["t"] = t_
                        cur["ct"] = ct_
                    cur["tt"] = hop2(h2, cur["ct"], call)
                    cur["rng"] = (call[3], call[3] + call[1])
                    cur["ci"] += 1
                return cur["tt"], cur["rng"][0]

            acc_hold = {}
            for (u, b, coff, R, first, last) in meta["tr_bins"]:
                ttt, o0 = advance_to(tr, tr_calls, tr_h2, tr_chunks, coff)
                acc = reduce_rounds(ttt, coff - o0, R)
                if not first:
                    nc.vector.tensor_tensor(out=acc_hold[(u, b)],
                                            in0=acc_hold[(u, b)], in1=acc,
                                            op=AluOp.add)
                else:
                    acc_hold[(u, b)] = acc
                if not last:
                    continue
                acc = acc_hold.pop((u, b))
                base = u * NPS + b * 128
                pb_ = ppb.tile([128, H], F32, tag="ps")
                nc.tensor.matmul(pb_[:], lhsT=acc[:, 0, :], rhs=whA[:],
                                 start=True, stop=False)
                nc.tensor.matmul(pb_[:], lhsT=acc[:, 1, :], rhs=whB[:],
                                 start=False, stop=True)
                bout = scp.tile([128, H], BF16, tag="bout")
                nc.scalar.activation(bout[:], pb_[:], Act.Copy)
                nc.sync.dma_start(beta_shard[base:base + 128, :], bout[:])
                pg_ = ppb.tile([128, H], F32, tag="ps")
                nc.tensor.matmul(pg_[:], lhsT=acc[:, 0, :], rhs=wobA[:],
                                 start=True, stop=False)
                nc.tensor.matmul(pg_[:], lhsT=acc[:, 1, :], rhs=wobB[:],
                                 start=False, stop=True)
                gt = scp.tile([128, H], BF16, tag="gt")
                nc.scalar.activation(gt[:], pg_[:], Act.Copy)
                nc.sync.dma_start(gamma_d[base:base + 128, :], gt[:])
            for _ in tr_chunks:
                pass
            allgather(beta_shard, beta_full, BPC)

            # ================= phase 0c: input2 =================
            bg_h1, bg_h2 = load_idx("bg", bg)
            bg_calls = bg["hop2_calls"]
            ci = 0
            for t, ct, B in hop1_chunks(bg, bg_h1, beta_full, B_ALL):
                while ci < len(bg_calls) and bg_calls[ci][0] == t:
                    call = bg_calls[ci]
                    (tt_, n, c2off, outoff) = call
                    ttt = hop2(bg_h2, ct, call)
                    nb = n // 128
                    for j0 in range(0, nb, 8):
                        jn = min(8, nb - j0)
                        lo = outoff + j0 * 128
                        ft = stp.tile([AF + BF, 8 * 128], F32, tag="ft")
                        nc.sync.dma_start(ft[:, :jn * 128],
                                          featT_d[:, lo:lo + jn * 128])
                        i2b = stp.tile([128, 8, H], BF16, tag="msgb")
                        for j in range(jn):
                            jj = j0 + j
                            pt = ppb.tile([128, H], BF16, tag="pt")
                            nc.tensor.transpose(
                                pt[:, 0:128], ttt[:, 0, jj * 128:(jj + 1) * 128], idb[:])
                            nc.tensor.transpose(
                                pt[:, 128:256], ttt[:, 1, jj * 128:(jj + 1) * 128], idb[:])
                            brow = scp.tile([128, H], BF16, tag="brow")
                            nc.scalar.activation(brow[:], pt[:], Act.Copy)
                            pa = ppb.tile([128, H], F32, tag="ps")
                            nc.tensor.matmul(pa[:], lhsT=ft[:, j * 128:(j + 1) * 128],
                                             rhs=wi_sb[:], start=True, stop=True)
                            nc.vector.tensor_tensor(out=i2b[:, j, :], in0=pa[:],
                                                    in1=brow[:], op=AluOp.add)
                        nc.sync.dma_start(
                            in2_d[lo:lo + jn * 128, :]
                            .rearrange("(m p) d -> p m d", p=128), i2b[:, :jn, :])
                    ci += 1

            # ================= BP iterations =================
            lg_calls = lg["hop2_calls"]
            lg_bin_list = meta["lg_bins"]   # (b, coff, R)
            nz_bins = len(lg_bin_list)
            for it in range(n_iters):
                src_full = msg_fulls[it]
                ci = 0
                for t, ct, B in hop1_chunks(lg, lg_h1, src_full, E_ALL):
                    while ci < len(lg_calls) and lg_calls[ci][0] == t:
                        call = lg_calls[ci]
                        (tt_, n, c2off, outoff) = call
                        ttt = hop2(lg_h2, ct, call)
                        bins_in = [x for x in lg_bin_list
                                   if outoff <= x[1] < outoff + n]
                        for j0 in range(0, len(bins_in), 8):
                            sub = bins_in[j0:j0 + 8]
                            b_first = sub[0][0]
                            jn = len(sub)
                            i2l = stp.tile([128, 8, H], BF16, tag="i2l")
                            nc.sync.dma_start(
                                i2l[:, :jn, :],
                                in2_d[b_first * 128:(b_first + jn) * 128, :]
                                .rearrange("(m p) d -> p m d", p=128))
                            msgb = stp.tile([128, 8, H], BF16, tag="msgb")
                            for j, (b, coff, R) in enumerate(sub):
                                assert b == b_first + j
                                acc = reduce_rounds(ttt, coff - outoff, R)
                                pb_ = ppb.tile([128, H], F32, tag="ps")
                                nc.tensor.matmul(pb_[:], lhsT=acc[:, 0, :],
                                                 rhs=whA[:], start=True, stop=False)
                                nc.tensor.matmul(pb_[:], lhsT=acc[:, 1, :],
                                                 rhs=whB[:], start=False, stop=True)
                                tmp = scp.tile([128, H], BF16, tag="tmp")
                                nc.vector.tensor_tensor(out=tmp[:], in0=pb_[:],
                                                        in1=i2l[:, j, :], op=AluOp.add)
                                nc.scalar.activation(msgb[:, j, :], tmp[:], Act.Relu)
                            nc.sync.dma_start(
                                msg_shard[b_first * 128:(b_first + jn) * 128, :]
                                .rearrange("(m p) d -> p m d", p=128), msgb[:, :jn, :])
                        ci += 1
                # zero-R tail bins: msg = relu(in2)
                for g0 in range(nz_bins, NBINS_E, 8):
                    gsz = min(8, NBINS_E - g0)
                    i2l = stp.tile([128, 8, H], BF16, tag="i2l")
                    nc.sync.dma_start(
                        i2l[:, :gsz, :],
                        in2_d[g0 * 128:(g0 + gsz) * 128, :]
                        .rearrange("(m p) d -> p m d", p=128))
                    msgb = stp.tile([128, 8, H], BF16, tag="msgb")
                    for j in range(gsz):
                        nc.scalar.activation(msgb[:, j, :], i2l[:, j, :], Act.Relu)
                    nc.sync.dma_start(
                        msg_shard[g0 * 128:(g0 + gsz) * 128, :]
                        .rearrange("(m p) d -> p m d", p=128), msgb[:, :gsz, :])
                allgather(msg_shard, msg_fulls[it], EPC)

            # ================= final =================
            m_h1, m_h2 = load_idx("m", m)
            m_calls = m["hop2_calls"]
            m_bin_map = {(u, b): (coff, R) for (u, b, coff, R) in meta["m_bins"]}
            m_tiles = {}
            for t, ct, B in hop1_chunks(m, m_h1, msg_fulls[n_iters], E_ALL):
                m_tiles[t] = ct
            ci = 0
            cur_tt = None
            cur_range = (0, 0)
            for u in range(SUP):
                pg = ppg.tile([128, H], F32, tag="pg")
                for b in range(NBINS_N):
                    base = u * NPS + b * 128
                    ftn = scp.tile([AF + 1, 128], F32, tag="ftn")
                    nc.sync.dma_start(ftn[:], featTn_d[:, base:base + 128])
                    pc_ = ppb.tile([128, H], F32, tag="ps")
                    if (u, b) in m_bin_map:
                        coff, R = m_bin_map[(u, b)]
                        if not (cur_range[0] <= coff < cur_range[1]):
                            call = m_calls[ci]
                            assert call[3] == coff, (call, coff)
                            cur_tt = hop2(m_h2, m_tiles[call[0]], call)
                            cur_range = (call[3], call[3] + call[1])
                            ci += 1
                        acc = reduce_rounds(cur_tt, coff - cur_range[0], R)
                        nc.tensor.matmul(pc_[:], lhsT=ftn[:], rhs=wot[:],
                                         start=True, stop=False)
                        nc.tensor.matmul(pc_[:], lhsT=acc[:, 0, :], rhs=wobA[:],
                                         start=False, stop=False)
                        nc.tensor.matmul(pc_[:], lhsT=acc[:, 1, :], rhs=wobB[:],
                                         start=False, stop=True)
                    else:
                        nc.tensor.matmul(pc_[:], lhsT=ftn[:], rhs=wot[:],
                                         start=True, stop=True)
                    gml = scp.tile([128, H], BF16, tag="gml")
                    nc.sync.dma_start(gml[:], gamma_d[base:base + 128, :])
                    hsum = scp.tile([128, H], F32, tag="hsum")
                    nc.vector.tensor_tensor(out=hsum[:], in0=pc_[:], in1=gml[:],
                                            op=AluOp.add)
                    h = scp.tile([128, H], F32, tag="h")
                    nc.scalar.activation(h[:], hsum[:], Act.Relu)
                    sp = scp.tile([128, 128], F32, tag="sp")
                    nc.sync.dma_start(sp[:], spool_d[base:base + 128, :])
                    nc.tensor.matmul(pg[:], lhsT=sp[:], rhs=h[:],
                                     start=(b == 0), stop=(b == NBINS_N - 1))
                go = scp.tile([128, H], F32, tag="go")
                nc.scalar.activation(go[:], pg[:], Act.Copy)
                nc.sync.dma_start(gout[u * 128:(u + 1) * 128, :], go[:])

    nc.finalize()
    return nc


def make_in_maps(inputs, per_core, meta):
    import ml_dtypes
    W_i = np.asarray(inputs["W_i"], np.float32)
    W_h = np.asarray(inputs["W_h"], np.float32)
    W_o = np.asarray(inputs["W_o"], np.float32)
    b_o = np.asarray(inputs["b_o"], np.float32)
    tree_pad = np.zeros((TREE_PAD, H), ml_dtypes.bfloat16)
    tree_pad[:60000] = np.asarray(inputs["tree_mess"], np.float32
                                  ).astype(ml_dtypes.bfloat16)
    shared = {
        "tree_bf": tree_pad,
        "Wi": W_i,
        "Wh_bf": W_h.astype(ml_dtypes.bfloat16),
        "WoTop": np.concatenate([W_o[:AF], b_o[None, :]], 0),
        "Wob_bf": W_o[AF:].astype(ml_dtypes.bfloat16),
        "ident_bf": np.eye(128).astype(ml_dtypes.bfloat16),
    }
    maps = []
    for c in range(CORES):
        pc = per_core[c]
        mp = dict(shared)
        mp["featT"] = pc["featT"]
        mp["featH1"] = pc["featH1"]
        mp["featTn"] = pc["featTn"]
        mp["spool"] = pc["spool"]
        for nm in ("lg", "bg", "tr", "m"):
            mp[nm + "_h1"] = pc[nm + "_h1"]
            mp[nm + "_h2"] = pc[nm + "_h2"]
        maps.append({k: np.ascontiguousarray(v) for k, v in mp.items()})
    return maps


_BUILD_CACHE = {}


def kernel(**inputs):
    from concourse import bass_utils
    per_core, meta = preprocess(inputs)
    key = (meta["lg"]["c1_cols"], meta["lg"]["c2_cols"], meta["m"]["c1_cols"],
           meta["bg"]["c1_cols"], meta["tr"]["c1_cols"], meta["NBINS_N"])
    nc = _BUILD_CACHE.get(key)
    if nc is None:
        nc = build(meta)
        _BUILD_CACHE[key] = nc
    in_maps = make_in_maps(inputs, per_core, meta)
    res = bass_utils.run_bass_kernel_spmd(nc, in_maps, core_ids=list(range(CORES)))
    out = np.concatenate([res.results[c]["gout"] for c in range(CORES)], axis=0)
    return out.astype(np.float32)


# revision 12
# speedup vs baseline: 1.0792x; 1.0009x over previous
"""JT-MPN GNN kernel for 8 trn2 NeuronCores (self-contained).

Two-hop dma_gather message passing: hop-1 packs needed message rows from
int16-addressable 32K-row windows of the AllGathered message table into
SBUF chunks; hop-2 re-gathers them SBUF->SBUF in consumer (bin, round,
slot) order directly in transposed (TT) layout. DVE sums rounds, PE runs
the W_h matmuls (bf16, f32 PSUM), ACT applies relu. bf16 AllGather
between the three BP iterations; graph mean-pool via a selection matmul.
"""

import numpy as np

N_NODES = 150000
N_EDGES = 300000
H = 256
DEPTH = 4
AF = 35
BF = 5
NG = 2048
CORES = 8

EPC = 37504               # edges per core (293 bins * 128)
NBINS_E = EPC // 128      # 293
GPC = NG // CORES         # 256
SUP = 2
ZR = 37500                # global msg row guaranteed zero
E_ALL = CORES * EPC       # 300032
WIN = 32768

TREE_PAD = 60416
ZT = 60000                # zero row in padded tree table

CHUNK_ROWS = 12288        # hop-1 chunk tile rows (B=96 blocks)
HOP2_MAX = 768            # transpose-mode ucode ring cap (1024 crashes)
H1_MAX = 1024
SEG_R = HOP2_MAX // 128   # max rounds per consumer segment


def _group_by(dst, n_groups):
    order = np.argsort(dst, kind="stable")
    counts = np.bincount(dst, minlength=n_groups)
    starts = np.zeros(n_groups + 1, dtype=np.int64)
    np.cumsum(counts, out=starts[1:])
    return order, starts


def wrap_idx(vals, cols):
    """[n] ints -> wrapped [128, cols] int16 (8x replicated); pad -1.
    Index i lives at [i%16, i//16]."""
    flat = np.full(cols * 16, -1, np.int16)
    flat[:len(vals)] = vals.astype(np.int16)
    w = np.ascontiguousarray(flat.reshape(cols, 16).T)
    return np.tile(w, (8, 1))


def build_2hop(src_rows, table_rows, block_sizes,
               chunk_rows=CHUNK_ROWS, hop2_max=HOP2_MAX):
    """Unified 2-hop tables for all cores.

    src_rows: [CORES, n_cons] global source row per consumer column.
    block_sizes: per atomic consumer block (bin), each multiple of 128.

    Returns dict:
      hop1_calls: list of (chunk, win_base, n_pad, col_off) ; n_valid is per
        core and encoded by -1 padding in idx (num_idxs_reg: use n_pad minus
        trailing -1 count? -> device passes per-core reg via ... ) NOTE:
        num_idxs_reg must be a compile-time constant in the unified program,
        so we pass n_pad and set padded idx entries to ZR-in-window when the
        window contains a guaranteed-zero row, else repeat the last valid
        index (harmless extra gather).
      hop1_idx: [CORES][128, C1] int16
      hop1_blocks: per chunk block count (unified)
      hop2_calls: list of (chunk, n, col_off, out_off)
      hop2_idx: [CORES][128, C2] int16
      n_chunks
    """
    n_wins = (table_rows + WIN - 1) // WIN
    nb = len(block_sizes)
    block_start = np.zeros(nb + 1, np.int64)
    np.cumsum(block_sizes, out=block_start[1:])
    n_cons = int(block_start[-1])
    assert src_rows.shape == (CORES, n_cons)

    # --- chunk assignment (unified): estimate per-core unique counts ---
    chunks = []
    cur_first = 0
    cur_rows = [set() for _ in range(CORES)]
    for b in range(nb):
        sl = slice(block_start[b], block_start[b + 1])
        newmax = 0
        for c in range(CORES):
            s = set(src_rows[c, sl].tolist())
            newmax = max(newmax, len(cur_rows[c] | s))
        if newmax > chunk_rows - 128 * n_wins and b > cur_first:
            chunks.append((cur_first, b))
            cur_first = b
            cur_rows = [set(src_rows[c, sl].tolist()) for c in range(CORES)]
        else:
            for c in range(CORES):
                cur_rows[c] |= set(src_rows[c, sl].tolist())
    chunks.append((cur_first, nb))

    hop1_calls = []
    hop1_vals = [[] for _ in range(CORES)]   # list of (colpos, array)
    hop1_blocks = []
    hop2_calls = []
    hop2_vals = [[] for _ in range(CORES)]
    c1_off = 0
    c2_off = 0
    for t, (b0, b1) in enumerate(chunks):
        sl = slice(block_start[b0], block_start[b1])
        uniqs = [np.unique(src_rows[c, sl]) for c in range(CORES)]
        poss = [np.full(len(u), -1, np.int64) for u in uniqs]
        p = 0
        for w in range(n_wins):
            wlo, whi = w * WIN, min((w + 1) * WIN, table_rows)
            sels = [(u >= wlo) & (u < whi) for u in uniqs]
            n_valid = [int(s.sum()) for s in sels]
            n_max = max(n_valid)
            if n_max == 0:
                continue
            n_pad = (n_max + 127) // 128 * 128
            for c in range(CORES):
                poss[c][sels[c]] = p + np.arange(n_valid[c])
                v = uniqs[c][sels[c]] - wlo
                if len(v) == 0:
                    v = np.array([0], np.int64)  # dummy row in window
                pad = np.full(n_pad - len(v), v[-1], np.int64)
                hop1_vals[c].append(np.concatenate([v, pad]))
            # split into sub-calls of <= H1_MAX indices (ucode ring cap)
            done = 0
            while done < n_pad:
                sub = min(H1_MAX, n_pad - done)
                hop1_calls.append((t, wlo, sub, c1_off))
                c1_off += sub // 16
                done += sub
            p += n_pad
        assert p <= chunk_rows, (p, chunk_rows)
        hop1_blocks.append(p // 128)

        h2 = []
        for c in range(CORES):
            j = np.searchsorted(uniqs[c], src_rows[c, sl])
            assert (uniqs[c][j] == src_rows[c, sl]).all()
            h2.append(poss[c][j])
            assert (poss[c][j] >= 0).all()
        # split into calls at block boundaries
        local_bs = block_start[b0:b1 + 1] - block_start[b0]
        bi = 0
        cstart = 0
        total = int(local_bs[-1])
        while cstart < total:
            cend = cstart
            while bi < b1 - b0 and local_bs[bi + 1] - cstart <= hop2_max:
                bi += 1
                cend = int(local_bs[bi])
            assert cend > cstart
            n = cend - cstart
            for c in range(CORES):
                hop2_vals[c].append(h2[c][cstart:cend])
            hop2_calls.append((t, n, c2_off, int(block_start[b0] + cstart)))
            c2_off += (n + 15) // 16
            cstart = cend

    hop1_idx = [wrap_idx(np.concatenate(hop1_vals[c]), max(c1_off, 1))
                for c in range(CORES)]
    hop2_idx = [wrap_idx(np.concatenate(hop2_vals[c]), max(c2_off, 1))
                for c in range(CORES)]
    return dict(hop1_calls=hop1_calls, hop1_idx=hop1_idx,
                hop1_blocks=hop1_blocks, hop2_calls=hop2_calls,
                hop2_idx=hop2_idx, n_chunks=len(chunks),
                c1_cols=max(c1_off, 1), c2_cols=max(c2_off, 1))


def preprocess(inputs):
    edge_src = np.asarray(inputs["edge_src"], dtype=np.int64)
    edge_dst = np.asarray(inputs["edge_dst"], dtype=np.int64)
    lg_src = np.asarray(inputs["lg_src"], dtype=np.int64)
    lg_dst = np.asarray(inputs["lg_dst"], dtype=np.int64)
    tgt_nodes = np.asarray(inputs["tgt_nodes"], dtype=np.int64)
    graph_ids = np.asarray(inputs["graph_ids"], dtype=np.int64)
    node_x = np.asarray(inputs["node_x"], dtype=np.float32)
    bond_x = np.asarray(inputs["bond_x"], dtype=np.float32)

    meta = {}

    # ---- edge -> core (snake deal by lg in-degree desc) ----
    deg = np.bincount(lg_dst, minlength=N_EDGES)
    order = np.argsort(-deg, kind="stable")
    cyc = np.arange(N_EDGES) % (2 * CORES)
    core_of_rank = np.where(cyc < CORES, cyc, 2 * CORES - 1 - cyc)
    slots = np.full((CORES, EPC), -1, dtype=np.int64)
    for c in range(CORES):
        mine = order[core_of_rank == c]
        slots[c, :len(mine)] = mine
    new_id = np.full(N_EDGES, -1, dtype=np.int64)
    for c in range(CORES):
        valid = slots[c] >= 0
        new_id[slots[c][valid]] = c * EPC + np.nonzero(valid)[0]
    assert (new_id >= 0).all()
    meta["slots"] = slots
    meta["new_id"] = new_id

    lg_order, lg_starts = _group_by(lg_dst, N_EDGES)
    slot_deg = np.where(slots >= 0, deg[np.clip(slots, 0, None)], 0)
    R_lg = slot_deg.reshape(CORES, NBINS_E, 128).max(axis=2).max(axis=0)
    meta["R_lg"] = R_lg

    # ---- nodes ----
    g_starts = np.zeros(NG + 1, dtype=np.int64)
    np.cumsum(np.bincount(graph_ids, minlength=NG), out=g_starts[1:])
    n_deg = np.bincount(edge_dst, minlength=N_NODES)
    t_cnt = np.bincount(tgt_nodes, minlength=N_NODES)
    counts_g = np.bincount(graph_ids, minlength=NG).astype(np.float64)

    sup_nodes = []
    for c in range(CORES):
        for u in range(SUP):
            g0 = c * GPC + u * 128
            nodes = np.arange(g_starts[g0], g_starts[g0 + 128])
            nodes = nodes[np.argsort(-n_deg[nodes], kind="stable")]
            sup_nodes.append(nodes)
    NBINS_N = int(max((len(x) + 127) // 128 for x in sup_nodes))
    NPS = NBINS_N * 128
    meta["NBINS_N"] = NBINS_N
    meta["NPS"] = NPS
    nslot = np.full((CORES, SUP, NPS), -1, dtype=np.int64)
    for c in range(CORES):
        for u in range(SUP):
            nodes = sup_nodes[c * SUP + u]
            nslot[c, u, :len(nodes)] = nodes
    meta["nslot"] = nslot

    BPC = SUP * NPS
    B_ALL = CORES * BPC
    beta_row_of_node = np.full(N_NODES, -1, np.int64)
    for c in range(CORES):
        sl = nslot[c].reshape(-1)
        v = sl >= 0
        beta_row_of_node[sl[v]] = c * BPC + np.nonzero(v)[0]
    assert (beta_row_of_node >= 0).all()
    meta["BPC"] = BPC
    meta["B_ALL"] = B_ALL
    pad_pos = np.nonzero(nslot[0].reshape(-1) < 0)[0]
    ZB = int(pad_pos[0]) if len(pad_pos) else 0
    meta["ZB"] = ZB

    slot_nd = np.where(nslot >= 0, n_deg[np.clip(nslot, 0, None)], 0)
    slot_nt = np.where(nslot >= 0, t_cnt[np.clip(nslot, 0, None)], 0)
    R_m = slot_nd.reshape(CORES, SUP * NBINS_N, 128).max(axis=2).max(axis=0)
    R_tn = slot_nt.reshape(CORES, SUP * NBINS_N, 128).max(axis=2).max(axis=0)
    meta["R_m"] = R_m
    meta["R_tn"] = R_tn

    e_order, e_starts = _group_by(edge_dst, N_NODES)
    t_order, t_starts = _group_by(tgt_nodes, N_NODES)

    def consumer_rows_edges(Rs, order_, starts_, src_map, zero_row, degs):
        """Build [CORES, n_cons] consumer source rows for edge bins.
        Segments of <= SEG_R rounds per bin (ucode call cap).
        bins_list entries: (bin, coloff, Rseg, first, last)."""
        bins_list = []
        blocks = []
        coff = 0
        for b in range(NBINS_E):
            R = int(Rs[b])
            if R == 0:
                continue
            r0 = 0
            while r0 < R:
                rs = min(SEG_R, R - r0)
                bins_list.append((b, coff, rs, r0 == 0, r0 + rs == R))
                blocks.append(rs * 128)
                coff += rs * 128
                r0 += rs
        n_cons = coff
        rows = np.full((CORES, n_cons), zero_row, np.int64)
        seg_round0 = {}
        r_run = {}
        for (b, co, rs, first, last) in bins_list:
            if first:
                r_run[b] = 0
            seg_round0[(b, co)] = r_run[b]
            r_run[b] += rs
        for c in range(CORES):
            for (b, co, rs, first, last) in bins_list:
                r0 = seg_round0[(b, co)]
                sl = slots[c, b*128:(b+1)*128]
                blk = np.full((rs, 128), zero_row, np.int64)
                for s in range(128):
                    e = sl[s]
                    if e < 0:
                        continue
                    d = int(degs[e])
                    lo, hi = min(r0, d), min(r0 + rs, d)
                    if hi <= lo:
                        continue
                    js = order_[starts_[e] + lo:starts_[e] + hi]
                    blk[:hi - lo, s] = src_map(js)
                rows[c, co:co + rs * 128] = blk.reshape(-1)
        return rows, blocks, bins_list

    # ---- lg ----
    rows_lg, blocks_lg, bins_lg = consumer_rows_edges(
        R_lg, lg_order, lg_starts, lambda js: new_id[lg_src[js]], ZR, deg)
    meta["lg"] = build_2hop(rows_lg, E_ALL, blocks_lg)
    meta["lg_bins"] = bins_lg
    meta["lg_zero_bins"] = [b for b in range(NBINS_E) if R_lg[b] == 0]

    # ---- beta-gather: 1 round per edge bin ----
    rows_bg = np.full((CORES, EPC), ZB, np.int64)
    for c in range(CORES):
        v = slots[c] >= 0
        rows_bg[c, v] = beta_row_of_node[edge_src[slots[c][v]]]
    meta["bg"] = build_2hop(rows_bg, B_ALL, [128] * NBINS_E)

    def consumer_rows_nodes(Rs, order_, starts_, src_map, zero_row, degs):
        """bins_list entries: (u, b, coloff, Rseg, first, last)."""
        bins_list = []
        blocks = []
        coff = 0
        for ub in range(SUP * NBINS_N):
            R = int(Rs[ub])
            if R == 0:
                continue
            r0 = 0
            while r0 < R:
                rs = min(SEG_R, R - r0)
                bins_list.append((ub // NBINS_N, ub % NBINS_N, coff, rs,
                                  r0 == 0, r0 + rs == R))
                blocks.append(rs * 128)
                coff += rs * 128
                r0 += rs
        n_cons = coff
        rows = np.full((CORES, n_cons), zero_row, np.int64)
        seg_round0 = {}
        r_run = {}
        for (u, b, co, rs, first, last) in bins_list:
            if first:
                r_run[(u, b)] = 0
            seg_round0[co] = r_run[(u, b)]
            r_run[(u, b)] += rs
        for c in range(CORES):
            for (u, b, co, rs, first, last) in bins_list:
                r0 = seg_round0[co]
                sl = nslot[c, u, b*128:(b+1)*128]
                blk = np.full((rs, 128), zero_row, np.int64)
                for s in range(128):
                    vtx = sl[s]
                    if vtx < 0:
                        continue
                    d = int(degs[vtx])
                    lo, hi = min(r0, d), min(r0 + rs, d)
                    if hi <= lo:
                        continue
                    js = order_[starts_[vtx] + lo:starts_[vtx] + hi]
                    blk[:hi - lo, s] = src_map(js)
                rows[c, co:co + rs * 128] = blk.reshape(-1)
        return rows, blocks, bins_list

    # ---- tree ----
    rows_tr, blocks_tr, bins_tr = consumer_rows_nodes(
        R_tn, t_order, t_starts, lambda js: js, ZT, t_cnt)
    meta["tr"] = build_2hop(rows_tr, TREE_PAD, blocks_tr)
    meta["tr_bins"] = bins_tr
    meta["tr_zero_bins"] = [(ub // NBINS_N, ub % NBINS_N)
                            for ub in range(SUP * NBINS_N) if R_tn[ub] == 0]

    # ---- m ----
    rows_m, blocks_m, bins_m = consumer_rows_nodes(
        R_m, e_order, e_starts, lambda js: new_id[js], ZR, n_deg)
    meta["m"] = build_2hop(rows_m, E_ALL, blocks_m)
    meta["m_bins"] = bins_m
    meta["m_zero_bins"] = [(ub // NBINS_N, ub % NBINS_N)
                           for ub in range(SUP * NBINS_N) if R_m[ub] == 0]

    # ---- per-core float layouts (pure permutations of inputs) ----
    per_core = []
    inv_cnt = (1.0 / np.maximum(counts_g, 1.0)).astype(np.float32)
    for c in range(CORES):
        pc = {}
        featT = np.zeros((AF + BF, EPC), np.float32)
        v = slots[c] >= 0
        featT[:AF, v] = node_x[edge_src[slots[c][v]]].T
        featT[AF:, v] = bond_x[slots[c][v]].T
        pc["featT"] = featT

        featTn = np.zeros((AF + 1, SUP * NPS), np.float32)
        spool = np.zeros((SUP * NPS, 128), np.float32)
        for u in range(SUP):
            sl = nslot[c, u]
            vv = sl >= 0
            base = u * NPS
            featTn[:AF, base:base + NPS][:, vv] = node_x[sl[vv]].T
            featTn[AF, base:base + NPS][vv] = 1.0
            gl = graph_ids[np.clip(sl, 0, None)] - (c * GPC + u * 128)
            idxs = np.nonzero(vv)[0]
            spool[base + idxs, gl[idxs]] = inv_cnt[graph_ids[sl[idxs]]]
        pc["featTn"] = featTn
        pc["spool"] = spool
        # features of the lg hop-1 rows (chunk-position order) for direct
        # on-device computation of the iter-1 chunk contents (skips AG0)
        i1 = meta["lg"]["hop1_idx"][c][:16].T.reshape(-1).astype(np.int64)
        rows_glob = np.zeros(len(i1), np.int64)
        pos = 0
        for (t, wlo, n_pad, c1off) in meta["lg"]["hop1_calls"]:
            iv = i1[c1off * 16: c1off * 16 + n_pad]
            rows_glob[c1off * 16: c1off * 16 + n_pad] = wlo + iv
            pos += n_pad
        # map global msg row -> original edge id (pads -> -1)
        row2edge = np.full(E_ALL, -1, np.int64)
        for cc in range(CORES):
            vv = slots[cc] >= 0
            row2edge[cc * EPC + np.nonzero(vv)[0]] = slots[cc][vv]
        eid = row2edge[np.clip(rows_glob, 0, E_ALL - 1)]
        fh1 = np.zeros((AF + BF, len(i1)), np.float32)
        ve = eid >= 0
        fh1[:AF, ve] = node_x[edge_src[eid[ve]]].T
        fh1[AF:, ve] = bond_x[eid[ve]].T
        pc["featH1"] = fh1
        pc["lg_h1"] = meta["lg"]["hop1_idx"][c]
        pc["lg_h2"] = meta["lg"]["hop2_idx"][c]
        pc["bg_h1"] = meta["bg"]["hop1_idx"][c]
        pc["bg_h2"] = meta["bg"]["hop2_idx"][c]
        pc["tr_h1"] = meta["tr"]["hop1_idx"][c]
        pc["tr_h2"] = meta["tr"]["hop2_idx"][c]
        pc["m_h1"] = meta["m"]["hop1_idx"][c]
        pc["m_h2"] = meta["m"]["hop2_idx"][c]
        per_core.append(pc)

    return per_core, meta


"""JT-MPN GNN kernel v2: 2-hop dma_gather message passing on 8 trn2 cores.

Per iteration: hop-1 window dma_gathers pack needed msg rows into SBUF
chunks (int16 indices), hop-2 SBUF-source transpose dma_gather re-reads
them in consumer (bin, round, slot) order directly in TT layout; DVE sums
rounds; PE does the W_h matmuls; DVE adds input2; ACT applies relu.
AllGather (bf16) between iterations. All float math on device.
"""
import concourse.bacc as bacc
import concourse.bass as bass
import concourse.mybir as mybir
import concourse.tile as tile

F32 = mybir.dt.float32
BF16 = mybir.dt.bfloat16
I16 = mybir.dt.int16
AluOp = mybir.AluOpType
Act = mybir.ActivationFunctionType


def build(meta, stub_collectives=False, n_iters=DEPTH - 1):
    NBINS_N = meta["NBINS_N"]
    NPS = meta["NPS"]
    BPC = meta["BPC"]
    B_ALL = meta["B_ALL"]
    lg, bg, tr, m = meta["lg"], meta["bg"], meta["tr"], meta["m"]
    SW1 = max(bg["c1_cols"], tr["c1_cols"], m["c1_cols"])
    SW2 = max(bg["c2_cols"], tr["c2_cols"], m["c2_cols"])

    nc = bacc.Bacc("TRN2", target_bir_lowering=False, debug=False)

    # ---- external IO ----
    featT_d = nc.dram_tensor("featT", [AF + BF, EPC], F32, kind="ExternalInput")
    featTn_d = nc.dram_tensor("featTn", [AF + 1, SUP * NPS], F32, kind="ExternalInput")
    spool_d = nc.dram_tensor("spool", [SUP * NPS, 128], F32, kind="ExternalInput")
    tree_d = nc.dram_tensor("tree_bf", [TREE_PAD, H], BF16, kind="ExternalInput")
    Wi_d = nc.dram_tensor("Wi", [AF + BF, H], F32, kind="ExternalInput")
    Wh_d = nc.dram_tensor("Wh_bf", [H, H], BF16, kind="ExternalInput")
    WoT_d = nc.dram_tensor("WoTop", [AF + 1, H], F32, kind="ExternalInput")
    Wob_d = nc.dram_tensor("Wob_bf", [H, H], BF16, kind="ExternalInput")
    identb_d = nc.dram_tensor("ident_bf", [128, 128], BF16, kind="ExternalInput")
    FH1 = 16 * lg["c1_cols"]
    featH1_d = nc.dram_tensor("featH1", [AF + BF, FH1], F32, kind="ExternalInput")
    idx_d = {}
    for nm, tab in (("lg", lg), ("bg", bg), ("tr", tr), ("m", m)):
        idx_d[nm + "_h1"] = nc.dram_tensor(nm + "_h1", [128, tab["c1_cols"]], I16,
                                           kind="ExternalInput")
        idx_d[nm + "_h2"] = nc.dram_tensor(nm + "_h2", [128, tab["c2_cols"]], I16,
                                           kind="ExternalInput")
    gout = nc.dram_tensor("gout", [GPC, H], F32, kind="ExternalOutput")

    with tile.TileContext(nc) as tc:
        with tc.tile_pool(name="dram", bufs=1, space="DRAM") as dram, \
             tc.tile_pool(name="const", bufs=1) as cpool, \
             tc.tile_pool(name="idxp", bufs=1) as idxp, \
             tc.tile_pool(name="chunk", bufs=2) as chp, \
             tc.tile_pool(name="tt", bufs=2) as ttp, \
             tc.tile_pool(name="stream", bufs=3) as stp, \
             tc.tile_pool(name="scratch", bufs=3) as scp, \
             tc.tile_pool(name="psum_b", bufs=4, space="PSUM") as ppb, \
             tc.tile_pool(name="psum_t", bufs=2, space="PSUM") as ppt, \
             tc.tile_pool(name="psum_g", bufs=1, space="PSUM") as ppg:

            beta_shard = dram.tile([BPC, H], BF16)
            beta_full = dram.tile([B_ALL, H], BF16,
                                  addr_space=("Local" if stub_collectives else "Shared"))
            in2_d = dram.tile([EPC, H], BF16)
            gamma_d = dram.tile([SUP * NPS, H], BF16)
            msg_shard = dram.tile([EPC, H], BF16)
            msg_fulls = [dram.tile([E_ALL, H], BF16,
                                   addr_space=("Local" if stub_collectives else "Shared"),
                                   name=f"msg_full_{k}") for k in range(n_iters)]

            # ---- constants ----
            wi_sb = cpool.tile([AF + BF, H], F32)
            nc.sync.dma_start(wi_sb[:], Wi_d[:])
            whA = cpool.tile([128, H], BF16)
            whB = cpool.tile([128, H], BF16)
            nc.sync.dma_start(whA[:], Wh_d[0:128, :])
            nc.sync.dma_start(whB[:], Wh_d[128:256, :])
            wot = cpool.tile([AF + 1, H], F32)
            nc.sync.dma_start(wot[:], WoT_d[:])
            wobA = cpool.tile([128, H], BF16)
            wobB = cpool.tile([128, H], BF16)
            nc.sync.dma_start(wobA[:], Wob_d[0:128, :])
            nc.sync.dma_start(wobB[:], Wob_d[128:256, :])
            idb = cpool.tile([128, 128], BF16)
            nc.sync.dma_start(idb[:], identb_d[:])

            lg_h1 = idxp.tile([128, lg["c1_cols"]], I16)
            nc.sync.dma_start(lg_h1[:], idx_d["lg_h1"][:])
            lg_h2 = idxp.tile([128, lg["c2_cols"]], I16)
            nc.sync.dma_start(lg_h2[:], idx_d["lg_h2"][:])

            def load_idx(nm, tab):
                h1 = idxp.tile([128, SW1], I16, tag="sw1")
                nc.sync.dma_start(h1[:, :tab["c1_cols"]], idx_d[nm + "_h1"][:])
                h2 = idxp.tile([128, SW2], I16, tag="sw2")
                nc.sync.dma_start(h2[:, :tab["c2_cols"]], idx_d[nm + "_h2"][:])
                return h1, h2

            def hop1_chunks(tab, h1, table_dram, table_rows):
                calls_by_chunk = {}
                for (t, wlo, n_pad, c1off) in tab["hop1_calls"]:
                    calls_by_chunk.setdefault(t, []).append((wlo, n_pad, c1off))
                for t in range(tab["n_chunks"]):
                    B = tab["hop1_blocks"][t]
                    ct = chp.tile([128, CHUNK_ROWS // 128, H], BF16, tag="ct")
                    o = 0
                    for (wlo, n_pad, c1off) in calls_by_chunk[t]:
                        wlen = min(WIN, table_rows - wlo)
                        nb = n_pad // 128
                        nc.gpsimd.dma_gather(
                            out_ap=ct[:, o:o + nb, :],
                            in_ap=table_dram[wlo:wlo + wlen, :],
                            idxs_ap=h1[:, c1off:c1off + n_pad // 16],
                            num_idxs=n_pad, num_idxs_reg=n_pad, elem_size=H)
                        o += nb
                    assert o == B
                    yield t, ct, B

            def hop2(h2, ct, call):
                (t, n, c2off, outoff) = call
                flat = ttp.tile([128, 2 * HOP2_MAX], BF16, tag="tt")
                ttt = flat[:, 0:2 * n].rearrange("p (k n) -> p k n", k=2)
                nc.gpsimd.dma_gather(
                    out_ap=ttt,
                    in_ap=ct[:],
                    idxs_ap=h2[:, c2off:c2off + n // 16],
                    num_idxs=n, num_idxs_reg=n, elem_size=H,
                    transpose=True,
                    sbuf_tokens_per_rank=128,
                    sbuf_free_dim_per_rank=H * 2)
                return ttt

            def reduce_rounds(ttt, c0, R):
                acc = ttt[:, :, c0:c0 + 128]
                for r in range(1, R):
                    nc.vector.tensor_tensor(
                        out=acc, in0=acc,
                        in1=ttt[:, :, c0 + r * 128:c0 + (r + 1) * 128],
                        op=AluOp.add)
                return acc

            def allgather(src, dst, shard_rows):
                if stub_collectives:
                    for rep in range(2):
                        lo = (rep * shard_rows) % max(dst.shape[0] - shard_rows, 1) \
                            if dst.shape[0] > shard_rows else 0
                        nc.sync.dma_start(dst[lo:lo + shard_rows, :], src[:])
                    return
                nc.gpsimd.collective_compute(
                    "AllGather", AluOp.bypass,
                    replica_groups=[list(range(CORES))],
                    ins=[src[:].opt()], outs=[dst[:].opt()])

            # ================= phase 0b: tree -> beta/gamma =================
            tr_h1, tr_h2 = load_idx("tr", tr)
            tr_calls = tr["hop2_calls"]
            zgb = scp.tile([128, H], BF16, tag="zgb")
            nc.vector.memset(zgb[:], 0.0)
            for (u, b) in meta["tr_zero_bins"]:
                base = u * NPS + b * 128
                nc.sync.dma_start(beta_shard[base:base + 128, :], zgb[:])
                nc.sync.dma_start(gamma_d[base:base + 128, :], zgb[:])
            tr_chunks = hop1_chunks(tr, tr_h1, tree_d, TREE_PAD)
            cur = {"t": -1, "ct": None, "tt": None, "rng": (0, 0), "ci": 0}

            def advance_to(tab, calls, h2, chunks_iter, coff):
                """Ensure the hop-2 call containing coff is current."""
                while not (cur["rng"][0] <= coff < cur["rng"][1]):
                    call = calls[cur["ci"]]
                    while cur["t"] < call[0]:
                        t_, ct_, B_ = next(chunks_iter)
                        cur["t"] = t_
                        cur["ct"] = ct_
                    cur["tt"] = hop2(h2, cur["ct"], call)
                    cur["rng"] = (call[3], call[3] + call[1])
                    cur["ci"] += 1
                return cur["tt"], cur["rng"][0]

            acc_hold = {}
            for (u, b, coff, R, first, last) in meta["tr_bins"]:
                ttt, o0 = advance_to(tr, tr_calls, tr_h2, tr_chunks, coff)
                acc = reduce_rounds(ttt, coff - o0, R)
                if not first:
                    nc.vector.tensor_tensor(out=acc_hold[(u, b)],
                                            in0=acc_hold[(u, b)], in1=acc,
                                            op=AluOp.add)
                else:
                    acc_hold[(u, b)] = acc
                if not last:
                    continue
                acc = acc_hold.pop((u, b))
                base = u * NPS + b * 128
                pb_ = ppb.tile([128, H], F32, tag="ps")
                nc.tensor.matmul(pb_[:], lhsT=acc[:, 0, :], rhs=whA[:],
                                 start=True, stop=False)
                nc.tensor.matmul(pb_[:], lhsT=acc[:, 1, :], rhs=whB[:],
                                 start=False, stop=True)
                bout = scp.tile([128, H], BF16, tag="bout")
                nc.scalar.activation(bout[:], pb_[:], Act.Copy)
                nc.sync.dma_start(beta_shard[base:base + 128, :], bout[:])
                pg_ = ppb.tile([128, H], F32, tag="ps")
                nc.tensor.matmul(pg_[:], lhsT=acc[:, 0, :], rhs=wobA[:],
                                 start=True, stop=False)
                nc.tensor.matmul(pg_[:], lhsT=acc[:, 1, :], rhs=wobB[:],
                                 start=False, stop=True)
                gt = scp.tile([128, H], BF16, tag="gt")
                nc.scalar.activation(gt[:], pg_[:], Act.Copy)
                nc.sync.dma_start(gamma_d[base:base + 128, :], gt[:])
            for _ in tr_chunks:
                pass
            allgather(beta_shard, beta_full, BPC)

            # ================= phase 0c: input2 =================
            bg_h1, bg_h2 = load_idx("bg", bg)
            bg_calls = bg["hop2_calls"]
            ci = 0
            for t, ct, B in hop1_chunks(bg, bg_h1, beta_full, B_ALL):
                while ci < len(bg_calls) and bg_calls[ci][0] == t:
                    call = bg_calls[ci]
                    (tt_, n, c2off, outoff) = call
                    ttt = hop2(bg_h2, ct, call)
                    nb = n // 128
                    for j0 in range(0, nb, 8):
                        jn = min(8, nb - j0)
                        lo = outoff + j0 * 128
                        ft = stp.tile([AF + BF, 8 * 128], F32, tag="ft")
                        nc.sync.dma_start(ft[:, :jn * 128],
                                          featT_d[:, lo:lo + jn * 128])
                        i2b = stp.tile([128, 8, H], BF16, tag="msgb")
                        for j in range(jn):
                            jj = j0 + j
                            pt = ppt.tile([128, H], BF16, tag="pt")
                            nc.tensor.transpose(
                                pt[:, 0:128], ttt[:, 0, jj * 128:(jj + 1) * 128], idb[:])
                            nc.tensor.transpose(
                                pt[:, 128:256], ttt[:, 1, jj * 128:(jj + 1) * 128], idb[:])
                            brow = scp.tile([128, H], BF16, tag="brow")
                            nc.scalar.activation(brow[:], pt[:], Act.Copy)
                            pa = ppb.tile([128, H], F32, tag="ps")
                            nc.tensor.matmul(pa[:], lhsT=ft[:, j * 128:(j + 1) * 128],
                                             rhs=wi_sb[:], start=True, stop=True)
                            nc.vector.tensor_tensor(out=i2b[:, j, :], in0=pa[:],
                                                    in1=brow[:], op=AluOp.add)
                        nc.sync.dma_start(
                            in2_d[lo:lo + jn * 128, :]
                            .rearrange("(m p) d -> p m d", p=128), i2b[:, :jn, :])
                    ci += 1

            # ================= BP iterations =================
            lg_calls = lg["hop2_calls"]
            lg_bin_list = meta["lg_bins"]   # (b, coff, R)
            nz_bins = len(lg_bin_list)
            for it in range(n_iters):
                src_full = msg_fulls[it]
                ci = 0
                for t, ct, B in hop1_chunks(lg, lg_h1, src_full, E_ALL):
                    while ci < len(lg_calls) and lg_calls[ci][0] == t:
                        call = lg_calls[ci]
                        (tt_, n, c2off, outoff) = call
                        ttt = hop2(lg_h2, ct, call)
                        bins_in = [x for x in lg_bin_list
                                   if outoff <= x[1] < outoff + n]
                        for j0 in range(0, len(bins_in), 8):
                            sub = bins_in[j0:j0 + 8]
                            b_first = sub[0][0]
                            jn = len(sub)
                            i2l = stp.tile([128, 8, H], BF16, tag="i2l")
                            nc.sync.dma_start(
                                i2l[:, :jn, :],
                                in2_d[b_first * 128:(b_first + jn) * 128, :]
                                .rearrange("(m p) d -> p m d", p=128))
                            msgb = stp.tile([128, 8, H], BF16, tag="msgb")
                            for j, (b, coff, R) in enumerate(sub):
                                assert b == b_first + j
                                acc = reduce_rounds(ttt, coff - outoff, R)
                                pb_ = ppb.tile([128, H], F32, tag="ps")
                                nc.tensor.matmul(pb_[:], lhsT=acc[:, 0, :],
                                                 rhs=whA[:], start=True, stop=False)
                                nc.tensor.matmul(pb_[:], lhsT=acc[:, 1, :],
                                                 rhs=whB[:], start=False, stop=True)
                                tmp = scp.tile([128, H], BF16, tag="tmp")
                                nc.vector.tensor_tensor(out=tmp[:], in0=pb_[:],
                                                        in1=i2l[:, j, :], op=AluOp.add)
                                nc.scalar.activation(msgb[:, j, :], tmp[:], Act.Relu)
                            nc.sync.dma_start(
                                msg_shard[b_first * 128:(b_first + jn) * 128, :]
                                .rearrange("(m p) d -> p m d", p=128), msgb[:, :jn, :])
                        ci += 1
                # zero-R tail bins: msg = relu(in2)
                for g0 in range(nz_bins, NBINS_E, 8):
                    gsz = min(8, NBINS_E - g0)
                    i2l = stp.tile([128, 8, H], BF16, tag="i2l")
                    nc.sync.dma_start(
                        i2l[:, :gsz, :],
                        in2_d[g0 * 128:(g0 + gsz) * 128, :]
                        .rearrange("(m p) d -> p m d", p=128))
                    msgb = stp.tile([128, 8, H], BF16, tag="msgb")
                    for j in range(gsz):
                        nc.scalar.activation(msgb[:, j, :], i2l[:, j, :], Act.Relu)
                    nc.sync.dma_start(
                        msg_shard[g0 * 128:(g0 + gsz) * 128, :]
                        .rearrange("(m p) d -> p m d", p=128), msgb[:, :gsz, :])
                allgather(msg_shard, msg_fulls[it], EPC)

            # ================= final =================
            m_h1, m_h2 = load_idx("m", m)
            m_calls = m["hop2_calls"]
            m_bin_map = {(u, b): (coff, R) for (u, b, coff, R) in meta["m_bins"]}
            m_tiles = {}
            for t, ct, B in hop1_chunks(m, m_h1, msg_fulls[n_iters], E_ALL):
                m_tiles[t] = ct
            ci = 0
            cur_tt = None
            cur_range = (0, 0)
            for u in range(SUP):
                pg = ppg.tile([128, H], F32, tag="pg")
                for b in range(NBINS_N):
                    base = u * NPS + b * 128
                    ftn = scp.tile([AF + 1, 128], F32, tag="ftn")
                    nc.sync.dma_start(ftn[:], featTn_d[:, base:base + 128])
                    pc_ = ppb.tile([128, H], F32, tag="ps")
                    if (u, b) in m_bin_map:
                        coff, R = m_bin_map[(u, b)]
                        if not (cur_range[0] <= coff < cur_range[1]):
                            call = m_calls[ci]
                            assert call[3] == coff, (call, coff)
                            cur_tt = hop2(m_h2, m_tiles[call[0]], call)
                            cur_range = (call[3], call[3] + call[1])
                            ci += 1
                        acc = reduce_rounds(cur_tt, coff - cur_range[0], R)
                        nc.tensor.matmul(pc_[:], lhsT=ftn[:], rhs=wot[:],
                                         start=True, stop=False)
                        nc.tensor.matmul(pc_[:], lhsT=acc[:, 0, :], rhs=wobA[:],
                                         start=False, stop=False)
                        nc.tensor.matmul(pc_[:], lhsT=acc[:, 1, :], rhs=wobB[:],
                                         start=False, stop=True)
                    else:
                        nc.tensor.matmul(pc_[:], lhsT=ftn[:], rhs=wot[:],
                                         start=True, stop=True)
                    gml = scp.tile([128, H], BF16, tag="gml")
                    nc.sync.dma_start(gml[:], gamma_d[base:base + 128, :])
                    hsum = scp.tile([128, H], F32, tag="hsum")
                    nc.vector.tensor_tensor(out=hsum[:], in0=pc_[:], in1=gml[:],
                                            op=AluOp.add)
                    h = scp.tile([128, H], F32, tag="h")
                    nc.scalar.activation(h[:], hsum[:], Act.Relu)
                    sp = scp.tile([128, 128], F32, tag="sp")
                    nc.sync.dma_start(sp[:], spool_d[base:base + 128, :])
                    nc.tensor.matmul(pg[:], lhsT=sp[:], rhs=h[:],
                                     start=(b == 0), stop=(b == NBINS_N - 1))
                go = scp.tile([128, H], F32, tag="go")
                nc.scalar.activation(go[:], pg[:], Act.Copy)
                nc.sync.dma_start(gout[u * 128:(u + 1) * 128, :], go[:])

    nc.finalize()
    return nc


def make_in_maps(inputs, per_core, meta):
    import ml_dtypes
    W_i = np.asarray(inputs["W_i"], np.float32)
    W_h = np.asarray(inputs["W_h"], np.float32)
    W_o = np.asarray(inputs["W_o"], np.float32)
    b_o = np.asarray(inputs["b_o"], np.float32)
    tree_pad = np.zeros((TREE_PAD, H), ml_dtypes.bfloat16)
    tree_pad[:60000] = np.asarray(inputs["tree_mess"], np.float32
                                  ).astype(ml_dtypes.bfloat16)
    shared = {
        "tree_bf": tree_pad,
        "Wi": W_i,
        "Wh_bf": W_h.astype(ml_dtypes.bfloat16),
        "WoTop": np.concatenate([W_o[:AF], b_o[None, :]], 0),
        "Wob_bf": W_o[AF:].astype(ml_dtypes.bfloat16),
        "ident_bf": np.eye(128).astype(ml_dtypes.bfloat16),
    }
    maps = []
    for c in range(CORES):
        pc = per_core[c]
        mp = dict(shared)
        mp["featT"] = pc["featT"]
        mp["featH1"] = pc["featH1"]
        mp["featTn"] = pc["featTn"]
        mp["spool"] = pc["spool"]
        for nm in ("lg", "bg", "tr", "m"):
            mp[nm + "_h1"] = pc[nm + "_h1"]
            mp[nm + "_h2"] = pc[nm + "_h2"]
        maps.append({k: np.ascontiguousarray(v) for k, v in mp.items()})
    return maps


_BUILD_CACHE = {}


def kernel(**inputs):
    from concourse import bass_utils
    per_core, meta = preprocess(inputs)
    key = (meta["lg"]["c1_cols"], meta["lg"]["c2_cols"], meta["m"]["c1_cols"],
           meta["bg"]["c1_cols"], meta["tr"]["c1_cols"], meta["NBINS_N"])
    nc = _BUILD_CACHE.get(key)
    if nc is None:
        nc = build(meta)
        _BUILD_CACHE[key] = nc
    in_maps = make_in_maps(inputs, per_core, meta)
    res = bass_utils.run_bass_kernel_spmd(nc, in_maps, core_ids=list(range(CORES)))
    out = np.concatenate([res.results[c]["gout"] for c in range(CORES)], axis=0)
    return out.astype(np.float32)


# revision 17
# speedup vs baseline: 1.0937x; 1.0135x over previous
"""JT-MPN GNN kernel for 8 trn2 NeuronCores (self-contained).

Two-hop dma_gather message passing: hop-1 packs needed message rows from
int16-addressable 32K-row windows of the AllGathered message table into
SBUF chunks; hop-2 re-gathers them SBUF->SBUF in consumer (bin, round,
slot) order directly in transposed (TT) layout. DVE sums rounds, PE runs
the W_h matmuls (bf16, f32 PSUM), ACT applies relu. bf16 AllGather
between the three BP iterations; graph mean-pool via a selection matmul.
"""

import numpy as np

N_NODES = 150000
N_EDGES = 300000
H = 256
DEPTH = 4
AF = 35
BF = 5
NG = 2048
CORES = 8

EPC = 37504               # edges per core (293 bins * 128)
NBINS_E = EPC // 128      # 293
GPC = NG // CORES         # 256
SUP = 2
ZR = 37500                # global msg row guaranteed zero
E_ALL = CORES * EPC       # 300032
WIN = 32768

TREE_PAD = 60416
ZT = 60000                # zero row in padded tree table

CHUNK_ROWS = 12288        # hop-1 chunk tile rows (B=96 blocks)
HOP2_MAX = 768            # transpose-mode ucode ring cap (1024 crashes)
H1_MAX = 1024
SEG_R = HOP2_MAX // 128   # max rounds per consumer segment


def _group_by(dst, n_groups):
    order = np.argsort(dst, kind="stable")
    counts = np.bincount(dst, minlength=n_groups)
    starts = np.zeros(n_groups + 1, dtype=np.int64)
    np.cumsum(counts, out=starts[1:])
    return order, starts


def wrap_idx(vals, cols):
    """[n] ints -> wrapped [128, cols] int16 (8x replicated); pad -1.
    Index i lives at [i%16, i//16]."""
    flat = np.full(cols * 16, -1, np.int16)
    flat[:len(vals)] = vals.astype(np.int16)
    w = np.ascontiguousarray(flat.reshape(cols, 16).T)
    return np.tile(w, (8, 1))


def build_2hop(src_rows, table_rows, block_sizes,
               chunk_rows=CHUNK_ROWS, hop2_max=HOP2_MAX):
    """Unified 2-hop tables for all cores.

    src_rows: [CORES, n_cons] global source row per consumer column.
    block_sizes: per atomic consumer block (bin), each multiple of 128.

    Returns dict:
      hop1_calls: list of (chunk, win_base, n_pad, col_off) ; n_valid is per
        core and encoded by -1 padding in idx (num_idxs_reg: use n_pad minus
        trailing -1 count? -> device passes per-core reg via ... ) NOTE:
        num_idxs_reg must be a compile-time constant in the unified program,
        so we pass n_pad and set padded idx entries to ZR-in-window when the
        window contains a guaranteed-zero row, else repeat the last valid
        index (harmless extra gather).
      hop1_idx: [CORES][128, C1] int16
      hop1_blocks: per chunk block count (unified)
      hop2_calls: list of (chunk, n, col_off, out_off)
      hop2_idx: [CORES][128, C2] int16
      n_chunks
    """
    n_wins = (table_rows + WIN - 1) // WIN
    nb = len(block_sizes)
    block_start = np.zeros(nb + 1, np.int64)
    np.cumsum(block_sizes, out=block_start[1:])
    n_cons = int(block_start[-1])
    assert src_rows.shape == (CORES, n_cons)

    # --- chunk assignment (unified): estimate per-core unique counts ---
    chunks = []
    cur_first = 0
    cur_rows = [set() for _ in range(CORES)]
    for b in range(nb):
        sl = slice(block_start[b], block_start[b + 1])
        newmax = 0
        for c in range(CORES):
            s = set(src_rows[c, sl].tolist())
            newmax = max(newmax, len(cur_rows[c] | s))
        if newmax > chunk_rows - 128 * n_wins and b > cur_first:
            chunks.append((cur_first, b))
            cur_first = b
            cur_rows = [set(src_rows[c, sl].tolist()) for c in range(CORES)]
        else:
            for c in range(CORES):
                cur_rows[c] |= set(src_rows[c, sl].tolist())
    chunks.append((cur_first, nb))

    hop1_calls = []
    hop1_vals = [[] for _ in range(CORES)]   # list of (colpos, array)
    hop1_blocks = []
    hop2_calls = []
    hop2_vals = [[] for _ in range(CORES)]
    c1_off = 0
    c2_off = 0
    for t, (b0, b1) in enumerate(chunks):
        sl = slice(block_start[b0], block_start[b1])
        uniqs = [np.unique(src_rows[c, sl]) for c in range(CORES)]
        poss = [np.full(len(u), -1, np.int64) for u in uniqs]
        p = 0
        for w in range(n_wins):
            wlo, whi = w * WIN, min((w + 1) * WIN, table_rows)
            sels = [(u >= wlo) & (u < whi) for u in uniqs]
            n_valid = [int(s.sum()) for s in sels]
            n_max = max(n_valid)
            if n_max == 0:
                continue
            n_pad = (n_max + 127) // 128 * 128
            for c in range(CORES):
                poss[c][sels[c]] = p + np.arange(n_valid[c])
                v = uniqs[c][sels[c]] - wlo
                if len(v) == 0:
                    v = np.array([0], np.int64)  # dummy row in window
                pad = np.full(n_pad - len(v), v[-1], np.int64)
                hop1_vals[c].append(np.concatenate([v, pad]))
            # split into sub-calls of <= H1_MAX indices (ucode ring cap)
            done = 0
            while done < n_pad:
                sub = min(H1_MAX, n_pad - done)
                hop1_calls.append((t, wlo, sub, c1_off))
                c1_off += sub // 16
                done += sub
            p += n_pad
        assert p <= chunk_rows, (p, chunk_rows)
        hop1_blocks.append(p // 128)

        h2 = []
        for c in range(CORES):
            j = np.searchsorted(uniqs[c], src_rows[c, sl])
            assert (uniqs[c][j] == src_rows[c, sl]).all()
            h2.append(poss[c][j])
            assert (poss[c][j] >= 0).all()
        # split into calls at block boundaries
        local_bs = block_start[b0:b1 + 1] - block_start[b0]
        bi = 0
        cstart = 0
        total = int(local_bs[-1])
        while cstart < total:
            cend = cstart
            while bi < b1 - b0 and local_bs[bi + 1] - cstart <= hop2_max:
                bi += 1
                cend = int(local_bs[bi])
            assert cend > cstart
            n = cend - cstart
            for c in range(CORES):
                hop2_vals[c].append(h2[c][cstart:cend])
            hop2_calls.append((t, n, c2_off, int(block_start[b0] + cstart)))
            c2_off += (n + 15) // 16
            cstart = cend

    hop1_idx = [wrap_idx(np.concatenate(hop1_vals[c]), max(c1_off, 1))
                for c in range(CORES)]
    hop2_idx = [wrap_idx(np.concatenate(hop2_vals[c]), max(c2_off, 1))
                for c in range(CORES)]
    return dict(hop1_calls=hop1_calls, hop1_idx=hop1_idx,
                hop1_blocks=hop1_blocks, hop2_calls=hop2_calls,
                hop2_idx=hop2_idx, n_chunks=len(chunks),
                c1_cols=max(c1_off, 1), c2_cols=max(c2_off, 1))


def preprocess(inputs):
    edge_src = np.asarray(inputs["edge_src"], dtype=np.int64)
    edge_dst = np.asarray(inputs["edge_dst"], dtype=np.int64)
    lg_src = np.asarray(inputs["lg_src"], dtype=np.int64)
    lg_dst = np.asarray(inputs["lg_dst"], dtype=np.int64)
    tgt_nodes = np.asarray(inputs["tgt_nodes"], dtype=np.int64)
    graph_ids = np.asarray(inputs["graph_ids"], dtype=np.int64)
    node_x = np.asarray(inputs["node_x"], dtype=np.float32)
    bond_x = np.asarray(inputs["bond_x"], dtype=np.float32)

    meta = {}

    # ---- edge -> core (snake deal by lg in-degree desc) ----
    deg = np.bincount(lg_dst, minlength=N_EDGES)
    order = np.argsort(-deg, kind="stable")
    cyc = np.arange(N_EDGES) % (2 * CORES)
    core_of_rank = np.where(cyc < CORES, cyc, 2 * CORES - 1 - cyc)
    slots = np.full((CORES, EPC), -1, dtype=np.int64)
    for c in range(CORES):
        mine = order[core_of_rank == c]
        slots[c, :len(mine)] = mine
    new_id = np.full(N_EDGES, -1, dtype=np.int64)
    for c in range(CORES):
        valid = slots[c] >= 0
        new_id[slots[c][valid]] = c * EPC + np.nonzero(valid)[0]
    assert (new_id >= 0).all()
    meta["slots"] = slots
    meta["new_id"] = new_id

    lg_order, lg_starts = _group_by(lg_dst, N_EDGES)
    slot_deg = np.where(slots >= 0, deg[np.clip(slots, 0, None)], 0)
    R_lg = slot_deg.reshape(CORES, NBINS_E, 128).max(axis=2).max(axis=0)
    meta["R_lg"] = R_lg

    # ---- nodes ----
    g_starts = np.zeros(NG + 1, dtype=np.int64)
    np.cumsum(np.bincount(graph_ids, minlength=NG), out=g_starts[1:])
    n_deg = np.bincount(edge_dst, minlength=N_NODES)
    t_cnt = np.bincount(tgt_nodes, minlength=N_NODES)
    counts_g = np.bincount(graph_ids, minlength=NG).astype(np.float64)

    sup_nodes = []
    for c in range(CORES):
        for u in range(SUP):
            g0 = c * GPC + u * 128
            nodes = np.arange(g_starts[g0], g_starts[g0 + 128])
            nodes = nodes[np.argsort(-n_deg[nodes], kind="stable")]
            sup_nodes.append(nodes)
    NBINS_N = int(max((len(x) + 127) // 128 for x in sup_nodes))
    NPS = NBINS_N * 128
    meta["NBINS_N"] = NBINS_N
    meta["NPS"] = NPS
    nslot = np.full((CORES, SUP, NPS), -1, dtype=np.int64)
    for c in range(CORES):
        for u in range(SUP):
            nodes = sup_nodes[c * SUP + u]
            nslot[c, u, :len(nodes)] = nodes
    meta["nslot"] = nslot

    BPC = SUP * NPS
    B_ALL = CORES * BPC
    beta_row_of_node = np.full(N_NODES, -1, np.int64)
    for c in range(CORES):
        sl = nslot[c].reshape(-1)
        v = sl >= 0
        beta_row_of_node[sl[v]] = c * BPC + np.nonzero(v)[0]
    assert (beta_row_of_node >= 0).all()
    meta["BPC"] = BPC
    meta["B_ALL"] = B_ALL
    pad_pos = np.nonzero(nslot[0].reshape(-1) < 0)[0]
    ZB = int(pad_pos[0]) if len(pad_pos) else 0
    meta["ZB"] = ZB

    slot_nd = np.where(nslot >= 0, n_deg[np.clip(nslot, 0, None)], 0)
    slot_nt = np.where(nslot >= 0, t_cnt[np.clip(nslot, 0, None)], 0)
    R_m = slot_nd.reshape(CORES, SUP * NBINS_N, 128).max(axis=2).max(axis=0)
    R_tn = slot_nt.reshape(CORES, SUP * NBINS_N, 128).max(axis=2).max(axis=0)
    meta["R_m"] = R_m
    meta["R_tn"] = R_tn

    e_order, e_starts = _group_by(edge_dst, N_NODES)
    t_order, t_starts = _group_by(tgt_nodes, N_NODES)

    def consumer_rows_edges(Rs, order_, starts_, src_map, zero_row, degs):
        """Build [CORES, n_cons] consumer source rows for edge bins.
        Segments of <= SEG_R rounds per bin (ucode call cap).
        bins_list entries: (bin, coloff, Rseg, first, last)."""
        bins_list = []
        blocks = []
        coff = 0
        for b in range(NBINS_E):
            R = int(Rs[b])
            if R == 0:
                continue
            r0 = 0
            while r0 < R:
                rs = min(SEG_R, R - r0)
                bins_list.append((b, coff, rs, r0 == 0, r0 + rs == R))
                blocks.append(rs * 128)
                coff += rs * 128
                r0 += rs
        n_cons = coff
        rows = np.full((CORES, n_cons), zero_row, np.int64)
        seg_round0 = {}
        r_run = {}
        for (b, co, rs, first, last) in bins_list:
            if first:
                r_run[b] = 0
            seg_round0[(b, co)] = r_run[b]
            r_run[b] += rs
        for c in range(CORES):
            for (b, co, rs, first, last) in bins_list:
                r0 = seg_round0[(b, co)]
                sl = slots[c, b*128:(b+1)*128]
                blk = np.full((rs, 128), zero_row, np.int64)
                for s in range(128):
                    e = sl[s]
                    if e < 0:
                        continue
                    d = int(degs[e])
                    lo, hi = min(r0, d), min(r0 + rs, d)
                    if hi <= lo:
                        continue
                    js = order_[starts_[e] + lo:starts_[e] + hi]
                    blk[:hi - lo, s] = src_map(js)
                rows[c, co:co + rs * 128] = blk.reshape(-1)
        return rows, blocks, bins_list

    # ---- lg ----
    rows_lg, blocks_lg, bins_lg = consumer_rows_edges(
        R_lg, lg_order, lg_starts, lambda js: new_id[lg_src[js]], ZR, deg)
    meta["lg"] = build_2hop(rows_lg, E_ALL, blocks_lg)
    meta["lg_bins"] = bins_lg
    meta["lg_zero_bins"] = [b for b in range(NBINS_E) if R_lg[b] == 0]

    # ---- beta-gather: 1 round per edge bin ----
    rows_bg = np.full((CORES, EPC), ZB, np.int64)
    for c in range(CORES):
        v = slots[c] >= 0
        rows_bg[c, v] = beta_row_of_node[edge_src[slots[c][v]]]
    meta["bg"] = build_2hop(rows_bg, B_ALL, [128] * NBINS_E)

    def consumer_rows_nodes(Rs, order_, starts_, src_map, zero_row, degs):
        """bins_list entries: (u, b, coloff, Rseg, first, last)."""
        bins_list = []
        blocks = []
        coff = 0
        for ub in range(SUP * NBINS_N):
            R = int(Rs[ub])
            if R == 0:
                continue
            r0 = 0
            while r0 < R:
                rs = min(SEG_R, R - r0)
                bins_list.append((ub // NBINS_N, ub % NBINS_N, coff, rs,
                                  r0 == 0, r0 + rs == R))
                blocks.append(rs * 128)
                coff += rs * 128
                r0 += rs
        n_cons = coff
        rows = np.full((CORES, n_cons), zero_row, np.int64)
        seg_round0 = {}
        r_run = {}
        for (u, b, co, rs, first, last) in bins_list:
            if first:
                r_run[(u, b)] = 0
            seg_round0[co] = r_run[(u, b)]
            r_run[(u, b)] += rs
        for c in range(CORES):
            for (u, b, co, rs, first, last) in bins_list:
                r0 = seg_round0[co]
                sl = nslot[c, u, b*128:(b+1)*128]
                blk = np.full((rs, 128), zero_row, np.int64)
                for s in range(128):
                    vtx = sl[s]
                    if vtx < 0:
                        continue
                    d = int(degs[vtx])
                    lo, hi = min(r0, d), min(r0 + rs, d)
                    if hi <= lo:
                        continue
                    js = order_[starts_[vtx] + lo:starts_[vtx] + hi]
                    blk[:hi - lo, s] = src_map(js)
                rows[c, co:co + rs * 128] = blk.reshape(-1)
        return rows, blocks, bins_list

    # ---- tree ----
    rows_tr, blocks_tr, bins_tr = consumer_rows_nodes(
        R_tn, t_order, t_starts, lambda js: js, ZT, t_cnt)
    meta["tr"] = build_2hop(rows_tr, TREE_PAD, blocks_tr)
    meta["tr_bins"] = bins_tr
    meta["tr_zero_bins"] = [(ub // NBINS_N, ub % NBINS_N)
                            for ub in range(SUP * NBINS_N) if R_tn[ub] == 0]

    # ---- m ----
    rows_m, blocks_m, bins_m = consumer_rows_nodes(
        R_m, e_order, e_starts, lambda js: new_id[js], ZR, n_deg)
    meta["m"] = build_2hop(rows_m, E_ALL, blocks_m)
    meta["m_bins"] = bins_m
    meta["m_zero_bins"] = [(ub // NBINS_N, ub % NBINS_N)
                           for ub in range(SUP * NBINS_N) if R_m[ub] == 0]

    # ---- per-core float layouts (pure permutations of inputs) ----
    per_core = []
    inv_cnt = (1.0 / np.maximum(counts_g, 1.0)).astype(np.float32)
    for c in range(CORES):
        pc = {}
        featT = np.zeros((AF + BF, EPC), np.float32)
        v = slots[c] >= 0
        featT[:AF, v] = node_x[edge_src[slots[c][v]]].T
        featT[AF:, v] = bond_x[slots[c][v]].T
        pc["featT"] = featT

        featTn = np.zeros((AF + 1, SUP * NPS), np.float32)
        spool = np.zeros((SUP * NPS, 128), np.float32)
        for u in range(SUP):
            sl = nslot[c, u]
            vv = sl >= 0
            base = u * NPS
            featTn[:AF, base:base + NPS][:, vv] = node_x[sl[vv]].T
            featTn[AF, base:base + NPS][vv] = 1.0
            gl = graph_ids[np.clip(sl, 0, None)] - (c * GPC + u * 128)
            idxs = np.nonzero(vv)[0]
            spool[base + idxs, gl[idxs]] = inv_cnt[graph_ids[sl[idxs]]]
        pc["featTn"] = featTn
        pc["spool"] = spool
        # features of the lg hop-1 rows (chunk-position order) for direct
        # on-device computation of the iter-1 chunk contents (skips AG0)
        i1 = meta["lg"]["hop1_idx"][c][:16].T.reshape(-1).astype(np.int64)
        rows_glob = np.zeros(len(i1), np.int64)
        pos = 0
        for (t, wlo, n_pad, c1off) in meta["lg"]["hop1_calls"]:
            iv = i1[c1off * 16: c1off * 16 + n_pad]
            rows_glob[c1off * 16: c1off * 16 + n_pad] = wlo + iv
            pos += n_pad
        # map global msg row -> original edge id (pads -> -1)
        row2edge = np.full(E_ALL, -1, np.int64)
        for cc in range(CORES):
            vv = slots[cc] >= 0
            row2edge[cc * EPC + np.nonzero(vv)[0]] = slots[cc][vv]
        eid = row2edge[np.clip(rows_glob, 0, E_ALL - 1)]
        fh1 = np.zeros((AF + BF, len(i1)), np.float32)
        ve = eid >= 0
        fh1[:AF, ve] = node_x[edge_src[eid[ve]]].T
        fh1[AF:, ve] = bond_x[eid[ve]].T
        pc["featH1"] = fh1
        pc["lg_h1"] = meta["lg"]["hop1_idx"][c]
        pc["lg_h2"] = meta["lg"]["hop2_idx"][c]
        pc["bg_h1"] = meta["bg"]["hop1_idx"][c]
        pc["bg_h2"] = meta["bg"]["hop2_idx"][c]
        pc["tr_h1"] = meta["tr"]["hop1_idx"][c]
        pc["tr_h2"] = meta["tr"]["hop2_idx"][c]
        pc["m_h1"] = meta["m"]["hop1_idx"][c]
        pc["m_h2"] = meta["m"]["hop2_idx"][c]
        per_core.append(pc)

    return per_core, meta


"""JT-MPN GNN kernel v2: 2-hop dma_gather message passing on 8 trn2 cores.

Per iteration: hop-1 window dma_gathers pack needed msg rows into SBUF
chunks (int16 indices), hop-2 SBUF-source transpose dma_gather re-reads
them in consumer (bin, round, slot) order directly in TT layout; DVE sums
rounds; PE does the W_h matmuls; DVE adds input2; ACT applies relu.
AllGather (bf16) between iterations. All float math on device.
"""
import concourse.bacc as bacc
import concourse.bass as bass
import concourse.mybir as mybir
import concourse.tile as tile

F32 = mybir.dt.float32
BF16 = mybir.dt.bfloat16
I16 = mybir.dt.int16
AluOp = mybir.AluOpType
Act = mybir.ActivationFunctionType


def build(meta, stub_collectives=False, n_iters=DEPTH - 1):
    NBINS_N = meta["NBINS_N"]
    NPS = meta["NPS"]
    BPC = meta["BPC"]
    B_ALL = meta["B_ALL"]
    lg, bg, tr, m = meta["lg"], meta["bg"], meta["tr"], meta["m"]
    SW1 = max(bg["c1_cols"], tr["c1_cols"], m["c1_cols"])
    SW2 = max(bg["c2_cols"], tr["c2_cols"], m["c2_cols"])

    nc = bacc.Bacc("TRN2", target_bir_lowering=False, debug=False)

    # ---- external IO ----
    featT_d = nc.dram_tensor("featT", [AF + BF, EPC], F32, kind="ExternalInput")
    featTn_d = nc.dram_tensor("featTn", [AF + 1, SUP * NPS], F32, kind="ExternalInput")
    spool_d = nc.dram_tensor("spool", [SUP * NPS, 128], F32, kind="ExternalInput")
    tree_d = nc.dram_tensor("tree_bf", [TREE_PAD, H], BF16, kind="ExternalInput")
    Wi_d = nc.dram_tensor("Wi", [AF + BF, H], F32, kind="ExternalInput")
    Wh_d = nc.dram_tensor("Wh_bf", [H, H], BF16, kind="ExternalInput")
    WoT_d = nc.dram_tensor("WoTop", [AF + 1, H], F32, kind="ExternalInput")
    Wob_d = nc.dram_tensor("Wob_bf", [H, H], BF16, kind="ExternalInput")
    identb_d = nc.dram_tensor("ident_bf", [128, 128], BF16, kind="ExternalInput")
    FH1 = 16 * lg["c1_cols"]
    featH1_d = nc.dram_tensor("featH1", [AF + BF, FH1], F32, kind="ExternalInput")
    idx_d = {}
    for nm, tab in (("lg", lg), ("bg", bg), ("tr", tr), ("m", m)):
        idx_d[nm + "_h1"] = nc.dram_tensor(nm + "_h1", [128, tab["c1_cols"]], I16,
                                           kind="ExternalInput")
        idx_d[nm + "_h2"] = nc.dram_tensor(nm + "_h2", [128, tab["c2_cols"]], I16,
                                           kind="ExternalInput")
    gout = nc.dram_tensor("gout", [GPC, H], F32, kind="ExternalOutput")

    with tile.TileContext(nc) as tc:
        with tc.tile_pool(name="dram", bufs=1, space="DRAM") as dram, \
             tc.tile_pool(name="const", bufs=1) as cpool, \
             tc.tile_pool(name="idxp", bufs=1) as idxp, \
             tc.tile_pool(name="chunk", bufs=2) as chp, \
             tc.tile_pool(name="tt", bufs=2) as ttp, \
             tc.tile_pool(name="stream", bufs=3) as stp, \
             tc.tile_pool(name="scratch", bufs=3) as scp, \
             tc.tile_pool(name="psum_b", bufs=4, space="PSUM") as ppb, \
             tc.tile_pool(name="psum_t", bufs=2, space="PSUM") as ppt, \
             tc.tile_pool(name="psum_g", bufs=1, space="PSUM") as ppg:

            beta_shard = dram.tile([BPC, H], BF16)
            beta_full = dram.tile([B_ALL, H], BF16,
                                  addr_space=("Local" if stub_collectives else "Shared"))
            in2_d = dram.tile([EPC, H], BF16)
            gamma_d = dram.tile([SUP * NPS, H], BF16)
            msg_shard = dram.tile([EPC, H], BF16)
            msg_fulls = [dram.tile([E_ALL, H], BF16,
                                   addr_space=("Local" if stub_collectives else "Shared"),
                                   name=f"msg_full_{k}") for k in range(n_iters)]

            # ---- constants ----
            wi_sb = cpool.tile([AF + BF, H], F32)
            nc.sync.dma_start(wi_sb[:], Wi_d[:])
            whA = cpool.tile([128, H], BF16)
            whB = cpool.tile([128, H], BF16)
            nc.sync.dma_start(whA[:], Wh_d[0:128, :])
            nc.sync.dma_start(whB[:], Wh_d[128:256, :])
            wot = cpool.tile([AF + 1, H], F32)
            nc.sync.dma_start(wot[:], WoT_d[:])
            wobA = cpool.tile([128, H], BF16)
            wobB = cpool.tile([128, H], BF16)
            nc.sync.dma_start(wobA[:], Wob_d[0:128, :])
            nc.sync.dma_start(wobB[:], Wob_d[128:256, :])
            idb = cpool.tile([128, 128], BF16)
            nc.sync.dma_start(idb[:], identb_d[:])

            lg_h1 = idxp.tile([128, lg["c1_cols"]], I16)
            nc.sync.dma_start(lg_h1[:], idx_d["lg_h1"][:])
            lg_h2 = idxp.tile([128, lg["c2_cols"]], I16)
            nc.sync.dma_start(lg_h2[:], idx_d["lg_h2"][:])

            def load_idx(nm, tab):
                h1 = idxp.tile([128, SW1], I16, tag="sw1")
                nc.sync.dma_start(h1[:, :tab["c1_cols"]], idx_d[nm + "_h1"][:])
                h2 = idxp.tile([128, SW2], I16, tag="sw2")
                nc.sync.dma_start(h2[:, :tab["c2_cols"]], idx_d[nm + "_h2"][:])
                return h1, h2

            def hop1_chunks(tab, h1, table_dram, table_rows):
                calls_by_chunk = {}
                for (t, wlo, n_pad, c1off) in tab["hop1_calls"]:
                    calls_by_chunk.setdefault(t, []).append((wlo, n_pad, c1off))
                for t in range(tab["n_chunks"]):
                    B = tab["hop1_blocks"][t]
                    ct = chp.tile([128, CHUNK_ROWS // 128, H], BF16, tag="ct")
                    o = 0
                    for (wlo, n_pad, c1off) in calls_by_chunk[t]:
                        wlen = min(WIN, table_rows - wlo)
                        nb = n_pad // 128
                        nc.gpsimd.dma_gather(
                            out_ap=ct[:, o:o + nb, :],
                            in_ap=table_dram[wlo:wlo + wlen, :],
                            idxs_ap=h1[:, c1off:c1off + n_pad // 16],
                            num_idxs=n_pad, num_idxs_reg=n_pad, elem_size=H)
                        o += nb
                    assert o == B
                    yield t, ct, B

            def hop2(h2, ct, call):
                (t, n, c2off, outoff) = call
                flat = ttp.tile([128, 2 * HOP2_MAX], BF16, tag="tt")
                ttt = flat[:, 0:2 * n].rearrange("p (k n) -> p k n", k=2)
                nc.gpsimd.dma_gather(
                    out_ap=ttt,
                    in_ap=ct[:],
                    idxs_ap=h2[:, c2off:c2off + n // 16],
                    num_idxs=n, num_idxs_reg=n, elem_size=H,
                    transpose=True,
                    sbuf_tokens_per_rank=128,
                    sbuf_free_dim_per_rank=H * 2)
                return ttt

            def reduce_rounds(ttt, c0, R):
                acc = ttt[:, :, c0:c0 + 128]
                for r in range(1, R):
                    nc.vector.tensor_tensor(
                        out=acc, in0=acc,
                        in1=ttt[:, :, c0 + r * 128:c0 + (r + 1) * 128],
                        op=AluOp.add)
                return acc

            def allgather(src, dst, shard_rows):
                if stub_collectives:
                    for rep in range(2):
                        lo = (rep * shard_rows) % max(dst.shape[0] - shard_rows, 1) \
                            if dst.shape[0] > shard_rows else 0
                        nc.sync.dma_start(dst[lo:lo + shard_rows, :], src[:])
                    return
                nc.gpsimd.collective_compute(
                    "AllGather", AluOp.bypass,
                    replica_groups=[list(range(CORES))],
                    ins=[src[:].opt()], outs=[dst[:].opt()])

            # ================= phase 0b: tree -> beta/gamma =================
            tr_h1, tr_h2 = load_idx("tr", tr)
            tr_calls = tr["hop2_calls"]
            zgb = scp.tile([128, H], BF16, tag="zgb")
            nc.vector.memset(zgb[:], 0.0)
            for (u, b) in meta["tr_zero_bins"]:
                base = u * NPS + b * 128
                nc.sync.dma_start(beta_shard[base:base + 128, :], zgb[:])
                nc.sync.dma_start(gamma_d[base:base + 128, :], zgb[:])
            tr_chunks = hop1_chunks(tr, tr_h1, tree_d, TREE_PAD)
            cur = {"t": -1, "ct": None, "tt": None, "rng": (0, 0), "ci": 0}

            def advance_to(tab, calls, h2, chunks_iter, coff):
                """Ensure the hop-2 call containing coff is current."""
                while not (cur["rng"][0] <= coff < cur["rng"][1]):
                    call = calls[cur["ci"]]
                    while cur["t"] < call[0]:
                        t_, ct_, B_ = next(chunks_iter)
                        cur["t"] = t_
                        cur["ct"] = ct_
                    cur["tt"] = hop2(h2, cur["ct"], call)
                    cur["rng"] = (call[3], call[3] + call[1])
                    cur["ci"] += 1
                return cur["tt"], cur["rng"][0]

            acc_hold = {}
            for (u, b, coff, R, first, last) in meta["tr_bins"]:
                ttt, o0 = advance_to(tr, tr_calls, tr_h2, tr_chunks, coff)
                acc = reduce_rounds(ttt, coff - o0, R)
                if not first:
                    nc.vector.tensor_tensor(out=acc_hold[(u, b)],
                                            in0=acc_hold[(u, b)], in1=acc,
                                            op=AluOp.add)
                else:
                    acc_hold[(u, b)] = acc
                if not last:
                    continue
                acc = acc_hold.pop((u, b))
                base = u * NPS + b * 128
                pb_ = ppb.tile([128, H], F32, tag="ps")
                nc.tensor.matmul(pb_[:], lhsT=acc[:, 0, :], rhs=whA[:],
                                 start=True, stop=False)
                nc.tensor.matmul(pb_[:], lhsT=acc[:, 1, :], rhs=whB[:],
                                 start=False, stop=True)
                bout = scp.tile([128, H], BF16, tag="bout")
                nc.scalar.activation(bout[:], pb_[:], Act.Copy)
                nc.sync.dma_start(beta_shard[base:base + 128, :], bout[:])
                pg_ = ppb.tile([128, H], F32, tag="ps")
                nc.tensor.matmul(pg_[:], lhsT=acc[:, 0, :], rhs=wobA[:],
                                 start=True, stop=False)
                nc.tensor.matmul(pg_[:], lhsT=acc[:, 1, :], rhs=wobB[:],
                                 start=False, stop=True)
                gt = scp.tile([128, H], BF16, tag="gt")
                nc.scalar.activation(gt[:], pg_[:], Act.Copy)
                nc.sync.dma_start(gamma_d[base:base + 128, :], gt[:])
            for _ in tr_chunks:
                pass
            allgather(beta_shard, beta_full, BPC)

            # ================= phase 0c: input2 =================
            bg_h1, bg_h2 = load_idx("bg", bg)
            bg_calls = bg["hop2_calls"]
            ci = 0
            for t, ct, B in hop1_chunks(bg, bg_h1, beta_full, B_ALL):
                while ci < len(bg_calls) and bg_calls[ci][0] == t:
                    call = bg_calls[ci]
                    (tt_, n, c2off, outoff) = call
                    ttt = hop2(bg_h2, ct, call)
                    nb = n // 128
                    for j0 in range(0, nb, 8):
                        jn = min(8, nb - j0)
                        lo = outoff + j0 * 128
                        ft = stp.tile([AF + BF, 8 * 128], F32, tag="ft")
                        nc.sync.dma_start(ft[:, :jn * 128],
                                          featT_d[:, lo:lo + jn * 128])
                        i2b = stp.tile([128, 8, H], BF16, tag="msgb")
                        for j in range(jn):
                            jj = j0 + j
                            pt = ppt.tile([128, H], BF16, tag="pt")
                            nc.tensor.transpose(
                                pt[:, 0:128], ttt[:, 0, jj * 128:(jj + 1) * 128], idb[:])
                            nc.tensor.transpose(
                                pt[:, 128:256], ttt[:, 1, jj * 128:(jj + 1) * 128], idb[:])
                            brow = scp.tile([128, H], BF16, tag="brow")
                            nc.scalar.activation(brow[:], pt[:], Act.Copy)
                            pa = ppb.tile([128, H], F32, tag="ps")
                            nc.tensor.matmul(pa[:], lhsT=ft[:, j * 128:(j + 1) * 128],
                                             rhs=wi_sb[:], start=True, stop=True)
                            nc.vector.tensor_tensor(out=i2b[:, j, :], in0=pa[:],
                                                    in1=brow[:], op=AluOp.add)
                        nc.sync.dma_start(
                            in2_d[lo:lo + jn * 128, :]
                            .rearrange("(m p) d -> p m d", p=128), i2b[:, :jn, :])
                    ci += 1

            # ================= BP iterations =================
            lg_calls = lg["hop2_calls"]
            lg_bin_list = meta["lg_bins"]   # (b, coff, R)
            nz_bins = len(lg_bin_list)
            for it in range(n_iters):
                src_full = msg_fulls[it]
                ci = 0
                for t, ct, B in hop1_chunks(lg, lg_h1, src_full, E_ALL):
                    while ci < len(lg_calls) and lg_calls[ci][0] == t:
                        call = lg_calls[ci]
                        (tt_, n, c2off, outoff) = call
                        ttt = hop2(lg_h2, ct, call)
                        bins_in = [x for x in lg_bin_list
                                   if outoff <= x[1] < outoff + n]
                        for j0 in range(0, len(bins_in), 8):
                            sub = bins_in[j0:j0 + 8]
                            b_first = sub[0][0]
                            jn = len(sub)
                            i2l = stp.tile([128, 8, H], BF16, tag="i2l")
                            nc.sync.dma_start(
                                i2l[:, :jn, :],
                                in2_d[b_first * 128:(b_first + jn) * 128, :]
                                .rearrange("(m p) d -> p m d", p=128))
                            msgb = stp.tile([128, 8, H], BF16, tag="msgb")
                            for j, (b, coff, R) in enumerate(sub):
                                assert b == b_first + j
                                acc = reduce_rounds(ttt, coff - outoff, R)
                                pb_ = ppb.tile([128, H], F32, tag="ps")
                                nc.tensor.matmul(pb_[:], lhsT=acc[:, 0, :],
                                                 rhs=whA[:], start=True, stop=False)
                                nc.tensor.matmul(pb_[:], lhsT=acc[:, 1, :],
                                                 rhs=whB[:], start=False, stop=True)
                                tmp = scp.tile([128, H], BF16, tag="tmp")
                                nc.vector.tensor_tensor(out=tmp[:], in0=pb_[:],
                                                        in1=i2l[:, j, :], op=AluOp.add)
                                nc.scalar.activation(msgb[:, j, :], tmp[:], Act.Relu)
                            nc.sync.dma_start(
                                msg_shard[b_first * 128:(b_first + jn) * 128, :]
                                .rearrange("(m p) d -> p m d", p=128), msgb[:, :jn, :])
                        ci += 1
                # zero-R tail bins: msg = relu(in2)
                for g0 in range(nz_bins, NBINS_E, 8):
                    gsz = min(8, NBINS_E - g0)
                    i2l = stp.tile([128, 8, H], BF16, tag="i2l")
                    nc.sync.dma_start(
                        i2l[:, :gsz, :],
                        in2_d[g0 * 128:(g0 + gsz) * 128, :]
                        .rearrange("(m p) d -> p m d", p=128))
                    msgb = stp.tile([128, 8, H], BF16, tag="msgb")
                    for j in range(gsz):
                        nc.scalar.activation(msgb[:, j, :], i2l[:, j, :], Act.Relu)
                    nc.sync.dma_start(
                        msg_shard[g0 * 128:(g0 + gsz) * 128, :]
                        .rearrange("(m p) d -> p m d", p=128), msgb[:, :gsz, :])
                allgather(msg_shard, msg_fulls[it], EPC)

            # ================= final =================
            m_h1, m_h2 = load_idx("m", m)
            m_calls = m["hop2_calls"]
            m_bin_map = {(u, b): (coff, R) for (u, b, coff, R) in meta["m_bins"]}
            m_tiles = {}
            for t, ct, B in hop1_chunks(m, m_h1, msg_fulls[n_iters], E_ALL):
                m_tiles[t] = ct
            ci = 0
            cur_tt = None
            cur_range = (0, 0)
            for u in range(SUP):
                pg = ppg.tile([128, H], F32, tag="pg")
                for b in range(NBINS_N):
                    base = u * NPS + b * 128
                    ftn = scp.tile([AF + 1, 128], F32, tag="ftn")
                    nc.sync.dma_start(ftn[:], featTn_d[:, base:base + 128])
                    pc_ = ppb.tile([128, H], F32, tag="ps")
                    if (u, b) in m_bin_map:
                        coff, R = m_bin_map[(u, b)]
                        if not (cur_range[0] <= coff < cur_range[1]):
                            call = m_calls[ci]
                            assert call[3] == coff, (call, coff)
                            cur_tt = hop2(m_h2, m_tiles[call[0]], call)
                            cur_range = (call[3], call[3] + call[1])
                            ci += 1
                        acc = reduce_rounds(cur_tt, coff - cur_range[0], R)
                        nc.tensor.matmul(pc_[:], lhsT=ftn[:], rhs=wot[:],
                                         start=True, stop=False)
                        nc.tensor.matmul(pc_[:], lhsT=acc[:, 0, :], rhs=wobA[:],
                                         start=False, stop=False)
                        nc.tensor.matmul(pc_[:], lhsT=acc[:, 1, :], rhs=wobB[:],
                                         start=False, stop=True)
                    else:
                        nc.tensor.matmul(pc_[:], lhsT=ftn[:], rhs=wot[:],
                                         start=True, stop=True)
                    gml = scp.tile([128, H], BF16, tag="gml")
                    nc.sync.dma_start(gml[:], gamma_d[base:base + 128, :])
                    hsum = scp.tile([128, H], F32, tag="hsum")
                    nc.vector.tensor_tensor(out=hsum[:], in0=pc_[:], in1=gml[:],
                                            op=AluOp.add)
                    h = scp.tile([128, H], F32, tag="h")
                    nc.scalar.activation(h[:], hsum[:], Act.Relu)
                    sp = scp.tile([128, 128], F32, tag="sp")
                    nc.sync.dma_start(sp[:], spool_d[base:base + 128, :])
                    nc.tensor.matmul(pg[:], lhsT=sp[:], rhs=h[:],
                                     start=(b == 0), stop=(b == NBINS_N - 1))
                go = scp.tile([128, H], F32, tag="go")
                nc.scalar.activation(go[:], pg[:], Act.Copy)
                nc.sync.dma_start(gout[u * 128:(u + 1) * 128, :], go[:])

    nc.finalize()
    return nc


def make_in_maps(inputs, per_core, meta):
    import ml_dtypes
    W_i = np.asarray(inputs["W_i"], np.float32)
    W_h = np.asarray(inputs["W_h"], np.float32)
    W_o = np.asarray(inputs["W_o"], np.float32)
    b_o = np.asarray(inputs["b_o"], np.float32)
    tree_pad = np.zeros((TREE_PAD, H), ml_dtypes.bfloat16)
    tree_pad[:60000] = np.asarray(inputs["tree_mess"], np.float32
                                  ).astype(ml_dtypes.bfloat16)
    shared = {
        "tree_bf": tree_pad,
        "Wi": W_i,
        "Wh_bf": W_h.astype(ml_dtypes.bfloat16),
        "WoTop": np.concatenate([W_o[:AF], b_o[None, :]], 0),
        "Wob_bf": W_o[AF:].astype(ml_dtypes.bfloat16),
        "ident_bf": np.eye(128).astype(ml_dtypes.bfloat16),
    }
    maps = []
    for c in range(CORES):
        pc = per_core[c]
        mp = dict(shared)
        mp["featT"] = pc["featT"]
        mp["featH1"] = pc["featH1"]
        mp["featTn"] = pc["featTn"]
        mp["spool"] = pc["spool"]
        for nm in ("lg", "bg", "tr", "m"):
            mp[nm + "_h1"] = pc[nm + "_h1"]
            mp[nm + "_h2"] = pc[nm + "_h2"]
        maps.append({k: np.ascontiguousarray(v) for k, v in mp.items()})
    return maps


_BUILD_CACHE = {}


def kernel(**inputs):
    from concourse import bass_utils
    per_core, meta = preprocess(inputs)
    key = (meta["lg"]["c1_cols"], meta["lg"]["c2_cols"], meta["m"]["c1_cols"],
           meta["bg"]["c1_cols"], meta["tr"]["c1_cols"], meta["NBINS_N"])
    nc = _BUILD_CACHE.get(key)
    if nc is None:
        nc = build(meta)
        _BUILD_CACHE[key] = nc
    in_maps = make_in_maps(inputs, per_core, meta)
    res = bass_utils.run_bass_kernel_spmd(nc, in_maps, core_ids=list(range(CORES)))
    out = np.concatenate([res.results[c]["gout"] for c in range(CORES)], axis=0)
    return out.astype(np.float32)
